# revision 12
# baseline (speedup 1.0000x reference)
"""Trainium2 Bass kernel for nn_L1Wav: 5-level 3D db4 wavelet soft-threshold
denoising of a 256^3 complex volume, SPMD over 8 NeuronCores.

Math notes (verified against the jax reference in a numpy sim):
  - The deterministic rng(1000) shift is 0 and the unit-modulus phase cancels
    through the prox (DWT is real-linear; |phase*w| = |w|), so the computation
    is exactly: 5-level 3D DWT -> complex soft-threshold -> inverse DWT.
  - Every 1D DWT/IDWT pass is a matmul against a banded filter matrix.
  - Sharding: volume split along axis 0 (32 planes/core). All a-axis passes
    use per-core weight-matrix slices, so the core-dependence lives entirely
    in host-provided matrices and one SPMD program serves all cores.
    Levels 1-2 are distributed; levels 3-5 are replicated on every core.
    The only communication is two small AllGathers of approx bands.

Level sizes: 256 -> 131 -> 69 -> 38 -> 22 -> 14.
Per-core windows: L1 band rows [16c,16c+19); L2 band rows [8c,8c+13);
output rows [32c,32c+32); input slab rows [32c-6,32c+38) zero-padded.

Layout: a volume at any level is stored (p, q, r). The forward a-pass
contracts p; the per-row bc-pass transforms q then r, emitting tiles
(r', q'), so child band tensors are stored (a_row, r', q').
"""
import sys
from contextlib import ExitStack

import numpy as np

sys.path.insert(0, "/opt/trn_rl_repo")

import concourse.bass as bass
import concourse.mybir as mybir
import concourse.tile as tile
from concourse import bacc
from concourse.bass_utils import run_bass_kernel_spmd
from concourse.masks import make_identity

DT = mybir.dt.float32
R32 = mybir.dt.float32r
F = 8
DEC_LO = np.array([-0.010597401784997278, 0.032883011666982945, 0.030841381835986965,
                   -0.18703481171888114, -0.02798376941698385, 0.6308807679295904,
                   0.7148465705525415, 0.23037781330885523])
REC_LO = DEC_LO[::-1].copy()
REC_HI = np.array([((-1) ** n) * DEC_LO[n] for n in range(F)])
DEC_HI = REC_HI[::-1].copy()

NS = [256, 131, 69, 38, 22, 14]     # sizes level 0..5
NCORE = 8
COMPS = ("re", "im")
BC_BS = {1: 1, 2: 3, 3: 6, 4: 11, 5: 14}       # fwd bc row batch
IBC_BS = {1: 2, 2: 5, 3: 7, 4: 13, 5: 14}      # inv bc row batch
INV_OUT_ROWS = {1: 32, 2: 19, 3: 69, 4: 38, 5: 22}


def W_mat(N, flt):
    L = (N + F - 1) // 2
    W = np.zeros((L, N), dtype=np.float32)
    for l in range(L):
        for j in range(F):
            n = 2 * l + 1 - j
            if 0 <= n < N:
                W[l, n] = flt[j]
    return W


def G_mat(L, crop, flt):
    G = np.zeros((crop, L), dtype=np.float32)
    for t in range(crop):
        for m in range(L):
            j = t + 6 - 2 * m
            if 0 <= j < F:
                G[t, m] = flt[j]
    return G


def host_matrices(core):
    """All weight matrices for one core (lhsT layout: (K, M))."""
    c = core
    m = {}
    for l in range(5):
        W2 = np.concatenate([W_mat(NS[l], DEC_LO), W_mat(NS[l], DEC_HI)], 0)
        m[f"WT{l + 1}"] = np.ascontiguousarray(W2.T)
        glo = G_mat(NS[l + 1], NS[l], REC_LO)
        ghi = G_mat(NS[l + 1], NS[l], REC_HI)
        m[f"IAB{l + 1}"] = np.ascontiguousarray(
            np.concatenate([glo.T, ghi.T], 0))
    # L1 fwd a-pass (per-core): A1 (38, 44) -> lhsT (44, 38)
    A1 = np.zeros((38, 44), dtype=np.float32)
    slab_lo = 32 * c - 6
    for half, flt in ((0, DEC_LO), (1, DEC_HI)):
        for i in range(19):
            l = 16 * c + i
            for k in range(44):
                n = slab_lo + k
                j = 2 * l + 1 - n
                if 0 <= j < F and 0 <= n < 256:
                    A1[half * 19 + i, k] = flt[j]
    m["A1T"] = np.ascontiguousarray(A1.T)
    # L2 fwd a-pass, merged with the replicated full-lo pass, contracting
    # directly over ag1_out rows (38k+19ci+i = VA1full row 16k+i, owner
    # k = min(row//16, 7)).  M = 26 per-core band rows + 69 full-lo rows.
    A2 = np.concatenate([W_mat(131, DEC_LO)[8 * c:8 * c + 13],
                         W_mat(131, DEC_HI)[8 * c:8 * c + 13]], 0)
    M95 = np.concatenate([A2, W_mat(131, DEC_LO)], 0)       # (95, 131)
    for ci in range(2):
        A2WT = np.zeros((304, 95), dtype=np.float32)
        for r in range(131):
            k = min(r // 16, 7)
            A2WT[38 * k + 19 * ci + (r - 16 * k), :] = M95[:, r]
        m["A2WTre" if ci == 0 else "A2WTim"] = A2WT
    # L1 inv a-pass: core-independent (38, 32)
    G1a = np.zeros((32, 19), dtype=np.float32)
    G1d = np.zeros((32, 19), dtype=np.float32)
    for u in range(32):
        for v in range(19):
            j = u + 6 - 2 * v
            if 0 <= j < F:
                G1a[u, v] = REC_LO[j]
                G1d[u, v] = REC_HI[j]
    m["IA1"] = np.ascontiguousarray(np.concatenate([G1a.T, G1d.T], 0))
    # L2 inv a-pass (per-core)
    glo1 = G_mat(69, 131, REC_LO)
    ghi1 = G_mat(69, 131, REC_HI)
    g2a_full = glo1[16 * c:16 * c + 19, :]                    # (19, 69)
    g2a13 = glo1[16 * c:16 * c + 19, 8 * c:8 * c + 13]
    g2d13 = ghi1[16 * c:16 * c + 19, 8 * c:8 * c + 13]
    m["IA2"] = np.ascontiguousarray(np.concatenate([g2a13.T, g2d13.T], 0))
    m["IA2LL"] = np.ascontiguousarray(np.concatenate([g2a_full.T, g2d13.T], 0))
    return {k: v.astype(np.float32) for k, v in m.items()}


MAT_SHAPES = {k: v.shape for k, v in host_matrices(0).items()}
# partition-chunk splits for SBUF-resident matrices (K dim)
MAT_SPLITS = {
    "IAB1": [(0, 128), (128, 3), (131, 128), (259, 3)],
    "IAB2": [(0, 69), (69, 69)],
    "IAB3": [(0, 38), (38, 38)],
    "IAB4": [(0, 22), (22, 22)],
    "IAB5": [(0, 14), (14, 14)],
}


def chunks_of(total, size=128):
    return [(i, min(size, total - i)) for i in range(0, total, size)]


class Builder:
    def __init__(self, nc, tc, ctx, thresh):
        self.nc = nc
        self.tc = tc
        self.thresh = float(thresh)
        self.p_dram = ctx.enter_context(
            tc.tile_pool(name="dram", bufs=1, space=bass.MemorySpace.DRAM))
        self.p_wts = ctx.enter_context(tc.tile_pool(name="wts", bufs=1))
        self.p_work = ctx.enter_context(tc.tile_pool(name="work", bufs=1))
        self.p_psum = ctx.enter_context(
            tc.tile_pool(name="psum", bufs=1, space=bass.MemorySpace.PSUM))
        self.mats = {}
        self.dram = {}
        self.uid = 0

    def _id(self):
        self.uid += 1
        return self.uid

    def dram_tile(self, name, shape, addr_space="Local"):
        t = self.p_dram.tile(list(shape), DT, name=name, tag=name,
                             addr_space=addr_space)
        self.dram[name] = t
        return t

    def sbuf(self, shape, tag, bufs=1):
        return self.p_work.tile(list(shape), DT, name=f"t{self._id()}",
                                tag=tag, bufs=bufs)

    def psum(self, shape, tag):
        return self.p_psum.tile(list(shape), DT, name=f"p{self._id()}",
                                tag=tag, bufs=1)

    # fp32r matmul: 1 cycle/row (vs 4 for fp32) when moving free size >= 256
    def mm(self, out, lhsT, rhs, **kw):
        self.nc.tensor.matmul(out, lhsT.bitcast(R32), rhs.bitcast(R32), **kw)

    def tp(self, out, in_, ident):
        self.nc.tensor.matmul(out.bitcast(R32), in_.bitcast(R32),
                              ident.bitcast(R32), is_transpose=True)

    def load_mat(self, name, dram_ap, splits=None):
        K, M = dram_ap.shape
        if splits is None:
            splits = MAT_SPLITS.get(name, chunks_of(K))
        tiles = []
        for (k0, kn) in splits:
            t = self.p_wts.tile([kn, M], DT, name=f"{name}_{k0}",
                                tag=f"{name}_{k0}", bufs=1)
            self.nc.sync.dma_start(t[:, :], dram_ap[k0:k0 + kn, :])
            tiles.append((t, k0, kn))
        self.mats[name] = tiles

    # ---- soft threshold: returns thresholded (re, im) tiles (full-shape)
    def soft_pair(self, s_re, s_im, shape, gb):
        nc = self.nc
        t = self.thresh
        mn = shape[0]
        tmp1 = self.sbuf(shape, "sm1")
        tmp2 = self.sbuf(shape, "sm2")
        a = tmp1[:, :gb, :]
        m = tmp2[:, :gb, :]
        nc.vector.tensor_mul(a, s_re, s_re)
        nc.vector.tensor_mul(m, s_im, s_im)
        nc.vector.tensor_add(a, a, m)
        nc.scalar.activation(m, a, mybir.ActivationFunctionType.Sqrt,
                             bias=self.bias_eps[:mn, :])
        nc.vector.tensor_scalar(a, m, -t, 0.0,
                                mybir.AluOpType.add, mybir.AluOpType.max)
        nc.vector.reciprocal(m, m)
        nc.vector.tensor_mul(a, a, m)
        th_re = self.sbuf(shape, "str", bufs=2)
        th_im = self.sbuf(shape, "sti", bufs=2)
        nc.vector.tensor_mul(th_re[:, :gb, :], s_re, a)
        nc.vector.tensor_mul(th_im[:, :gb, :], s_im, a)
        return th_re, th_im

    # ---- forward a-pass: out (M, n, n) = lhsT^T @ in (K, n, n)
    def fwd_a(self, lname, in_keys, out_keys, M, n, ntile=512):
        nc = self.nc
        lhsT = self.mats[lname]
        for comp in COMPS:
            srcf = self.dram[in_keys[comp]].rearrange("a b c -> a (b c)")
            dstf = self.dram[out_keys[comp]].rearrange("a b c -> a (b c)")
            tot = n * n
            for t0 in range(0, tot, ntile):
                tn = min(ntile, tot - t0)
                rts = []
                for i, (lt, k0, kn) in enumerate(lhsT):
                    rt = self.sbuf([kn, ntile], f"fa_in_{i}", bufs=3)
                    nc.sync.dma_start(rt[:, :tn], srcf[k0:k0 + kn, t0:t0 + tn])
                    rts.append(rt)
                p = self.psum([M, ntile], "P0")
                for i, (lt, k0, kn) in enumerate(lhsT):
                    self.mm(p[:, :tn], lt[:, :], rts[i][:, :tn],
                            start=(i == 0), stop=(i == len(lhsT) - 1))
                s = self.sbuf([M, ntile], "fa_o", bufs=3)
                nc.scalar.copy(s[:, :tn], p[:, :tn])
                nc.sync.dma_start(dstf[:, t0:t0 + tn], s[:, :tn])

    # ---- merged L2 a-pass: contract ag1_out rows directly; outputs both
    # the per-core 26 band rows and the replicated 69 full-lo rows.
    def fwd_a2_merged(self, ag_out, ntile=512):
        nc = self.nc
        lhs = {c: self.mats[f"A2WT{c}"] for c in COMPS}
        src = ag_out.rearrange("a b c -> a (b c)")
        dsts = {c: self.dram[f"Af2C{c}"].rearrange("a b c -> a (b c)")
                for c in COMPS}
        tot = 131 * 131
        for t0 in range(0, tot, ntile):
            tn = min(ntile, tot - t0)
            rts = []
            for i, (lt, k0, kn) in enumerate(lhs["re"]):
                rt = self.sbuf([kn, ntile], f"fa2_in_{i}", bufs=2)
                nc.sync.dma_start(rt[:, :tn], src[k0:k0 + kn, t0:t0 + tn])
                rts.append(rt)
            for pi, comp in enumerate(COMPS):
                lT = lhs[comp]
                p = self.psum([95, ntile], f"P{pi}")
                for i, (lt, k0, kn) in enumerate(lT):
                    self.mm(p[:, :tn], lt[:, :], rts[i][:, :tn],
                            start=(i == 0), stop=(i == len(lT) - 1))
                s = self.sbuf([95, ntile], f"fa2_o_{comp}", bufs=3)
                nc.scalar.copy(s[:, :tn], p[:, :tn])
                nc.sync.dma_start(dsts[comp][:, t0:t0 + tn], s[:, :tn])

    # ---- forward bc-pass for one level
    def bc_fwd(self, lvl, rows, band_dest):
        nc = self.nc
        bs = BC_BS[lvl]
        Q = NS[lvl - 1]
        L = NS[lvl]
        twoL = 2 * L
        WT = self.mats[f"WT{lvl}"]
        qch = chunks_of(Q)
        mch = chunks_of(twoL)
        for g0 in range(0, rows, bs):
            gb = min(bs, rows - g0)
            S3 = {}
            for comp in COMPS:
                src = self.dram[f"Af{lvl}{comp}"]
                ins = []
                for qi, (q0, qn) in enumerate(qch):
                    it = self.sbuf([qn, bs, Q], f"bci_{qi}", bufs=2)
                    sap = src[g0:g0 + gb, q0:q0 + qn, :].rearrange(
                        "b q n -> q b n")
                    nc.sync.dma_start(it[:, :gb, :], sap)
                    ins.append(it)
                # M1: transform q -> (twoL chunks, gb, Q)
                s1 = []
                for mi, (m0, mn) in enumerate(mch):
                    p = self.psum([mn, bs, Q], f"P{mi}")
                    for ki in range(len(qch)):
                        self.mm(p[:, :gb, :],
                                WT[ki][0][:, m0:m0 + mn],
                                ins[ki][:, :gb, :],
                                start=(ki == 0),
                                stop=(ki == len(qch) - 1))
                    s = self.sbuf([mn, bs, Q], f"bs1_{mi}")
                    nc.scalar.copy(s[:, :gb, :], p[:, :gb, :])
                    s1.append(s)
                # transpose -> (Q chunks, gb, twoL)
                pT = [self.psum([fn, bs, twoL], f"P{3 + fi}")
                      for fi, (f0, fn) in enumerate(qch)]
                for b in range(gb):
                    for mi, (m0, mn) in enumerate(mch):
                        for fi, (f0, fn) in enumerate(qch):
                            self.tp(
                                pT[fi][0:fn, b, m0:m0 + mn],
                                s1[mi][:, b, f0:f0 + fn],
                                self.ident[:mn, :mn])
                s2 = []
                for fi, (f0, fn) in enumerate(qch):
                    s = self.sbuf([fn, bs, twoL], f"bs2_{fi}")
                    nc.scalar.copy(s[:, :gb, :], pT[fi][:, :gb, :])
                    s2.append(s)
                # M2: transform r -> (twoL chunks, gb, twoL)
                S3[comp] = []
                for mi, (m0, mn) in enumerate(mch):
                    p = self.psum([mn, bs, twoL], f"P{5 + mi}")
                    for ki in range(len(qch)):
                        self.mm(p[:, :gb, :],
                                WT[ki][0][:, m0:m0 + mn],
                                s2[ki][:, :gb, :],
                                start=(ki == 0),
                                stop=(ki == len(qch) - 1))
                    s = self.sbuf([mn, bs, twoL], f"bs3_{comp}_{mi}")
                    nc.scalar.copy(s[:, :gb, :], p[:, :gb, :])
                    S3[comp].append(s)
            TH = {"re": [], "im": []}
            for mi, (m0, mn) in enumerate(mch):
                tr, ti = self.soft_pair(S3["re"][mi][:, :gb, :],
                                        S3["im"][mi][:, :gb, :],
                                        [mn, bs, twoL], gb)
                TH["re"].append(tr)
                TH["im"].append(ti)
            for comp in COMPS:
                for b in range(gb):
                    bg = g0 + b
                    for mi, (m0, mn) in enumerate(mch):
                        for X in (0, 1):
                            lo = max(m0, X * L)
                            hi = min(m0 + mn, (X + 1) * L)
                            if lo >= hi:
                                continue
                            rr0, h = lo - m0, hi - lo
                            rx0 = lo - X * L
                            for Y in (0, 1):
                                for dest, use_th in band_dest(
                                        comp, bg, X, Y, rx0, h):
                                    st = TH[comp][mi] if use_th else S3[comp][mi]
                                    nc.sync.dma_start(
                                        dest, st[rr0:rr0 + h, b,
                                                 Y * L:(Y + 1) * L])

    # ---- replicated lo-lo-lo quadrant of L2 (full 69 rows) -> VA2full
    def bc_ll_l2(self):
        nc = self.nc
        bs = 3
        Q, L = 131, 69
        WT = self.mats["WT2"]
        qch = chunks_of(Q)
        for comp in COMPS:
            src = self.dram[f"Af2F{comp}"]
            dst = self.dram[f"VA2full{comp}"]
            for g0 in range(0, L, bs):
                gb = min(bs, L - g0)
                ins = []
                for qi, (q0, qn) in enumerate(qch):
                    it = self.sbuf([qn, bs, Q], f"bci_{qi}", bufs=2)
                    sap = src[g0:g0 + gb, q0:q0 + qn, :].rearrange(
                        "b q n -> q b n")
                    nc.sync.dma_start(it[:, :gb, :], sap)
                    ins.append(it)
                p = self.psum([L, bs, Q], "P0")
                for ki in range(len(qch)):
                    self.mm(p[:, :gb, :], WT[ki][0][:, 0:L],
                            ins[ki][:, :gb, :], start=(ki == 0),
                            stop=(ki == len(qch) - 1))
                s1 = self.sbuf([L, bs, Q], "bs1_0")
                nc.scalar.copy(s1[:, :gb, :], p[:, :gb, :])
                pT = [self.psum([fn, bs, L], f"P{3 + fi}")
                      for fi, (f0, fn) in enumerate(qch)]
                for b in range(gb):
                    for fi, (f0, fn) in enumerate(qch):
                        self.tp(pT[fi][0:fn, b, 0:L],
                                s1[:, b, f0:f0 + fn],
                                self.ident[:L, :L])
                s2 = []
                for fi, (f0, fn) in enumerate(qch):
                    s = self.sbuf([fn, bs, L], f"bs2_{fi}")
                    nc.scalar.copy(s[:, :gb, :], pT[fi][:, :gb, :])
                    s2.append(s)
                p2 = self.psum([L, bs, L], "P5")
                for ki in range(len(qch)):
                    self.mm(p2[:, :gb, :], WT[ki][0][:, 0:L],
                            s2[ki][:, :gb, :], start=(ki == 0),
                            stop=(ki == len(qch) - 1))
                s3 = self.sbuf([L, bs, L], "bs3_re_0")
                nc.scalar.copy(s3[:, :gb, :], p2[:, :gb, :])
                for b in range(gb):
                    nc.sync.dma_start(dst[g0 + b, :, :], s3[:, b, :])

    # ---- inverse a-pass
    def inv_a(self, lvl, band_src, ntile=512):
        nc = self.nc
        L = NS[lvl]
        M = INV_OUT_ROWS[lvl]
        tot = L * L
        for comp in COMPS:
            for X in (0, 1):
                for Y in (0, 1):
                    A_ap, KA, D_ap, KD, lname = band_src(comp, X, Y)
                    lt = self.mats[lname][0][0]
                    dst = self.dram[f"O{lvl}{comp}{X}{Y}"].rearrange(
                        "a b c -> a (b c)")
                    for t0 in range(0, tot, ntile):
                        tn = min(ntile, tot - t0)
                        rt = self.sbuf([KA + KD, ntile], "ia_in", bufs=3)
                        nc.sync.dma_start(rt[0:KA, :tn], A_ap[:, t0:t0 + tn])
                        nc.sync.dma_start(rt[KA:KA + KD, :tn],
                                          D_ap[:, t0:t0 + tn])
                        p = self.psum([M, ntile], "P7")
                        self.mm(p[:, :tn], lt[:, :], rt[:, :tn],
                                start=True, stop=True)
                        s = self.sbuf([M, ntile], "ia_o", bufs=3)
                        nc.scalar.copy(s[:, :tn], p[:, :tn])
                        nc.sync.dma_start(dst[:, t0:t0 + tn], s[:, :tn])

    # ---- inverse bc-pass: O tensors (rows, L, L) -> parent rows (rows, P, P)
    def inv_bc(self, lvl, out_dest):
        nc = self.nc
        rows = INV_OUT_ROWS[lvl]
        bs = IBC_BS[lvl]
        L = NS[lvl]
        P = NS[lvl - 1]
        IAB = self.mats[f"IAB{lvl}"]
        lch = chunks_of(L)
        pch = chunks_of(P)

        def iab_slice(half, l0, ln, m0, mn):
            r0 = half * L + l0
            for (t, k0, kn) in IAB:
                if k0 <= r0 and r0 + ln <= k0 + kn:
                    return t[r0 - k0:r0 - k0 + ln, m0:m0 + mn]
            raise AssertionError(f"IAB{lvl} chunk misaligned {half} {l0} {ln}")

        for comp in COMPS:
            dst = out_dest(comp)
            for g0 in range(0, rows, bs):
                gb = min(bs, rows - g0)
                ot = {}
                for X in (0, 1):
                    for Y in (0, 1):
                        src = self.dram[f"O{lvl}{comp}{X}{Y}"]
                        for li, (l0, ln) in enumerate(lch):
                            t = self.sbuf([ln, bs, L], f"ibi_{X}{Y}_{li}")
                            sap = src[g0:g0 + gb, l0:l0 + ln, :].rearrange(
                                "b l n -> l b n")
                            nc.sync.dma_start(t[:, :gb, :], sap)
                            ot[(X, Y, li)] = t
                sU = {}
                for Y in (0, 1):
                    sU[Y] = []
                    for mi, (m0, mn) in enumerate(pch):
                        p = self.psum([mn, bs, L], f"P{mi}")
                        nkt = 2 * len(lch)
                        ki = 0
                        for X in (0, 1):
                            for li, (l0, ln) in enumerate(lch):
                                self.mm(
                                    p[:, :gb, :],
                                    iab_slice(X, l0, ln, m0, mn),
                                    ot[(X, Y, li)][:, :gb, :],
                                    start=(ki == 0), stop=(ki == nkt - 1))
                                ki += 1
                        s = self.sbuf([mn, bs, L], f"ibsu_{Y}_{mi}")
                        nc.scalar.copy(s[:, :gb, :], p[:, :gb, :])
                        sU[Y].append(s)
                sT = {}
                gsub = max(1, min(bs, 512 // P))   # b-rows per transpose psum
                for Y in (0, 1):
                    sT[Y] = [self.sbuf([ln, bs, P], f"ibst_{Y}_{li}")
                             for li, (l0, ln) in enumerate(lch)]
                    for b0 in range(0, gb, gsub):
                        bn = min(gsub, gb - b0)
                        pT = [self.psum([ln, gsub, P], f"P{2 + li}")
                              for li, (l0, ln) in enumerate(lch)]
                        for b in range(b0, b0 + bn):
                            for mi, (m0, mn) in enumerate(pch):
                                for li, (l0, ln) in enumerate(lch):
                                    self.tp(
                                        pT[li][0:ln, b - b0, m0:m0 + mn],
                                        sU[Y][mi][:, b, l0:l0 + ln],
                                        self.ident[:mn, :mn])
                        for li, (l0, ln) in enumerate(lch):
                            nc.scalar.copy(sT[Y][li][:, b0:b0 + bn, :],
                                           pT[li][:, :bn, :])
                # final matmul, column-chunked so each psum fits one bank
                nfch = max(1, -(-(bs * P * 4) // 2048))
                fch = chunks_of(P, -(-P // nfch))
                for mi, (m0, mn) in enumerate(pch):
                    s = self.sbuf([mn, bs, P], f"ibs3_{mi}", bufs=2)
                    for fi, (f0, fn) in enumerate(fch):
                        p = self.psum([mn, bs, fn], f"P{(6, 4)[fi] + mi}")
                        nkt = 2 * len(lch)
                        ki = 0
                        for Y in (0, 1):
                            for li, (l0, ln) in enumerate(lch):
                                self.mm(
                                    p[:, :gb, :],
                                    iab_slice(Y, l0, ln, m0, mn),
                                    sT[Y][li][:, :gb, f0:f0 + fn],
                                    start=(ki == 0), stop=(ki == nkt - 1))
                                ki += 1
                        nc.scalar.copy(s[:, :gb, f0:f0 + fn], p[:, :gb, :])
                    for b in range(gb):
                        nc.sync.dma_start(dst[g0 + b, m0:m0 + mn, :],
                                          s[:, b, :])


def build_program(thresh, use_collective=(True, True)):
    if isinstance(use_collective, bool):
        use_collective = (use_collective, use_collective)
    nc = bacc.Bacc("TRN2", target_bir_lowering=False, debug=False,
                   num_devices=NCORE)
    ext = {}
    for comp in COMPS:
        ext[f"xs_{comp}"] = nc.dram_tensor(f"xs_{comp}", [44, 256, 256], DT,
                                           kind="ExternalInput").ap()
    for name, shp in MAT_SHAPES.items():
        ext[name] = nc.dram_tensor(name, list(shp), DT,
                                   kind="ExternalInput").ap()
    outs = {}
    for comp in COMPS:
        outs[comp] = nc.dram_tensor(f"out_{comp}", [32, 256, 256], DT,
                                    kind="ExternalOutput").ap()

    with tile.TileContext(nc) as tc, ExitStack() as ctx:
        b = Builder(nc, tc, ctx, thresh)

        ident = b.p_wts.tile([128, 128], DT, name="ident", tag="ident")
        make_identity(nc, ident[:, :])
        b.ident = ident
        bias_eps = b.p_wts.tile([128, 1], DT, name="bias_eps", tag="bias_eps")
        nc.gpsimd.memset(bias_eps[:, :], 1e-38)
        b.bias_eps = bias_eps

        for name in MAT_SHAPES:
            b.load_mat(name, ext[name])
        for lvl in (3, 4, 5):
            b.load_mat(f"IABF{lvl}", ext[f"IAB{lvl}"],
                       splits=[(0, 2 * NS[lvl])])

        for comp in COMPS:
            b.dram[f"xs{comp}"] = ext[f"xs_{comp}"]
            b.dram_tile(f"Af1{comp}", (38, 256, 256))
            b.dram_tile(f"Af2{comp}", (26, 131, 131))
            b.dram_tile(f"Af2F{comp}", (69, 131, 131))
            b.dram_tile(f"Af3{comp}", (76, 69, 69))
            b.dram_tile(f"Af4{comp}", (44, 38, 38))
            b.dram_tile(f"Af5{comp}", (28, 22, 22))
            for af in (0, 1):
                for X in (0, 1):
                    for Y in (0, 1):
                        if af == 0 and X == 0 and Y == 0:
                            continue
                        for lvl, (rn, L) in {1: (19, 131), 2: (13, 69),
                                             3: (38, 38), 4: (22, 22),
                                             5: (14, 14)}.items():
                            b.dram_tile(f"B{lvl}{comp}{af}{X}{Y}", (rn, L, L))
            b.dram_tile(f"B5{comp}000", (14, 14, 14))
            b.dram_tile(f"VA3{comp}", (38, 38, 38))
            b.dram_tile(f"VA4{comp}", (22, 22, 22))
            b.dram_tile(f"VA1full{comp}", (131, 131, 131))
            b.dram_tile(f"VA2full{comp}", (69, 69, 69))
            b.dram_tile(f"VA1rec{comp}", (19, 131, 131))
            b.dram_tile(f"VA2rec{comp}", (69, 69, 69))
            b.dram_tile(f"VA3rec{comp}", (38, 38, 38))
            b.dram_tile(f"VA4rec{comp}", (22, 22, 22))
            for lvl, L in {1: 131, 2: 69, 3: 38, 4: 22, 5: 14}.items():
                for X in (0, 1):
                    for Y in (0, 1):
                        b.dram_tile(f"O{lvl}{comp}{X}{Y}",
                                    (INV_OUT_ROWS[lvl], L, L))
        ag1_in = b.dram_tile("ag1_in", (38, 131, 131))
        ag1_out = b.dram_tile("ag1_out", (NCORE * 38, 131, 131),
                              addr_space="Shared")

        # ============ forward ============
        b.fwd_a("A1T", {c: f"xs{c}" for c in COMPS},
                {c: f"Af1{c}" for c in COMPS}, 38, 256)

        def bd1(comp, bg, X, Y, rx0, h):
            af, br = (0, bg) if bg < 19 else (1, bg - 19)
            if af == 0 and X == 0 and Y == 0:
                ci = 0 if comp == "re" else 1
                return [(ag1_in[ci * 19 + br, rx0:rx0 + h, :], False)]
            return [(b.dram[f"B1{comp}{af}{X}{Y}"][br, rx0:rx0 + h, :], True)]

        b.bc_fwd(1, 38, bd1)

        if use_collective[0]:
            nc.gpsimd.collective_compute(
                "AllGather", mybir.AluOpType.bypass,
                ins=[ag1_in.opt()], outs=[ag1_out.opt()],
                replica_groups=[list(range(NCORE))])
        else:
            nc.sync.dma_start(ag1_out[0:38], ag1_in[0:38])
        for ci, comp in enumerate(COMPS):
            for k in range(NCORE):
                nrows = 16 if k < 7 else 19
                nc.sync.dma_start(
                    b.dram[f"VA1full{comp}"][16 * k:16 * k + nrows],
                    ag1_out[38 * k + ci * 19:38 * k + ci * 19 + nrows])

        b.fwd_a("A2T", {c: f"VA1full{c}" for c in COMPS},
                {c: f"Af2{c}" for c in COMPS}, 26, 131)

        def bd2(comp, bg, X, Y, rx0, h):
            af, br = (0, bg) if bg < 13 else (1, bg - 13)
            if af == 0 and X == 0 and Y == 0:
                return []    # full aaa2 is recomputed replicated below
            return [(b.dram[f"B2{comp}{af}{X}{Y}"][br, rx0:rx0 + h, :], True)]

        b.bc_fwd(2, 26, bd2)

        # replicated full aaa2 from the replicated VA1full (avoids 2nd AG)
        b.fwd_a("W2LOT", {c: f"VA1full{c}" for c in COMPS},
                {c: f"Af2F{c}" for c in COMPS}, 69, 131)
        b.bc_ll_l2()

        def bd_rep(lvl, half_rows, va_name):
            def f(comp, bg, X, Y, rx0, h):
                af, br = (0, bg) if bg < half_rows else (1, bg - half_rows)
                if af == 0 and X == 0 and Y == 0:
                    if lvl == 5:
                        return [(b.dram[f"B5{comp}000"][br, rx0:rx0 + h, :],
                                 True)]
                    return [(b.dram[f"{va_name}{comp}"][br, rx0:rx0 + h, :],
                             False)]
                return [(b.dram[f"B{lvl}{comp}{af}{X}{Y}"][br, rx0:rx0 + h, :],
                         True)]
            return f

        b.fwd_a("WT3", {c: f"VA2full{c}" for c in COMPS},
                {c: f"Af3{c}" for c in COMPS}, 76, 69)
        b.bc_fwd(3, 76, bd_rep(3, 38, "VA3"))
        b.fwd_a("WT4", {c: f"VA3{c}" for c in COMPS},
                {c: f"Af4{c}" for c in COMPS}, 44, 38)
        b.bc_fwd(4, 44, bd_rep(4, 22, "VA4"))
        b.fwd_a("WT5", {c: f"VA4{c}" for c in COMPS},
                {c: f"Af5{c}" for c in COMPS}, 28, 22)
        b.bc_fwd(5, 28, bd_rep(5, 14, None))

        # ============ inverse ============
        def bsrc_rep(lvl, va_rec):
            L = NS[lvl]

            def f(comp, X, Y):
                if X == 0 and Y == 0:
                    A = (b.dram[f"B5{comp}000"] if lvl == 5
                         else b.dram[va_rec + comp])
                else:
                    A = b.dram[f"B{lvl}{comp}0{X}{Y}"]
                D = b.dram[f"B{lvl}{comp}1{X}{Y}"]
                return (A.rearrange("a b c -> a (b c)"), L,
                        D.rearrange("a b c -> a (b c)"), L, f"IABF{lvl}")
            return f

        b.inv_a(5, bsrc_rep(5, None))
        b.inv_bc(5, lambda comp: b.dram[f"VA4rec{comp}"])
        b.inv_a(4, bsrc_rep(4, "VA4rec"))
        b.inv_bc(4, lambda comp: b.dram[f"VA3rec{comp}"])
        b.inv_a(3, bsrc_rep(3, "VA3rec"))
        b.inv_bc(3, lambda comp: b.dram[f"VA2rec{comp}"])

        def bsrc2(comp, X, Y):
            D = b.dram[f"B2{comp}1{X}{Y}"].rearrange("a b c -> a (b c)")
            if X == 0 and Y == 0:
                A = b.dram[f"VA2rec{comp}"].rearrange("a b c -> a (b c)")
                return (A, 69, D, 13, "IA2LL")
            A = b.dram[f"B2{comp}0{X}{Y}"].rearrange("a b c -> a (b c)")
            return (A, 13, D, 13, "IA2")

        b.inv_a(2, bsrc2)
        b.inv_bc(2, lambda comp: b.dram[f"VA1rec{comp}"])

        def bsrc1(comp, X, Y):
            D = b.dram[f"B1{comp}1{X}{Y}"].rearrange("a b c -> a (b c)")
            if X == 0 and Y == 0:
                A = b.dram[f"VA1rec{comp}"].rearrange("a b c -> a (b c)")
            else:
                A = b.dram[f"B1{comp}0{X}{Y}"].rearrange("a b c -> a (b c)")
            return (A, 19, D, 19, "IA1")

        b.inv_a(1, bsrc1)
        b.inv_bc(1, lambda comp: outs[comp])

    nc.compile()
    return nc


_CACHE = {}


def make_in_maps(x_real, x_imag):
    x_real = np.ascontiguousarray(x_real, dtype=np.float32)
    x_imag = np.ascontiguousarray(x_imag, dtype=np.float32)
    in_maps = []
    for c in range(NCORE):
        m = host_matrices(c)
        slab_lo = 32 * c - 6
        im = {}
        for comp, x in (("re", x_real), ("im", x_imag)):
            s = np.zeros((44, 256, 256), dtype=np.float32)
            g0, g1 = max(0, slab_lo), min(256, slab_lo + 44)
            s[g0 - slab_lo:g1 - slab_lo] = x[g0:g1]
            im[f"xs_{comp}"] = s
        im.update(m)
        in_maps.append(im)
    return in_maps


def kernel(x_real, x_imag, alpha):
    thresh = 1e-3 * float(np.asarray(alpha))
    if thresh not in _CACHE:
        _CACHE[thresh] = build_program(thresh)
    nc = _CACHE[thresh]

    in_maps = make_in_maps(x_real, x_imag)
    res = run_bass_kernel_spmd(nc, in_maps, core_ids=list(range(NCORE)))
    out = np.empty((256, 256, 256), dtype=np.complex64)
    for c in range(NCORE):
        r = res.results[c]
        out[32 * c:32 * c + 32] = r["out_re"] + 1j * r["out_im"]
    return out



# revision 46
# speedup vs baseline: 1.3526x; 1.3526x over previous
"""Trainium2 Bass kernel for nn_L1Wav: 5-level 3D db4 wavelet soft-threshold
denoising of a 256^3 complex volume, SPMD over 8 NeuronCores.

Math notes (verified against the jax reference in a numpy sim):
  - The deterministic rng(1000) shift is 0 and the unit-modulus phase cancels
    through the prox (DWT is real-linear; |phase*w| = |w|), so the computation
    is exactly: 5-level 3D DWT -> complex soft-threshold -> inverse DWT.
  - Every 1D DWT/IDWT pass is a matmul against a banded filter matrix.
  - Sharding: volume split along axis 0 (32 planes/core). All a-axis passes
    use per-core weight-matrix slices, so the core-dependence lives entirely
    in host-provided matrices and one SPMD program serves all cores.
    Levels 1-2 are distributed; levels 3-5 are replicated on every core.
    The only communication is two small AllGathers of approx bands.

Level sizes: 256 -> 131 -> 69 -> 38 -> 22 -> 14.
Per-core windows: L1 band rows [16c,16c+19); L2 band rows [8c,8c+13);
output rows [32c,32c+32); input slab rows [32c-6,32c+38) zero-padded.

Layout: a volume at any level is stored (p, q, r). The forward a-pass
contracts p; the per-row bc-pass transforms q then r, emitting tiles
(r', q'), so child band tensors are stored (a_row, r', q').
"""
import sys
from contextlib import ExitStack

import numpy as np

sys.path.insert(0, "/opt/trn_rl_repo")

import concourse.bass as bass
import concourse.mybir as mybir
import concourse.tile as tile
from concourse import bacc
from concourse.bass_utils import run_bass_kernel_spmd
from concourse.masks import make_identity

DT = mybir.dt.float32
R32 = mybir.dt.float32r
F = 8
DEC_LO = np.array([-0.010597401784997278, 0.032883011666982945, 0.030841381835986965,
                   -0.18703481171888114, -0.02798376941698385, 0.6308807679295904,
                   0.7148465705525415, 0.23037781330885523])
REC_LO = DEC_LO[::-1].copy()
REC_HI = np.array([((-1) ** n) * DEC_LO[n] for n in range(F)])
DEC_HI = REC_HI[::-1].copy()

NS = [256, 131, 69, 38, 22, 14]     # sizes level 0..5
NCORE = 8
COMPS = ("re", "im")
BC_BS = {1: 1, 2: 3, 3: 6, 4: 11, 5: 14}       # fwd bc row batch
IBC_BS = {1: 2, 2: 5, 3: 7, 4: 13, 5: 14}      # inv bc row batch
INV_OUT_ROWS = {1: 32, 2: 19, 3: 69, 4: 38, 5: 22}


def W_mat(N, flt):
    L = (N + F - 1) // 2
    W = np.zeros((L, N), dtype=np.float32)
    for l in range(L):
        for j in range(F):
            n = 2 * l + 1 - j
            if 0 <= n < N:
                W[l, n] = flt[j]
    return W


def G_mat(L, crop, flt):
    G = np.zeros((crop, L), dtype=np.float32)
    for t in range(crop):
        for m in range(L):
            j = t + 6 - 2 * m
            if 0 <= j < F:
                G[t, m] = flt[j]
    return G


def pad_even(a):
    """Pad the last (M) dim to an even count (fp32r stationary rule)."""
    if a.shape[-1] % 2:
        a = np.pad(a, [(0, 0)] * (a.ndim - 1) + [(0, 1)])
    return a


def host_matrices(core):
    """All weight matrices for one core (lhsT layout: (K, M))."""
    c = core
    m = {}
    for l in range(5):
        W2 = np.concatenate([W_mat(NS[l], DEC_LO), W_mat(NS[l], DEC_HI)], 0)
        m[f"WT{l + 1}"] = np.ascontiguousarray(W2.T)
        glo = G_mat(NS[l + 1], NS[l], REC_LO)
        ghi = G_mat(NS[l + 1], NS[l], REC_HI)
        m[f"IAB{l + 1}"] = pad_even(np.ascontiguousarray(
            np.concatenate([glo.T, ghi.T], 0)))
    # L1 fwd a-pass (per-core): A1 (38, 44) -> lhsT (44, 38)
    A1 = np.zeros((38, 44), dtype=np.float32)
    slab_lo = 32 * c - 6
    for half, flt in ((0, DEC_LO), (1, DEC_HI)):
        for i in range(19):
            l = 16 * c + i
            for k in range(44):
                n = slab_lo + k
                j = 2 * l + 1 - n
                if 0 <= j < F and 0 <= n < 256:
                    A1[half * 19 + i, k] = flt[j]
    m["A1T"] = np.ascontiguousarray(A1.T)
    # L2 fwd a-pass, merged with the replicated full-lo pass, contracting
    # directly over ag1_out rows (38k+19ci+i = VA1full row 16k+i, owner
    # k = min(row//16, 7)).  M = 26 per-core band rows + 69 full-lo rows.
    A2 = np.concatenate([W_mat(131, DEC_LO)[8 * c:8 * c + 13],
                         W_mat(131, DEC_HI)[8 * c:8 * c + 13]], 0)
    M95 = np.concatenate([A2, W_mat(131, DEC_LO)], 0)       # (95, 131)
    for ci in range(2):
        A2WT = np.zeros((304, 96), dtype=np.float32)
        for r in range(131):
            k = min(r // 16, 7)
            A2WT[38 * k + 19 * ci + (r - 16 * k), :95] = M95[:, r]
        m["A2WTre" if ci == 0 else "A2WTim"] = A2WT
    # L1 inv a-pass: core-independent (38, 32)
    G1a = np.zeros((32, 19), dtype=np.float32)
    G1d = np.zeros((32, 19), dtype=np.float32)
    for u in range(32):
        for v in range(19):
            j = u + 6 - 2 * v
            if 0 <= j < F:
                G1a[u, v] = REC_LO[j]
                G1d[u, v] = REC_HI[j]
    m["IA1"] = np.ascontiguousarray(np.concatenate([G1a.T, G1d.T], 0))
    # L2 inv a-pass (per-core)
    glo1 = G_mat(69, 131, REC_LO)
    ghi1 = G_mat(69, 131, REC_HI)
    g2a_full = glo1[16 * c:16 * c + 19, :]                    # (19, 69)
    g2a13 = glo1[16 * c:16 * c + 19, 8 * c:8 * c + 13]
    g2d13 = ghi1[16 * c:16 * c + 19, 8 * c:8 * c + 13]
    m["IA2"] = pad_even(np.ascontiguousarray(
        np.concatenate([g2a13.T, g2d13.T], 0)))
    m["IA2LL"] = pad_even(np.ascontiguousarray(
        np.concatenate([g2a_full.T, g2d13.T], 0)))
    return {k: v.astype(np.float32) for k, v in m.items()}


MAT_SHAPES = {k: v.shape for k, v in host_matrices(0).items()}
# partition-chunk splits for SBUF-resident matrices (K dim)
MAT_SPLITS = {
    "IAB1": [(0, 128), (128, 3), (131, 128), (259, 3)],
    "IAB2": [(0, 69), (69, 69)],
    "IAB3": [(0, 38), (38, 38)],
    "IAB4": [(0, 22), (22, 22)],
    "IAB5": [(0, 14), (14, 14)],
}


def chunks_of(total, size=128):
    return [(i, min(size, total - i)) for i in range(0, total, size)]


class Builder:
    def __init__(self, nc, tc, ctx, thresh):
        self.nc = nc
        self.tc = tc
        self.thresh = float(thresh)
        self.p_dram = ctx.enter_context(
            tc.tile_pool(name="dram", bufs=1, space=bass.MemorySpace.DRAM))
        self.p_wts = ctx.enter_context(tc.tile_pool(name="wts", bufs=1))
        self.p_work = ctx.enter_context(tc.tile_pool(name="work", bufs=1))
        self.p_psum = ctx.enter_context(
            tc.tile_pool(name="psum", bufs=1, space=bass.MemorySpace.PSUM))
        self.mats = {}
        self.dram = {}
        self.uid = 0

    def _id(self):
        self.uid += 1
        return self.uid

    def dram_tile(self, name, shape, addr_space="Local"):
        t = self.p_dram.tile(list(shape), DT, name=name, tag=name,
                             addr_space=addr_space)
        self.dram[name] = t
        return t

    def sbuf(self, shape, tag, bufs=1, dt=DT):
        return self.p_work.tile(list(shape), dt, name=f"t{self._id()}",
                                tag=tag, bufs=bufs)

    def psum(self, shape, tag):
        return self.p_psum.tile(list(shape), DT, name=f"p{self._id()}",
                                tag=tag, bufs=1)

    # fp32r matmul: 1 cycle/row (vs 4 for fp32) when moving free size >= 256.
    # Operand tiles are declared float32r (bit-identical to fp32); the BIR
    # verifier requires producers to carry the fp32r dtype, so loads bitcast
    # the DRAM AP and psum->sbuf copies write fp32r-typed tiles.
    def mm(self, out, lhsT, rhs, **kw):
        self.nc.tensor.matmul(out, lhsT, rhs, **kw)

    def dmaR(self, dst, src):
        self.nc.sync.dma_start(dst, src.bitcast(R32))

    def load_mat(self, name, dram_ap, splits=None):
        K, M = dram_ap.shape
        if splits is None:
            splits = MAT_SPLITS.get(name, chunks_of(K))
        tiles = []
        for (k0, kn) in splits:
            t = self.p_wts.tile([kn, M], R32, name=f"{name}_{k0}",
                                tag=f"{name}_{k0}", bufs=1)
            self.dmaR(t[:, :], dram_ap[k0:k0 + kn, :])
            tiles.append((t, k0, kn))
        self.mats[name] = tiles

    # ---- soft threshold: returns thresholded (re, im) tiles (full-shape)
    def soft_pair(self, s_re, s_im, shape, gb):
        nc = self.nc
        t = self.thresh
        mn = shape[0]
        tmp1 = self.sbuf(shape, "sm1")
        tmp2 = self.sbuf(shape, "sm2")
        a = tmp1[:, :gb, :]
        m = tmp2[:, :gb, :]
        nc.vector.tensor_mul(a, s_re, s_re)
        nc.vector.tensor_mul(m, s_im, s_im)
        nc.vector.tensor_add(a, a, m)
        nc.scalar.activation(m, a, mybir.ActivationFunctionType.Sqrt,
                             bias=self.bias_eps[:mn, :])
        nc.vector.tensor_scalar(a, m, -t, 0.0,
                                mybir.AluOpType.add, mybir.AluOpType.max)
        nc.vector.reciprocal(m, m)
        nc.vector.tensor_mul(a, a, m)
        th_re = self.sbuf(shape, "str", bufs=2)
        th_im = self.sbuf(shape, "sti", bufs=2)
        nc.vector.tensor_mul(th_re[:, :gb, :], s_re, a)
        nc.vector.tensor_mul(th_im[:, :gb, :], s_im, a)
        return th_re, th_im

    # ---- forward a-pass: out (M, n, n) = lhsT^T @ in (K, n, n)
    def fwd_a(self, lname, in_keys, out_keys, M, n, ntile=512):
        nc = self.nc
        lhsT = self.mats[lname]
        for comp in COMPS:
            srcf = self.dram[in_keys[comp]].rearrange("a b c -> a (b c)")
            dstf = self.dram[out_keys[comp]].rearrange("a b c -> a (b c)")
            tot = n * n
            for t0 in range(0, tot, ntile):
                tn = min(ntile, tot - t0)
                tp = tn + (tn & 1)      # fp32r needs even moving/psum width
                rts = []
                for i, (lt, k0, kn) in enumerate(lhsT):
                    rt = self.sbuf([kn, ntile], f"fa_in_{i}", bufs=3, dt=R32)
                    self.dmaR(rt[:, :tn], srcf[k0:k0 + kn, t0:t0 + tn])
                    rts.append(rt)
                p = self.psum([M, ntile], "P0")
                for i, (lt, k0, kn) in enumerate(lhsT):
                    self.mm(p[:, :tp], lt[:, :], rts[i][:, :tp],
                            start=(i == 0), stop=(i == len(lhsT) - 1))
                s = self.sbuf([M, ntile], "fa_o", bufs=3)
                nc.scalar.copy(s[:, :tn], p[:, :tn])
                nc.sync.dma_start(dstf[:, t0:t0 + tn], s[:, :tn])

    # ---- merged L2 a-pass: contract ag1_out rows directly; outputs both
    # the per-core 26 band rows and the replicated 69 full-lo rows.
    def fwd_a2_merged(self, ag_out, ntile=512):
        nc = self.nc
        lhs = {c: self.mats[f"A2WT{c}"] for c in COMPS}
        src = ag_out.rearrange("a b c -> a (b c)")
        dsts = {c: self.dram[f"Af2C{c}"].rearrange("a b c -> a (b c)")
                for c in COMPS}
        tot = 131 * 131
        for t0 in range(0, tot, ntile):
            tn = min(ntile, tot - t0)
            tp = tn + (tn & 1)
            rts = []
            for i, (lt, k0, kn) in enumerate(lhs["re"]):
                rt = self.sbuf([kn, ntile], f"fa2_in_{i}", bufs=2, dt=R32)
                self.dmaR(rt[:, :tn], src[k0:k0 + kn, t0:t0 + tn])
                rts.append(rt)
            for pi, comp in enumerate(COMPS):
                lT = lhs[comp]
                p = self.psum([96, ntile], f"P{pi}")
                for i, (lt, k0, kn) in enumerate(lT):
                    self.mm(p[:, :tp], lt[:, :], rts[i][:, :tp],
                            start=(i == 0), stop=(i == len(lT) - 1))
                s = self.sbuf([95, ntile], f"fa2_o_{comp}", bufs=3)
                nc.scalar.copy(s[:, :tn], p[0:95, :tn])
                nc.sync.dma_start(dsts[comp][:, t0:t0 + tn], s[:, :tn])

    # ---- forward bc-pass for one level
    def bc_fwd(self, lvl, rows, band_dest):
        nc = self.nc
        bs = BC_BS[lvl]
        Q = NS[lvl - 1]
        Qp = Q + (Q & 1)                # fp32r even-width padding
        L = NS[lvl]
        twoL = 2 * L
        WT = self.mats[f"WT{lvl}"]
        qch = chunks_of(Q)
        mch = chunks_of(twoL)
        for g0 in range(0, rows, bs):
            gb = min(bs, rows - g0)
            S3 = {}
            for comp in COMPS:
                src = self.dram[f"Af{lvl}{comp}"]
                ins = []
                for qi, (q0, qn) in enumerate(qch):
                    it = self.sbuf([qn, bs, Qp], f"bci_{qi}", bufs=2, dt=R32)
                    sap = src[g0:g0 + gb, q0:q0 + qn, :].rearrange(
                        "b q n -> q b n")
                    self.dmaR(it[:, :gb, :Q], sap)
                    ins.append(it)
                # M1: transform q -> (twoL chunks, gb, Q)
                s1 = []
                for mi, (m0, mn) in enumerate(mch):
                    p = self.psum([mn, bs, Qp], f"P{mi}")
                    for ki in range(len(qch)):
                        self.mm(p[:, :gb, :],
                                WT[ki][0][:, m0:m0 + mn],
                                ins[ki][:, :gb, :],
                                start=(ki == 0),
                                stop=(ki == len(qch) - 1))
                    s = self.sbuf([mn, bs, Q], f"bs1_{mi}")
                    nc.scalar.copy(s[:, :gb, :], p[:, :gb, :Q])
                    s1.append(s)
                # transpose -> (Q chunks, gb, twoL)
                pT = [self.psum([fn, bs, twoL], f"P{3 + fi}")
                      for fi, (f0, fn) in enumerate(qch)]
                for b in range(gb):
                    for mi, (m0, mn) in enumerate(mch):
                        for fi, (f0, fn) in enumerate(qch):
                            nc.tensor.transpose(
                                pT[fi][0:fn, b, m0:m0 + mn],
                                s1[mi][:, b, f0:f0 + fn],
                                self.ident[:mn, :mn])
                s2 = []
                for fi, (f0, fn) in enumerate(qch):
                    s = self.sbuf([fn, bs, twoL], f"bs2_{fi}", dt=R32)
                    nc.scalar.copy(s[:, :gb, :], pT[fi][:, :gb, :])
                    s2.append(s)
                # M2: transform r -> (twoL chunks, gb, twoL)
                S3[comp] = []
                for mi, (m0, mn) in enumerate(mch):
                    p = self.psum([mn, bs, twoL], f"P{5 + mi}")
                    for ki in range(len(qch)):
                        self.mm(p[:, :gb, :],
                                WT[ki][0][:, m0:m0 + mn],
                                s2[ki][:, :gb, :],
                                start=(ki == 0),
                                stop=(ki == len(qch) - 1))
                    s = self.sbuf([mn, bs, twoL], f"bs3_{comp}_{mi}")
                    nc.scalar.copy(s[:, :gb, :], p[:, :gb, :])
                    S3[comp].append(s)
            TH = {"re": [], "im": []}
            for mi, (m0, mn) in enumerate(mch):
                tr, ti = self.soft_pair(S3["re"][mi][:, :gb, :],
                                        S3["im"][mi][:, :gb, :],
                                        [mn, bs, twoL], gb)
                TH["re"].append(tr)
                TH["im"].append(ti)
            for comp in COMPS:
                for b in range(gb):
                    bg = g0 + b
                    for mi, (m0, mn) in enumerate(mch):
                        for X in (0, 1):
                            lo = max(m0, X * L)
                            hi = min(m0 + mn, (X + 1) * L)
                            if lo >= hi:
                                continue
                            rr0, h = lo - m0, hi - lo
                            rx0 = lo - X * L
                            for Y in (0, 1):
                                for dest, use_th in band_dest(
                                        comp, bg, X, Y, rx0, h):
                                    st = TH[comp][mi] if use_th else S3[comp][mi]
                                    nc.sync.dma_start(
                                        dest, st[rr0:rr0 + h, b,
                                                 Y * L:(Y + 1) * L])

    # ---- replicated lo-lo-lo quadrant of L2 (full 69 rows) -> VA2full
    def bc_ll_l2(self):
        nc = self.nc
        bs = 3
        Q, L = 131, 69
        Qp, Lp = Q + 1, L + 1
        WT = self.mats["WT2"]
        qch = chunks_of(Q)
        for comp in COMPS:
            src = self.dram[f"Af2F{comp}"]
            dst = self.dram[f"VA2full{comp}"]
            for g0 in range(0, L, bs):
                gb = min(bs, L - g0)
                ins = []
                for qi, (q0, qn) in enumerate(qch):
                    it = self.sbuf([qn, bs, Qp], f"bci_{qi}", bufs=2, dt=R32)
                    sap = src[g0:g0 + gb, q0:q0 + qn, :].rearrange(
                        "b q n -> q b n")
                    self.dmaR(it[:, :gb, :Q], sap)
                    ins.append(it)
                p = self.psum([Lp, bs, Qp], "P0")
                for ki in range(len(qch)):
                    self.mm(p[:, :gb, :], WT[ki][0][:, 0:Lp],
                            ins[ki][:, :gb, :], start=(ki == 0),
                            stop=(ki == len(qch) - 1))
                s1 = self.sbuf([L, bs, Q], "bs1_0")
                nc.scalar.copy(s1[:, :gb, :], p[0:L, :gb, :Q])
                pT = [self.psum([fn, bs, L], f"P{3 + fi}")
                      for fi, (f0, fn) in enumerate(qch)]
                for b in range(gb):
                    for fi, (f0, fn) in enumerate(qch):
                        nc.tensor.transpose(pT[fi][0:fn, b, 0:L],
                                            s1[:, b, f0:f0 + fn],
                                            self.ident[:L, :L])
                s2 = []
                for fi, (f0, fn) in enumerate(qch):
                    s = self.sbuf([fn, bs, Lp], f"bs2_{fi}", dt=R32)
                    nc.scalar.copy(s[:, :gb, :L], pT[fi][:, :gb, :])
                    s2.append(s)
                p2 = self.psum([Lp, bs, Lp], "P5")
                for ki in range(len(qch)):
                    self.mm(p2[:, :gb, :], WT[ki][0][:, 0:Lp],
                            s2[ki][:, :gb, :], start=(ki == 0),
                            stop=(ki == len(qch) - 1))
                s3 = self.sbuf([L, bs, L], "bs3_re_0")
                nc.scalar.copy(s3[:, :gb, :], p2[0:L, :gb, :L])
                for b in range(gb):
                    nc.sync.dma_start(dst[g0 + b, :, :], s3[:, b, :])

    # ---- inverse a-pass
    def inv_a(self, lvl, band_src, ntile=512):
        nc = self.nc
        L = NS[lvl]
        M = INV_OUT_ROWS[lvl]
        Mp = M + (M & 1)
        tot = L * L
        for comp in COMPS:
            for X in (0, 1):
                for Y in (0, 1):
                    A_ap, KA, D_ap, KD, lname = band_src(comp, X, Y)
                    lt = self.mats[lname][0][0]
                    dst = self.dram[f"O{lvl}{comp}{X}{Y}"].rearrange(
                        "a b c -> a (b c)")
                    for t0 in range(0, tot, ntile):
                        tn = min(ntile, tot - t0)
                        tp = tn + (tn & 1)
                        rt = self.sbuf([KA + KD, ntile], "ia_in", bufs=3,
                                       dt=R32)
                        self.dmaR(rt[0:KA, :tn], A_ap[:, t0:t0 + tn])
                        self.dmaR(rt[KA:KA + KD, :tn], D_ap[:, t0:t0 + tn])
                        p = self.psum([Mp, ntile], "P7")
                        self.mm(p[:, :tp], lt[:, :], rt[:, :tp],
                                start=True, stop=True)
                        s = self.sbuf([M, ntile], "ia_o", bufs=3)
                        nc.scalar.copy(s[:, :tn], p[0:M, :tn])
                        nc.sync.dma_start(dst[:, t0:t0 + tn], s[:, :tn])

    # ---- inverse bc-pass: O tensors (rows, L, L) -> parent rows (rows, P, P)
    def inv_bc(self, lvl, out_dest):
        nc = self.nc
        rows = INV_OUT_ROWS[lvl]
        bs = IBC_BS[lvl]
        L = NS[lvl]
        P = NS[lvl - 1]
        Lp = L + (L & 1)
        Pp = P + (P & 1)
        IAB = self.mats[f"IAB{lvl}"]
        lch = chunks_of(L)
        pch = chunks_of(Pp)             # even chunks (fp32r stationary rule)

        def iab_slice(half, l0, ln, m0, mn):
            r0 = half * L + l0
            for (t, k0, kn) in IAB:
                if k0 <= r0 and r0 + ln <= k0 + kn:
                    return t[r0 - k0:r0 - k0 + ln, m0:m0 + mn]
            raise AssertionError(f"IAB{lvl} chunk misaligned {half} {l0} {ln}")

        for comp in COMPS:
            dst = out_dest(comp)
            for g0 in range(0, rows, bs):
                gb = min(bs, rows - g0)
                ot = {}
                for X in (0, 1):
                    for Y in (0, 1):
                        src = self.dram[f"O{lvl}{comp}{X}{Y}"]
                        for li, (l0, ln) in enumerate(lch):
                            t = self.sbuf([ln, bs, Lp], f"ibi_{X}{Y}_{li}",
                                          dt=R32)
                            sap = src[g0:g0 + gb, l0:l0 + ln, :].rearrange(
                                "b l n -> l b n")
                            self.dmaR(t[:, :gb, :L], sap)
                            ot[(X, Y, li)] = t
                sU = {}
                for Y in (0, 1):
                    sU[Y] = []
                    for mi, (m0, mn) in enumerate(pch):
                        p = self.psum([mn, bs, Lp], f"P{mi}")
                        nkt = 2 * len(lch)
                        ki = 0
                        for X in (0, 1):
                            for li, (l0, ln) in enumerate(lch):
                                self.mm(
                                    p[:, :gb, :],
                                    iab_slice(X, l0, ln, m0, mn),
                                    ot[(X, Y, li)][:, :gb, :],
                                    start=(ki == 0), stop=(ki == nkt - 1))
                                ki += 1
                        s = self.sbuf([mn, bs, L], f"ibsu_{Y}_{mi}")
                        nc.scalar.copy(s[:, :gb, :], p[:, :gb, :L])
                        sU[Y].append(s)
                sT = {}
                gsub = max(1, min(bs, 512 // Pp))  # b-rows per transpose psum
                for Y in (0, 1):
                    sT[Y] = [self.sbuf([ln, bs, Pp], f"ibst_{Y}_{li}", dt=R32)
                             for li, (l0, ln) in enumerate(lch)]
                    for b0 in range(0, gb, gsub):
                        bn = min(gsub, gb - b0)
                        pT = [self.psum([ln, gsub, Pp], f"P{2 + li}")
                              for li, (l0, ln) in enumerate(lch)]
                        for b in range(b0, b0 + bn):
                            for mi, (m0, mn) in enumerate(pch):
                                for li, (l0, ln) in enumerate(lch):
                                    nc.tensor.transpose(
                                        pT[li][0:ln, b - b0, m0:m0 + mn],
                                        sU[Y][mi][:, b, l0:l0 + ln],
                                        self.ident[:mn, :mn])
                        for li, (l0, ln) in enumerate(lch):
                            nc.scalar.copy(sT[Y][li][:, b0:b0 + bn, :],
                                           pT[li][:, :bn, :])
                # final matmul, column-chunked (even widths, one psum bank)
                nfch = max(1, -(-(bs * Pp * 4) // 2048))
                fw = -(-Pp // nfch)
                fw += fw & 1
                fch = chunks_of(Pp, fw)
                for mi, (m0, mn) in enumerate(pch):
                    s = self.sbuf([mn, bs, Pp], f"ibs3_{mi}", bufs=2)
                    for fi, (f0, fn) in enumerate(fch):
                        p = self.psum([mn, bs, fn], f"P{(6, 4)[fi] + mi}")
                        nkt = 2 * len(lch)
                        ki = 0
                        for Y in (0, 1):
                            for li, (l0, ln) in enumerate(lch):
                                self.mm(
                                    p[:, :gb, :],
                                    iab_slice(Y, l0, ln, m0, mn),
                                    sT[Y][li][:, :gb, f0:f0 + fn],
                                    start=(ki == 0), stop=(ki == nkt - 1))
                                ki += 1
                        nc.scalar.copy(s[:, :gb, f0:f0 + fn], p[:, :gb, :])
                    rmn = min(mn, P - m0)
                    for b in range(gb):
                        nc.sync.dma_start(dst[g0 + b, m0:m0 + rmn, :],
                                          s[0:rmn, b, :P])


def build_program(thresh, use_collective=(True, True)):
    if isinstance(use_collective, bool):
        use_collective = (use_collective, use_collective)
    nc = bacc.Bacc("TRN2", target_bir_lowering=False, debug=False,
                   num_devices=NCORE)
    ext = {}
    for comp in COMPS:
        ext[f"xs_{comp}"] = nc.dram_tensor(f"xs_{comp}", [44, 256, 256], DT,
                                           kind="ExternalInput").ap()
    for name, shp in MAT_SHAPES.items():
        ext[name] = nc.dram_tensor(name, list(shp), DT,
                                   kind="ExternalInput").ap()
    outs = {}
    for comp in COMPS:
        outs[comp] = nc.dram_tensor(f"out_{comp}", [32, 256, 256], DT,
                                    kind="ExternalOutput").ap()

    with tile.TileContext(nc) as tc, ExitStack() as ctx:
        b = Builder(nc, tc, ctx, thresh)

        ident = b.p_wts.tile([128, 128], DT, name="ident", tag="ident")
        make_identity(nc, ident[:, :])
        b.ident = ident
        bias_eps = b.p_wts.tile([128, 1], DT, name="bias_eps", tag="bias_eps")
        nc.gpsimd.memset(bias_eps[:, :], 1e-38)
        b.bias_eps = bias_eps

        for name in MAT_SHAPES:
            b.load_mat(name, ext[name])
        for lvl in (3, 4, 5):
            b.load_mat(f"IABF{lvl}", ext[f"IAB{lvl}"],
                       splits=[(0, 2 * NS[lvl])])

        for comp in COMPS:
            b.dram[f"xs{comp}"] = ext[f"xs_{comp}"]
            b.dram_tile(f"Af1{comp}", (38, 256, 256))
            af2c = b.dram_tile(f"Af2C{comp}", (95, 131, 131))
            b.dram[f"Af2{comp}"] = af2c[0:26]
            b.dram[f"Af2F{comp}"] = af2c[26:95]
            b.dram_tile(f"Af3{comp}", (76, 69, 69))
            b.dram_tile(f"Af4{comp}", (44, 38, 38))
            b.dram_tile(f"Af5{comp}", (28, 22, 22))
            for af in (0, 1):
                for X in (0, 1):
                    for Y in (0, 1):
                        if af == 0 and X == 0 and Y == 0:
                            continue
                        for lvl, (rn, L) in {1: (19, 131), 2: (13, 69),
                                             3: (38, 38), 4: (22, 22),
                                             5: (14, 14)}.items():
                            b.dram_tile(f"B{lvl}{comp}{af}{X}{Y}", (rn, L, L))
            b.dram_tile(f"B5{comp}000", (14, 14, 14))
            b.dram_tile(f"VA3{comp}", (38, 38, 38))
            b.dram_tile(f"VA4{comp}", (22, 22, 22))
            b.dram_tile(f"VA2full{comp}", (69, 69, 69))
            b.dram_tile(f"VA1rec{comp}", (19, 131, 131))
            b.dram_tile(f"VA2rec{comp}", (69, 69, 69))
            b.dram_tile(f"VA3rec{comp}", (38, 38, 38))
            b.dram_tile(f"VA4rec{comp}", (22, 22, 22))
            for lvl, L in {1: 131, 2: 69, 3: 38, 4: 22, 5: 14}.items():
                for X in (0, 1):
                    for Y in (0, 1):
                        b.dram_tile(f"O{lvl}{comp}{X}{Y}",
                                    (INV_OUT_ROWS[lvl], L, L))
        ag1_in = b.dram_tile("ag1_in", (38, 131, 131))
        ag1_out = b.dram_tile("ag1_out", (NCORE * 38, 131, 131),
                              addr_space="Shared")

        # ============ forward ============
        b.fwd_a("A1T", {c: f"xs{c}" for c in COMPS},
                {c: f"Af1{c}" for c in COMPS}, 38, 256)

        def bd1(comp, bg, X, Y, rx0, h):
            af, br = (0, bg) if bg < 19 else (1, bg - 19)
            if af == 0 and X == 0 and Y == 0:
                ci = 0 if comp == "re" else 1
                return [(ag1_in[ci * 19 + br, rx0:rx0 + h, :], False)]
            return [(b.dram[f"B1{comp}{af}{X}{Y}"][br, rx0:rx0 + h, :], True)]

        b.bc_fwd(1, 38, bd1)

        if use_collective[0]:
            nc.gpsimd.collective_compute(
                "AllGather", mybir.AluOpType.bypass,
                ins=[ag1_in.opt()], outs=[ag1_out.opt()],
                replica_groups=[list(range(NCORE))])
        else:
            nc.sync.dma_start(ag1_out[0:38], ag1_in[0:38])

        # merged L2 a-pass straight off ag1_out (no VA1full materialization)
        b.fwd_a2_merged(ag1_out)

        def bd2(comp, bg, X, Y, rx0, h):
            af, br = (0, bg) if bg < 13 else (1, bg - 13)
            if af == 0 and X == 0 and Y == 0:
                return []    # full aaa2 is recomputed replicated below
            return [(b.dram[f"B2{comp}{af}{X}{Y}"][br, rx0:rx0 + h, :], True)]

        b.bc_fwd(2, 26, bd2)
        b.bc_ll_l2()

        def bd_rep(lvl, half_rows, va_name):
            def f(comp, bg, X, Y, rx0, h):
                af, br = (0, bg) if bg < half_rows else (1, bg - half_rows)
                if af == 0 and X == 0 and Y == 0:
                    if lvl == 5:
                        return [(b.dram[f"B5{comp}000"][br, rx0:rx0 + h, :],
                                 True)]
                    return [(b.dram[f"{va_name}{comp}"][br, rx0:rx0 + h, :],
                             False)]
                return [(b.dram[f"B{lvl}{comp}{af}{X}{Y}"][br, rx0:rx0 + h, :],
                         True)]
            return f

        b.fwd_a("WT3", {c: f"VA2full{c}" for c in COMPS},
                {c: f"Af3{c}" for c in COMPS}, 76, 69)
        b.bc_fwd(3, 76, bd_rep(3, 38, "VA3"))
        b.fwd_a("WT4", {c: f"VA3{c}" for c in COMPS},
                {c: f"Af4{c}" for c in COMPS}, 44, 38)
        b.bc_fwd(4, 44, bd_rep(4, 22, "VA4"))
        b.fwd_a("WT5", {c: f"VA4{c}" for c in COMPS},
                {c: f"Af5{c}" for c in COMPS}, 28, 22)
        b.bc_fwd(5, 28, bd_rep(5, 14, None))

        # ============ inverse ============
        def bsrc_rep(lvl, va_rec):
            L = NS[lvl]

            def f(comp, X, Y):
                if X == 0 and Y == 0:
                    A = (b.dram[f"B5{comp}000"] if lvl == 5
                         else b.dram[va_rec + comp])
                else:
                    A = b.dram[f"B{lvl}{comp}0{X}{Y}"]
                D = b.dram[f"B{lvl}{comp}1{X}{Y}"]
                return (A.rearrange("a b c -> a (b c)"), L,
                        D.rearrange("a b c -> a (b c)"), L, f"IABF{lvl}")
            return f

        b.inv_a(5, bsrc_rep(5, None))
        b.inv_bc(5, lambda comp: b.dram[f"VA4rec{comp}"])
        b.inv_a(4, bsrc_rep(4, "VA4rec"))
        b.inv_bc(4, lambda comp: b.dram[f"VA3rec{comp}"])
        b.inv_a(3, bsrc_rep(3, "VA3rec"))
        b.inv_bc(3, lambda comp: b.dram[f"VA2rec{comp}"])

        def bsrc2(comp, X, Y):
            D = b.dram[f"B2{comp}1{X}{Y}"].rearrange("a b c -> a (b c)")
            if X == 0 and Y == 0:
                A = b.dram[f"VA2rec{comp}"].rearrange("a b c -> a (b c)")
                return (A, 69, D, 13, "IA2LL")
            A = b.dram[f"B2{comp}0{X}{Y}"].rearrange("a b c -> a (b c)")
            return (A, 13, D, 13, "IA2")

        b.inv_a(2, bsrc2)
        b.inv_bc(2, lambda comp: b.dram[f"VA1rec{comp}"])

        def bsrc1(comp, X, Y):
            D = b.dram[f"B1{comp}1{X}{Y}"].rearrange("a b c -> a (b c)")
            if X == 0 and Y == 0:
                A = b.dram[f"VA1rec{comp}"].rearrange("a b c -> a (b c)")
            else:
                A = b.dram[f"B1{comp}0{X}{Y}"].rearrange("a b c -> a (b c)")
            return (A, 19, D, 19, "IA1")

        b.inv_a(1, bsrc1)
        b.inv_bc(1, lambda comp: outs[comp])

    nc.compile()
    return nc


_CACHE = {}


def make_in_maps(x_real, x_imag):
    x_real = np.ascontiguousarray(x_real, dtype=np.float32)
    x_imag = np.ascontiguousarray(x_imag, dtype=np.float32)
    in_maps = []
    for c in range(NCORE):
        m = host_matrices(c)
        slab_lo = 32 * c - 6
        im = {}
        for comp, x in (("re", x_real), ("im", x_imag)):
            s = np.zeros((44, 256, 256), dtype=np.float32)
            g0, g1 = max(0, slab_lo), min(256, slab_lo + 44)
            s[g0 - slab_lo:g1 - slab_lo] = x[g0:g1]
            im[f"xs_{comp}"] = s
        im.update(m)
        in_maps.append(im)
    return in_maps


def kernel(x_real, x_imag, alpha):
    thresh = 1e-3 * float(np.asarray(alpha))
    if thresh not in _CACHE:
        _CACHE[thresh] = build_program(thresh)
    nc = _CACHE[thresh]

    in_maps = make_in_maps(x_real, x_imag)
    res = run_bass_kernel_spmd(nc, in_maps, core_ids=list(range(NCORE)))
    out = np.empty((256, 256, 256), dtype=np.complex64)
    for c in range(NCORE):
        r = res.results[c]
        out[32 * c:32 * c + 32] = r["out_re"] + 1j * r["out_im"]
    return out



# revision 86
# speedup vs baseline: 1.4211x; 1.0506x over previous
"""Trainium2 Bass kernel for nn_L1Wav: 5-level 3D db4 wavelet soft-threshold
denoising of a 256^3 complex volume, SPMD over 8 NeuronCores.

Math notes (verified against the jax reference in a numpy sim):
  - The deterministic rng(1000) shift is 0 and the unit-modulus phase cancels
    through the prox (DWT is real-linear; |phase*w| = |w|), so the computation
    is exactly: 5-level 3D DWT -> complex soft-threshold -> inverse DWT.
  - Every 1D DWT/IDWT pass is a matmul against a banded filter matrix.
  - Sharding: volume split along axis 0 (32 planes/core). All a-axis passes
    use per-core weight-matrix slices, so the core-dependence lives entirely
    in host-provided matrices and one SPMD program serves all cores.
    Levels 1-2 are distributed; levels 3-5 are replicated on every core.
    The only communication is two small AllGathers of approx bands.

Level sizes: 256 -> 131 -> 69 -> 38 -> 22 -> 14.
Per-core windows: L1 band rows [16c,16c+19); L2 band rows [8c,8c+13);
output rows [32c,32c+32); input slab rows [32c-6,32c+38) zero-padded.

Layout: a volume at any level is stored (p, q, r). The forward a-pass
contracts p; the per-row bc-pass transforms q then r, emitting tiles
(r', q'), so child band tensors are stored (a_row, r', q').
"""
import sys
from contextlib import ExitStack

import numpy as np

sys.path.insert(0, "/opt/trn_rl_repo")

import concourse.bass as bass
import concourse.mybir as mybir
import concourse.tile as tile
from concourse import bacc
from concourse.bass_utils import run_bass_kernel_spmd
from concourse.masks import make_identity

DT = mybir.dt.float32
R32 = mybir.dt.float32r
F = 8
DEC_LO = np.array([-0.010597401784997278, 0.032883011666982945, 0.030841381835986965,
                   -0.18703481171888114, -0.02798376941698385, 0.6308807679295904,
                   0.7148465705525415, 0.23037781330885523])
REC_LO = DEC_LO[::-1].copy()
REC_HI = np.array([((-1) ** n) * DEC_LO[n] for n in range(F)])
DEC_HI = REC_HI[::-1].copy()

NS = [256, 131, 69, 38, 22, 14]     # sizes level 0..5
NCORE = 8
COMPS = ("re", "im")
BC_BS = {1: 1, 2: 3, 3: 6, 4: 11, 5: 14}       # fwd bc row batch
IBC_BS = {1: 2, 2: 5, 3: 7, 4: 13, 5: 14}      # inv bc row batch
INV_OUT_ROWS = {1: 32, 2: 19, 3: 69, 4: 38, 5: 22}


def W_mat(N, flt):
    L = (N + F - 1) // 2
    W = np.zeros((L, N), dtype=np.float32)
    for l in range(L):
        for j in range(F):
            n = 2 * l + 1 - j
            if 0 <= n < N:
                W[l, n] = flt[j]
    return W


def G_mat(L, crop, flt):
    G = np.zeros((crop, L), dtype=np.float32)
    for t in range(crop):
        for m in range(L):
            j = t + 6 - 2 * m
            if 0 <= j < F:
                G[t, m] = flt[j]
    return G


def pad_even(a):
    """Pad the last (M) dim to an even count (fp32r stationary rule)."""
    if a.shape[-1] % 2:
        a = np.pad(a, [(0, 0)] * (a.ndim - 1) + [(0, 1)])
    return a


def host_matrices(core):
    """All weight matrices for one core (lhsT layout: (K, M))."""
    c = core
    m = {}
    for l in range(5):
        W2 = np.concatenate([W_mat(NS[l], DEC_LO), W_mat(NS[l], DEC_HI)], 0)
        m[f"WT{l + 1}"] = np.ascontiguousarray(W2.T)
        glo = G_mat(NS[l + 1], NS[l], REC_LO)
        ghi = G_mat(NS[l + 1], NS[l], REC_HI)
        m[f"IAB{l + 1}"] = pad_even(np.ascontiguousarray(
            np.concatenate([glo.T, ghi.T], 0)))
    # L1 fwd a-pass (per-core): A1 (38, 44) -> lhsT (44, 38)
    A1 = np.zeros((38, 44), dtype=np.float32)
    slab_lo = 32 * c - 6
    for half, flt in ((0, DEC_LO), (1, DEC_HI)):
        for i in range(19):
            l = 16 * c + i
            for k in range(44):
                n = slab_lo + k
                j = 2 * l + 1 - n
                if 0 <= j < F and 0 <= n < 256:
                    A1[half * 19 + i, k] = flt[j]
    m["A1T"] = np.ascontiguousarray(A1.T)
    # L2 fwd a-pass, merged with the replicated full-lo pass, contracting
    # directly over ag1_out rows (38k+19ci+i = VA1full row 16k+i, owner
    # k = min(row//16, 7)).  M = 26 per-core band rows + 69 full-lo rows.
    A2 = np.concatenate([W_mat(131, DEC_LO)[8 * c:8 * c + 13],
                         W_mat(131, DEC_HI)[8 * c:8 * c + 13]], 0)
    M95 = np.concatenate([A2, W_mat(131, DEC_LO)], 0)       # (95, 131)
    for ci in range(2):
        A2WT = np.zeros((304, 96), dtype=np.float32)
        for r in range(131):
            k = min(r // 16, 7)
            A2WT[38 * k + 19 * ci + (r - 16 * k), :95] = M95[:, r]
        m["A2WTre" if ci == 0 else "A2WTim"] = A2WT
    # L1 inv a-pass: core-independent (38, 32)
    G1a = np.zeros((32, 19), dtype=np.float32)
    G1d = np.zeros((32, 19), dtype=np.float32)
    for u in range(32):
        for v in range(19):
            j = u + 6 - 2 * v
            if 0 <= j < F:
                G1a[u, v] = REC_LO[j]
                G1d[u, v] = REC_HI[j]
    m["IA1"] = np.ascontiguousarray(np.concatenate([G1a.T, G1d.T], 0))
    # L2 inv a-pass (per-core)
    glo1 = G_mat(69, 131, REC_LO)
    ghi1 = G_mat(69, 131, REC_HI)
    g2a_full = glo1[16 * c:16 * c + 19, :]                    # (19, 69)
    g2a13 = glo1[16 * c:16 * c + 19, 8 * c:8 * c + 13]
    g2d13 = ghi1[16 * c:16 * c + 19, 8 * c:8 * c + 13]
    m["IA2"] = pad_even(np.ascontiguousarray(
        np.concatenate([g2a13.T, g2d13.T], 0)))
    m["IA2LL"] = pad_even(np.ascontiguousarray(
        np.concatenate([g2a_full.T, g2d13.T], 0)))
    return {k: v.astype(np.float32) for k, v in m.items()}


MAT_SHAPES = {k: v.shape for k, v in host_matrices(0).items()}
# partition-chunk splits for SBUF-resident matrices (K dim)
MAT_SPLITS = {
    "IAB1": [(0, 128), (128, 3), (131, 128), (259, 3)],
    "IAB2": [(0, 69), (69, 69)],
    "IAB3": [(0, 38), (38, 38)],
    "IAB4": [(0, 22), (22, 22)],
    "IAB5": [(0, 14), (14, 14)],
}


def chunks_of(total, size=128):
    return [(i, min(size, total - i)) for i in range(0, total, size)]


class Builder:
    def __init__(self, nc, tc, ctx, thresh):
        self.nc = nc
        self.tc = tc
        self.thresh = float(thresh)
        self.p_dram = ctx.enter_context(
            tc.tile_pool(name="dram", bufs=1, space=bass.MemorySpace.DRAM))
        self.p_wts = ctx.enter_context(tc.tile_pool(name="wts", bufs=1))
        self.p_work = ctx.enter_context(tc.tile_pool(name="work", bufs=1))
        self.p_psum = ctx.enter_context(
            tc.tile_pool(name="psum", bufs=1, space=bass.MemorySpace.PSUM))
        self.mats = {}
        self.dram = {}
        self.uid = 0
        self._dmai = 0

    def _id(self):
        self.uid += 1
        return self.uid

    def dram_tile(self, name, shape, addr_space="Local"):
        t = self.p_dram.tile(list(shape), DT, name=name, tag=name,
                             addr_space=addr_space)
        self.dram[name] = t
        return t

    def sbuf(self, shape, tag, bufs=1, dt=DT):
        return self.p_work.tile(list(shape), dt, name=f"t{self._id()}",
                                tag=tag, bufs=bufs)

    def psum(self, shape, tag):
        return self.p_psum.tile(list(shape), DT, name=f"p{self._id()}",
                                tag=tag, bufs=1)

    # fp32r matmul: 1 cycle/row (vs 4 for fp32) when moving free size >= 256.
    # Operand tiles are declared float32r (bit-identical to fp32); the BIR
    # verifier requires producers to carry the fp32r dtype, so loads bitcast
    # the DRAM AP and psum->sbuf copies write fp32r-typed tiles.
    def mm(self, out, lhsT, rhs, **kw):
        self.nc.tensor.matmul(out, lhsT, rhs, **kw)

    # DMA triggers: each dma_start costs ~630ns of serialized queue time.
    # (Round-robin over SP+Activation HWDGE queues corrupted early deep-level
    # reads — cross-queue write->read ordering was not honored — so all
    # triggers stay on the SP queue.)
    def _dmaq(self):
        self._dmai += 1
        return (self.nc.sync, self.nc.scalar)[self._dmai & 1]

    def dma(self, dst, src):
        self._dmaq().dma_start(dst, src)

    def dmaR(self, dst, src):
        self._dmaq().dma_start(dst, src.bitcast(R32))

    def load_mat(self, name, dram_ap, splits=None):
        K, M = dram_ap.shape
        if splits is None:
            splits = MAT_SPLITS.get(name, chunks_of(K))
        tiles = []
        for (k0, kn) in splits:
            t = self.p_wts.tile([kn, M], R32, name=f"{name}_{k0}",
                                tag=f"{name}_{k0}", bufs=1)
            self.dmaR(t[:, :], dram_ap[k0:k0 + kn, :])
            tiles.append((t, k0, kn))
        self.mats[name] = tiles

    # ---- soft threshold: returns thresholded (re, im) tiles (full-shape)
    def soft_pair(self, s_re, s_im, shape, gb):
        nc = self.nc
        t = self.thresh
        mn = shape[0]
        tmp1 = self.sbuf(shape, "sm1")
        tmp2 = self.sbuf(shape, "sm2")
        a = tmp1[:, :gb, :]
        m = tmp2[:, :gb, :]
        nc.vector.tensor_mul(a, s_re, s_re)
        nc.vector.tensor_mul(m, s_im, s_im)
        nc.vector.tensor_add(a, a, m)
        nc.scalar.activation(m, a, mybir.ActivationFunctionType.Sqrt,
                             bias=self.bias_eps[:mn, :])
        nc.vector.tensor_scalar(a, m, -t, 0.0,
                                mybir.AluOpType.add, mybir.AluOpType.max)
        nc.vector.reciprocal(m, m)
        nc.vector.tensor_mul(a, a, m)
        th_re = self.sbuf(shape, "str", bufs=2)
        th_im = self.sbuf(shape, "sti", bufs=2)
        nc.vector.tensor_mul(th_re[:, :gb, :], s_re, a)
        nc.vector.tensor_mul(th_im[:, :gb, :], s_im, a)
        return th_re, th_im

    # ---- forward a-pass: out (M, n, n) = lhsT^T @ in (K, n, n)
    # Supertiled: one DMA load/store per `lds` columns, matmuls per `ntile`.
    def fwd_a(self, lname, in_keys, out_keys, M, n, ntile=512, lds=2048):
        nc = self.nc
        lhsT = self.mats[lname]
        for comp in COMPS:
            srcf = self.dram[in_keys[comp]].rearrange("a b c -> a (b c)")
            dstf = self.dram[out_keys[comp]].rearrange("a b c -> a (b c)")
            tot = n * n
            for t0 in range(0, tot, lds):
                ls = min(lds, tot - t0)
                rts = []
                for i, (lt, k0, kn) in enumerate(lhsT):
                    rt = self.sbuf([kn, lds], f"fa_in_{i}", bufs=2, dt=R32)
                    self.dmaR(rt[:, :ls], srcf[k0:k0 + kn, t0:t0 + ls])
                    rts.append(rt)
                s = self.sbuf([M, lds], "fa_o", bufs=2)
                for ui, u0 in enumerate(range(0, ls, ntile)):
                    tn = min(ntile, ls - u0)
                    tp = tn + (tn & 1)  # fp32r needs even psum width
                    p = self.psum([M, ntile], f"P{ui & 1}")
                    for i, (lt, k0, kn) in enumerate(lhsT):
                        self.mm(p[:, :tp], lt[:, :], rts[i][:, u0:u0 + tp],
                                start=(i == 0), stop=(i == len(lhsT) - 1))
                    nc.scalar.copy(s[:, u0:u0 + tn], p[:, :tn])
                self.dma(dstf[:, t0:t0 + ls], s[:, :ls])

    # ---- merged L2 a-pass: contract ag1_out rows directly; outputs both
    # the per-core 26 band rows and the replicated 69 full-lo rows.
    def fwd_a2_merged(self, ag_out, ntile=512, lds=1024):
        nc = self.nc
        lhs = {c: self.mats[f"A2WT{c}"] for c in COMPS}
        src = ag_out.rearrange("a b c -> a (b c)")
        dsts = {c: self.dram[f"Af2C{c}"].rearrange("a b c -> a (b c)")
                for c in COMPS}
        tot = 131 * 131
        for t0 in range(0, tot, lds):
            ls = min(lds, tot - t0)
            rts = []
            for i, (lt, k0, kn) in enumerate(lhs["re"]):
                rt = self.sbuf([kn, lds], f"fa2_in_{i}", bufs=2, dt=R32)
                self.dmaR(rt[:, :ls], src[k0:k0 + kn, t0:t0 + ls])
                rts.append(rt)
            for pi, comp in enumerate(COMPS):
                lT = lhs[comp]
                s = self.sbuf([95, lds], f"fa2_o_{comp}", bufs=2)
                for ui, u0 in enumerate(range(0, ls, ntile)):
                    tn = min(ntile, ls - u0)
                    tp = tn + (tn & 1)
                    p = self.psum([96, ntile], f"P{2 * pi + (ui & 1)}")
                    for i, (lt, k0, kn) in enumerate(lT):
                        self.mm(p[:, :tp], lt[:, :], rts[i][:, u0:u0 + tp],
                                start=(i == 0), stop=(i == len(lT) - 1))
                    nc.scalar.copy(s[:, u0:u0 + tn], p[0:95, :tn])
                self.dma(dsts[comp][:, t0:t0 + ls], s[:, :ls])

    # ---- forward bc-pass for one level
    def bc_fwd(self, lvl, rows, band_dest):
        nc = self.nc
        bs = BC_BS[lvl]
        Q = NS[lvl - 1]
        Qp = Q + (Q & 1)                # fp32r even-width padding
        L = NS[lvl]
        twoL = 2 * L
        WT = self.mats[f"WT{lvl}"]
        qch = chunks_of(Q)
        mch = chunks_of(twoL)
        half = rows // 2
        for af, g0 in [(a, g) for a in (0, 1) for g in range(0, half, bs)]:
            gb = min(bs, half - g0)
            ga = af * half + g0         # absolute a-row of this batch
            S3 = {}
            for comp in COMPS:
                src = self.dram[f"Af{lvl}{comp}"]
                ins = []
                for qi, (q0, qn) in enumerate(qch):
                    it = self.sbuf([qn, bs, Qp], f"bci_{qi}", bufs=2, dt=R32)
                    sap = src[ga:ga + gb, q0:q0 + qn, :].rearrange(
                        "b q n -> q b n")
                    self.dmaR(it[:, :gb, :Q], sap)
                    ins.append(it)
                # M1: transform q -> (twoL chunks, gb, Q)
                s1 = []
                for mi, (m0, mn) in enumerate(mch):
                    p = self.psum([mn, bs, Qp], f"P{mi}")
                    for ki in range(len(qch)):
                        self.mm(p[:, :gb, :],
                                WT[ki][0][:, m0:m0 + mn],
                                ins[ki][:, :gb, :],
                                start=(ki == 0),
                                stop=(ki == len(qch) - 1))
                    s = self.sbuf([mn, bs, Q], f"bs1_{mi}")
                    nc.scalar.copy(s[:, :gb, :], p[:, :gb, :Q])
                    s1.append(s)
                # transpose -> (Q chunks, gb, twoL)
                pT = [self.psum([fn, bs, twoL], f"P{3 + fi}")
                      for fi, (f0, fn) in enumerate(qch)]
                for b in range(gb):
                    for mi, (m0, mn) in enumerate(mch):
                        for fi, (f0, fn) in enumerate(qch):
                            nc.tensor.transpose(
                                pT[fi][0:fn, b, m0:m0 + mn],
                                s1[mi][:, b, f0:f0 + fn],
                                self.ident[:mn, :mn])
                s2 = []
                for fi, (f0, fn) in enumerate(qch):
                    s = self.sbuf([fn, bs, twoL], f"bs2_{fi}", dt=R32)
                    nc.scalar.copy(s[:, :gb, :], pT[fi][:, :gb, :])
                    s2.append(s)
                # M2: transform r -> (twoL chunks, gb, twoL)
                S3[comp] = []
                for mi, (m0, mn) in enumerate(mch):
                    p = self.psum([mn, bs, twoL], f"P{5 + mi}")
                    for ki in range(len(qch)):
                        self.mm(p[:, :gb, :],
                                WT[ki][0][:, m0:m0 + mn],
                                s2[ki][:, :gb, :],
                                start=(ki == 0),
                                stop=(ki == len(qch) - 1))
                    s = self.sbuf([mn, bs, twoL], f"bs3_{comp}_{mi}")
                    nc.scalar.copy(s[:, :gb, :], p[:, :gb, :])
                    S3[comp].append(s)
            TH = {"re": [], "im": []}
            for mi, (m0, mn) in enumerate(mch):
                tr, ti = self.soft_pair(S3["re"][mi][:, :gb, :],
                                        S3["im"][mi][:, :gb, :],
                                        [mn, bs, twoL], gb)
                TH["re"].append(tr)
                TH["im"].append(ti)
            for comp in COMPS:
                for mi, (m0, mn) in enumerate(mch):
                    for X in (0, 1):
                        lo = max(m0, X * L)
                        hi = min(m0 + mn, (X + 1) * L)
                        if lo >= hi:
                            continue
                        rr0, h = lo - m0, hi - lo
                        rx0 = lo - X * L
                        for Y in (0, 1):
                            for dest, use_th in band_dest(
                                    comp, af, g0, gb, X, Y, rx0, h):
                                st = TH[comp][mi] if use_th else S3[comp][mi]
                                for bb in range(gb):
                                    self.dma(
                                        dest[bb],
                                        st[rr0:rr0 + h, bb,
                                           Y * L:(Y + 1) * L])

    # ---- replicated lo-lo-lo quadrant of L2 (full 69 rows) -> VA2full
    def bc_ll_l2(self):
        nc = self.nc
        bs = 3
        Q, L = 131, 69
        Qp, Lp = Q + 1, L + 1
        WT = self.mats["WT2"]
        qch = chunks_of(Q)
        for comp in COMPS:
            src = self.dram[f"Af2F{comp}"]
            dst = self.dram[f"VA2full{comp}"]
            for g0 in range(0, L, bs):
                gb = min(bs, L - g0)
                ins = []
                for qi, (q0, qn) in enumerate(qch):
                    it = self.sbuf([qn, bs, Qp], f"bci_{qi}", bufs=2, dt=R32)
                    sap = src[g0:g0 + gb, q0:q0 + qn, :].rearrange(
                        "b q n -> q b n")
                    self.dmaR(it[:, :gb, :Q], sap)
                    ins.append(it)
                p = self.psum([Lp, bs, Qp], "P0")
                for ki in range(len(qch)):
                    self.mm(p[:, :gb, :], WT[ki][0][:, 0:Lp],
                            ins[ki][:, :gb, :], start=(ki == 0),
                            stop=(ki == len(qch) - 1))
                s1 = self.sbuf([L, bs, Q], "bs1_0")
                nc.scalar.copy(s1[:, :gb, :], p[0:L, :gb, :Q])
                pT = [self.psum([fn, bs, L], f"P{3 + fi}")
                      for fi, (f0, fn) in enumerate(qch)]
                for b in range(gb):
                    for fi, (f0, fn) in enumerate(qch):
                        nc.tensor.transpose(pT[fi][0:fn, b, 0:L],
                                            s1[:, b, f0:f0 + fn],
                                            self.ident[:L, :L])
                s2 = []
                for fi, (f0, fn) in enumerate(qch):
                    s = self.sbuf([fn, bs, Lp], f"bs2_{fi}", dt=R32)
                    nc.scalar.copy(s[:, :gb, :L], pT[fi][:, :gb, :])
                    s2.append(s)
                p2 = self.psum([Lp, bs, Lp], "P5")
                for ki in range(len(qch)):
                    self.mm(p2[:, :gb, :], WT[ki][0][:, 0:Lp],
                            s2[ki][:, :gb, :], start=(ki == 0),
                            stop=(ki == len(qch) - 1))
                s3 = self.sbuf([L, bs, L], "bs3_re_0")
                nc.scalar.copy(s3[:, :gb, :], p2[0:L, :gb, :L])
                for bb in range(gb):
                    self.dma(dst[g0 + bb, :, :], s3[:, bb, :])

    # ---- inverse a-pass (A and D band loads into one stacked sbuf tile)
    def inv_a(self, lvl, band_src, ntile=512, lds=2048):
        nc = self.nc
        L = NS[lvl]
        M = INV_OUT_ROWS[lvl]
        Mp = M + (M & 1)
        tot = L * L
        for comp in COMPS:
            for X in (0, 1):
                for Y in (0, 1):
                    A_ap, KA, D_ap, KD, lname = band_src(comp, X, Y)
                    lt = self.mats[lname][0][0]
                    dst = self.dram[f"O{lvl}{comp}{X}{Y}"].rearrange(
                        "a b c -> a (b c)")
                    for t0 in range(0, tot, lds):
                        ls = min(lds, tot - t0)
                        rt = self.sbuf([KA + KD, lds], "ia_in", bufs=2,
                                       dt=R32)
                        self.dmaR(rt[0:KA, :ls], A_ap[:, t0:t0 + ls])
                        self.dmaR(rt[KA:KA + KD, :ls], D_ap[:, t0:t0 + ls])
                        s = self.sbuf([M, lds], "ia_o", bufs=2)
                        for ui, u0 in enumerate(range(0, ls, ntile)):
                            tn = min(ntile, ls - u0)
                            tp = tn + (tn & 1)
                            p = self.psum([Mp, ntile], f"P{6 + (ui & 1)}")
                            self.mm(p[:, :tp], lt[:, :], rt[:, u0:u0 + tp],
                                    start=True, stop=True)
                            nc.scalar.copy(s[:, u0:u0 + tn],
                                                  p[0:M, :tn])
                        self.dma(dst[:, t0:t0 + ls], s[:, :ls])

    # ---- inverse bc-pass: O tensors (rows, L, L) -> parent rows (rows, P, P)
    def inv_bc(self, lvl, out_dest):
        nc = self.nc
        rows = INV_OUT_ROWS[lvl]
        bs = IBC_BS[lvl]
        L = NS[lvl]
        P = NS[lvl - 1]
        Lp = L + (L & 1)
        Pp = P + (P & 1)
        IAB = self.mats[f"IAB{lvl}"]
        lch = chunks_of(L)
        pch = chunks_of(Pp)             # even chunks (fp32r stationary rule)

        def iab_slice(half, l0, ln, m0, mn):
            r0 = half * L + l0
            for (t, k0, kn) in IAB:
                if k0 <= r0 and r0 + ln <= k0 + kn:
                    return t[r0 - k0:r0 - k0 + ln, m0:m0 + mn]
            raise AssertionError(f"IAB{lvl} chunk misaligned {half} {l0} {ln}")

        for comp in COMPS:
            dst = out_dest(comp)
            for g0 in range(0, rows, bs):
                gb = min(bs, rows - g0)
                ot = {}
                for X in (0, 1):
                    for Y in (0, 1):
                        src = self.dram[f"O{lvl}{comp}{X}{Y}"]
                        for li, (l0, ln) in enumerate(lch):
                            t = self.sbuf([ln, bs, Lp], f"ibi_{X}{Y}_{li}",
                                          dt=R32)
                            sap = src[g0:g0 + gb, l0:l0 + ln, :].rearrange(
                                "b l n -> l b n")
                            self.dmaR(t[:, :gb, :L], sap)
                            ot[(X, Y, li)] = t
                sU = {}
                for Y in (0, 1):
                    sU[Y] = []
                    for mi, (m0, mn) in enumerate(pch):
                        p = self.psum([mn, bs, Lp], f"P{mi}")
                        nkt = 2 * len(lch)
                        ki = 0
                        for X in (0, 1):
                            for li, (l0, ln) in enumerate(lch):
                                self.mm(
                                    p[:, :gb, :],
                                    iab_slice(X, l0, ln, m0, mn),
                                    ot[(X, Y, li)][:, :gb, :],
                                    start=(ki == 0), stop=(ki == nkt - 1))
                                ki += 1
                        s = self.sbuf([mn, bs, L], f"ibsu_{Y}_{mi}")
                        nc.scalar.copy(s[:, :gb, :], p[:, :gb, :L])
                        sU[Y].append(s)
                sT = {}
                gsub = max(1, min(bs, 512 // Pp))  # b-rows per transpose psum
                for Y in (0, 1):
                    sT[Y] = [self.sbuf([ln, bs, Pp], f"ibst_{Y}_{li}", dt=R32)
                             for li, (l0, ln) in enumerate(lch)]
                    for b0 in range(0, gb, gsub):
                        bn = min(gsub, gb - b0)
                        pT = [self.psum([ln, gsub, Pp], f"P{2 + li}")
                              for li, (l0, ln) in enumerate(lch)]
                        for b in range(b0, b0 + bn):
                            for mi, (m0, mn) in enumerate(pch):
                                for li, (l0, ln) in enumerate(lch):
                                    nc.tensor.transpose(
                                        pT[li][0:ln, b - b0, m0:m0 + mn],
                                        sU[Y][mi][:, b, l0:l0 + ln],
                                        self.ident[:mn, :mn])
                        for li, (l0, ln) in enumerate(lch):
                            nc.scalar.copy(sT[Y][li][:, b0:b0 + bn, :],
                                           pT[li][:, :bn, :])
                # final matmul, column-chunked (even widths, one psum bank)
                nfch = max(1, -(-(bs * Pp * 4) // 2048))
                fw = -(-Pp // nfch)
                fw += fw & 1
                fch = chunks_of(Pp, fw)
                for mi, (m0, mn) in enumerate(pch):
                    s = self.sbuf([mn, bs, Pp], f"ibs3_{mi}", bufs=2)
                    for fi, (f0, fn) in enumerate(fch):
                        p = self.psum([mn, bs, fn], f"P{(6, 4)[fi] + mi}")
                        nkt = 2 * len(lch)
                        ki = 0
                        for Y in (0, 1):
                            for li, (l0, ln) in enumerate(lch):
                                self.mm(
                                    p[:, :gb, :],
                                    iab_slice(Y, l0, ln, m0, mn),
                                    sT[Y][li][:, :gb, f0:f0 + fn],
                                    start=(ki == 0), stop=(ki == nkt - 1))
                                ki += 1
                        nc.scalar.copy(s[:, :gb, f0:f0 + fn],
                                              p[:, :gb, :])
                    rmn = min(mn, P - m0)
                    for bb in range(gb):
                        self.dma(dst[g0 + bb, m0:m0 + rmn, :],
                                 s[0:rmn, bb, :P])


def build_program(thresh, use_collective=(True, True), debug_dump=False):
    if isinstance(use_collective, bool):
        use_collective = (use_collective, use_collective)
    nc = bacc.Bacc("TRN2", target_bir_lowering=False, debug=False,
                   num_devices=NCORE)
    ext = {}
    for comp in COMPS:
        ext[f"xs_{comp}"] = nc.dram_tensor(f"xs_{comp}", [44, 256, 256], DT,
                                           kind="ExternalInput").ap()
    for name, shp in MAT_SHAPES.items():
        ext[name] = nc.dram_tensor(name, list(shp), DT,
                                   kind="ExternalInput").ap()
    outs = {}
    for comp in COMPS:
        outs[comp] = nc.dram_tensor(f"out_{comp}", [32, 256, 256], DT,
                                    kind="ExternalOutput").ap()

    with tile.TileContext(nc) as tc, ExitStack() as ctx:
        b = Builder(nc, tc, ctx, thresh)

        ident = b.p_wts.tile([128, 128], DT, name="ident", tag="ident")
        make_identity(nc, ident[:, :])
        b.ident = ident
        bias_eps = b.p_wts.tile([128, 1], DT, name="bias_eps", tag="bias_eps")
        nc.gpsimd.memset(bias_eps[:, :], 1e-38)
        b.bias_eps = bias_eps

        for name in MAT_SHAPES:
            b.load_mat(name, ext[name])
        for lvl in (3, 4, 5):
            b.load_mat(f"IABF{lvl}", ext[f"IAB{lvl}"],
                       splits=[(0, 2 * NS[lvl])])

        for comp in COMPS:
            b.dram[f"xs{comp}"] = ext[f"xs_{comp}"]
            b.dram_tile(f"Af1{comp}", (38, 256, 256))
            af2c = b.dram_tile(f"Af2C{comp}", (95, 131, 131))
            b.dram[f"Af2{comp}"] = af2c[0:26]
            b.dram[f"Af2F{comp}"] = af2c[26:95]
            b.dram_tile(f"Af3{comp}", (76, 69, 69))
            b.dram_tile(f"Af4{comp}", (44, 38, 38))
            b.dram_tile(f"Af5{comp}", (28, 22, 22))
            for af in (0, 1):
                for X in (0, 1):
                    for Y in (0, 1):
                        if af == 0 and X == 0 and Y == 0:
                            continue
                        for lvl, (rn, L) in {1: (19, 131), 2: (13, 69),
                                             3: (38, 38), 4: (22, 22),
                                             5: (14, 14)}.items():
                            b.dram_tile(f"B{lvl}{comp}{af}{X}{Y}", (rn, L, L))
            b.dram_tile(f"B5{comp}000", (14, 14, 14))
            b.dram_tile(f"VA3{comp}", (38, 38, 38))
            b.dram_tile(f"VA4{comp}", (22, 22, 22))
            b.dram_tile(f"VA2full{comp}", (69, 69, 69))
            b.dram_tile(f"VA1rec{comp}", (19, 131, 131))
            b.dram_tile(f"VA2rec{comp}", (69, 69, 69))
            b.dram_tile(f"VA3rec{comp}", (38, 38, 38))
            b.dram_tile(f"VA4rec{comp}", (22, 22, 22))
            for lvl, L in {1: 131, 2: 69, 3: 38, 4: 22, 5: 14}.items():
                for X in (0, 1):
                    for Y in (0, 1):
                        b.dram_tile(f"O{lvl}{comp}{X}{Y}",
                                    (INV_OUT_ROWS[lvl], L, L))
        ag1_in = b.dram_tile("ag1_in", (38, 131, 131))
        ag1_out = b.dram_tile("ag1_out", (NCORE * 38, 131, 131),
                              addr_space="Shared")

        # ============ forward ============
        b.fwd_a("A1T", {c: f"xs{c}" for c in COMPS},
                {c: f"Af1{c}" for c in COMPS}, 38, 256)

        def bd1(comp, af, b0, gb, X, Y, rx0, h):
            if af == 0 and X == 0 and Y == 0:
                ci = 0 if comp == "re" else 1
                return [(ag1_in[ci * 19 + b0:ci * 19 + b0 + gb,
                                rx0:rx0 + h, :], False)]
            return [(b.dram[f"B1{comp}{af}{X}{Y}"][b0:b0 + gb,
                                                   rx0:rx0 + h, :], True)]

        b.bc_fwd(1, 38, bd1)

        if use_collective[0]:
            nc.gpsimd.collective_compute(
                "AllGather", mybir.AluOpType.bypass,
                ins=[ag1_in.opt()], outs=[ag1_out.opt()],
                replica_groups=[list(range(NCORE))])
        else:
            nc.sync.dma_start(ag1_out[0:38], ag1_in[0:38])

        # merged L2 a-pass straight off ag1_out (no VA1full materialization)
        b.fwd_a2_merged(ag1_out)

        def bd2(comp, af, b0, gb, X, Y, rx0, h):
            if af == 0 and X == 0 and Y == 0:
                return []    # full aaa2 is recomputed replicated below
            return [(b.dram[f"B2{comp}{af}{X}{Y}"][b0:b0 + gb,
                                                   rx0:rx0 + h, :], True)]

        b.bc_fwd(2, 26, bd2)
        b.bc_ll_l2()

        def bd_rep(lvl, half_rows, va_name):
            def f(comp, af, b0, gb, X, Y, rx0, h):
                if af == 0 and X == 0 and Y == 0:
                    if lvl == 5:
                        return [(b.dram[f"B5{comp}000"][b0:b0 + gb,
                                                        rx0:rx0 + h, :],
                                 True)]
                    return [(b.dram[f"{va_name}{comp}"][b0:b0 + gb,
                                                        rx0:rx0 + h, :],
                             False)]
                return [(b.dram[f"B{lvl}{comp}{af}{X}{Y}"][b0:b0 + gb,
                                                           rx0:rx0 + h, :],
                         True)]
            return f

        b.fwd_a("WT3", {c: f"VA2full{c}" for c in COMPS},
                {c: f"Af3{c}" for c in COMPS}, 76, 69)
        b.bc_fwd(3, 76, bd_rep(3, 38, "VA3"))
        b.fwd_a("WT4", {c: f"VA3{c}" for c in COMPS},
                {c: f"Af4{c}" for c in COMPS}, 44, 38)
        b.bc_fwd(4, 44, bd_rep(4, 22, "VA4"))
        b.fwd_a("WT5", {c: f"VA4{c}" for c in COMPS},
                {c: f"Af5{c}" for c in COMPS}, 28, 22)
        b.bc_fwd(5, 28, bd_rep(5, 14, None))

        # ============ inverse ============
        def bsrc_rep(lvl, va_rec):
            L = NS[lvl]

            def f(comp, X, Y):
                if X == 0 and Y == 0:
                    A = (b.dram[f"B5{comp}000"] if lvl == 5
                         else b.dram[va_rec + comp])
                else:
                    A = b.dram[f"B{lvl}{comp}0{X}{Y}"]
                D = b.dram[f"B{lvl}{comp}1{X}{Y}"]
                return (A.rearrange("a b c -> a (b c)"), L,
                        D.rearrange("a b c -> a (b c)"), L, f"IABF{lvl}")
            return f

        b.inv_a(5, bsrc_rep(5, None))
        b.inv_bc(5, lambda comp: b.dram[f"VA4rec{comp}"])
        b.inv_a(4, bsrc_rep(4, "VA4rec"))
        b.inv_bc(4, lambda comp: b.dram[f"VA3rec{comp}"])
        b.inv_a(3, bsrc_rep(3, "VA3rec"))
        b.inv_bc(3, lambda comp: b.dram[f"VA2rec{comp}"])

        def bsrc2(comp, X, Y):
            D = b.dram[f"B2{comp}1{X}{Y}"].rearrange("a b c -> a (b c)")
            if X == 0 and Y == 0:
                A = b.dram[f"VA2rec{comp}"].rearrange("a b c -> a (b c)")
                return (A, 69, D, 13, "IA2LL")
            A = b.dram[f"B2{comp}0{X}{Y}"].rearrange("a b c -> a (b c)")
            return (A, 13, D, 13, "IA2")

        b.inv_a(2, bsrc2)
        b.inv_bc(2, lambda comp: b.dram[f"VA1rec{comp}"])

        def bsrc1(comp, X, Y):
            D = b.dram[f"B1{comp}1{X}{Y}"].rearrange("a b c -> a (b c)")
            if X == 0 and Y == 0:
                A = b.dram[f"VA1rec{comp}"].rearrange("a b c -> a (b c)")
            else:
                A = b.dram[f"B1{comp}0{X}{Y}"].rearrange("a b c -> a (b c)")
            return (A, 19, D, 19, "IA1")

        b.inv_a(1, bsrc1)
        b.inv_bc(1, lambda comp: outs[comp])

    nc.compile()
    return nc


_CACHE = {}


def make_in_maps(x_real, x_imag):
    x_real = np.ascontiguousarray(x_real, dtype=np.float32)
    x_imag = np.ascontiguousarray(x_imag, dtype=np.float32)
    in_maps = []
    for c in range(NCORE):
        m = host_matrices(c)
        slab_lo = 32 * c - 6
        im = {}
        for comp, x in (("re", x_real), ("im", x_imag)):
            s = np.zeros((44, 256, 256), dtype=np.float32)
            g0, g1 = max(0, slab_lo), min(256, slab_lo + 44)
            s[g0 - slab_lo:g1 - slab_lo] = x[g0:g1]
            im[f"xs_{comp}"] = s
        im.update(m)
        in_maps.append(im)
    return in_maps


def kernel(x_real, x_imag, alpha):
    thresh = 1e-3 * float(np.asarray(alpha))
    if thresh not in _CACHE:
        _CACHE[thresh] = build_program(thresh)
    nc = _CACHE[thresh]

    in_maps = make_in_maps(x_real, x_imag)
    res = run_bass_kernel_spmd(nc, in_maps, core_ids=list(range(NCORE)))
    out = np.empty((256, 256, 256), dtype=np.complex64)
    for c in range(NCORE):
        r = res.results[c]
        out[32 * c:32 * c + 32] = r["out_re"] + 1j * r["out_im"]
    return out



# revision 91
# speedup vs baseline: 1.6045x; 1.1291x over previous
"""Trainium2 Bass kernel for nn_L1Wav: 5-level 3D db4 wavelet soft-threshold
denoising of a 256^3 complex volume, SPMD over 8 NeuronCores.

Math notes (verified against the jax reference in a numpy sim):
  - The deterministic rng(1000) shift is 0 and the unit-modulus phase cancels
    through the prox (DWT is real-linear; |phase*w| = |w|), so the computation
    is exactly: 5-level 3D DWT -> complex soft-threshold -> inverse DWT.
  - Every 1D DWT/IDWT pass is a matmul against a banded filter matrix.
  - Sharding: volume split along axis 0 (32 planes/core). All a-axis passes
    use per-core weight-matrix slices, so the core-dependence lives entirely
    in host-provided matrices and one SPMD program serves all cores.
    Levels 1-2 are distributed; levels 3-5 are replicated on every core.
    The only communication is two small AllGathers of approx bands.

Level sizes: 256 -> 131 -> 69 -> 38 -> 22 -> 14.
Per-core windows: L1 band rows [16c,16c+19); L2 band rows [8c,8c+13);
output rows [32c,32c+32); input slab rows [32c-6,32c+38) zero-padded.

Layout: a volume at any level is stored (p, q, r). The forward a-pass
contracts p; the per-row bc-pass transforms q then r, emitting tiles
(r', q'), so child band tensors are stored (a_row, r', q').
"""
import sys
from contextlib import ExitStack

import numpy as np

sys.path.insert(0, "/opt/trn_rl_repo")

import concourse.bass as bass
import concourse.mybir as mybir
import concourse.tile as tile
from concourse import bacc
from concourse.bass_utils import run_bass_kernel_spmd
from concourse.masks import make_identity

DT = mybir.dt.float32
R32 = mybir.dt.float32r
F = 8
DEC_LO = np.array([-0.010597401784997278, 0.032883011666982945, 0.030841381835986965,
                   -0.18703481171888114, -0.02798376941698385, 0.6308807679295904,
                   0.7148465705525415, 0.23037781330885523])
REC_LO = DEC_LO[::-1].copy()
REC_HI = np.array([((-1) ** n) * DEC_LO[n] for n in range(F)])
DEC_HI = REC_HI[::-1].copy()

NS = [256, 131, 69, 38, 22, 14]     # sizes level 0..5
NCORE = 8
COMPS = ("re", "im")
BC_BS = {1: 1, 2: 3, 3: 6, 4: 11, 5: 14}       # fwd bc row batch
IBC_BS = {1: 3, 2: 5, 3: 7, 4: 13, 5: 14}      # inv bc row batch
INV_OUT_ROWS = {1: 32, 2: 19, 3: 69, 4: 38, 5: 22}


def W_mat(N, flt):
    L = (N + F - 1) // 2
    W = np.zeros((L, N), dtype=np.float32)
    for l in range(L):
        for j in range(F):
            n = 2 * l + 1 - j
            if 0 <= n < N:
                W[l, n] = flt[j]
    return W


def G_mat(L, crop, flt):
    G = np.zeros((crop, L), dtype=np.float32)
    for t in range(crop):
        for m in range(L):
            j = t + 6 - 2 * m
            if 0 <= j < F:
                G[t, m] = flt[j]
    return G


def pad_even(a):
    """Pad the last (M) dim to an even count (fp32r stationary rule)."""
    if a.shape[-1] % 2:
        a = np.pad(a, [(0, 0)] * (a.ndim - 1) + [(0, 1)])
    return a


def host_matrices(core):
    """All weight matrices for one core (lhsT layout: (K, M))."""
    c = core
    m = {}
    for l in range(5):
        W2 = np.concatenate([W_mat(NS[l], DEC_LO), W_mat(NS[l], DEC_HI)], 0)
        m[f"WT{l + 1}"] = np.ascontiguousarray(W2.T)
        glo = G_mat(NS[l + 1], NS[l], REC_LO)
        ghi = G_mat(NS[l + 1], NS[l], REC_HI)
        m[f"IAB{l + 1}"] = pad_even(np.ascontiguousarray(
            np.concatenate([glo.T, ghi.T], 0)))
    # L1 fwd a-pass (per-core): A1 (38, 44) -> lhsT (44, 38)
    A1 = np.zeros((38, 44), dtype=np.float32)
    slab_lo = 32 * c - 6
    for half, flt in ((0, DEC_LO), (1, DEC_HI)):
        for i in range(19):
            l = 16 * c + i
            for k in range(44):
                n = slab_lo + k
                j = 2 * l + 1 - n
                if 0 <= j < F and 0 <= n < 256:
                    A1[half * 19 + i, k] = flt[j]
    m["A1T"] = np.ascontiguousarray(A1.T)
    # L2 fwd a-pass, merged with the replicated full-lo pass, contracting
    # directly over ag1_out rows (38k+19ci+i = VA1full row 16k+i, owner
    # k = min(row//16, 7)).  M = 26 per-core band rows + 69 full-lo rows.
    A2 = np.concatenate([W_mat(131, DEC_LO)[8 * c:8 * c + 13],
                         W_mat(131, DEC_HI)[8 * c:8 * c + 13]], 0)
    M95 = np.concatenate([A2, W_mat(131, DEC_LO)], 0)       # (95, 131)
    for ci in range(2):
        A2WT = np.zeros((304, 96), dtype=np.float32)
        for r in range(131):
            k = min(r // 16, 7)
            A2WT[38 * k + 19 * ci + (r - 16 * k), :95] = M95[:, r]
        m["A2WTre" if ci == 0 else "A2WTim"] = A2WT
    # L1 inv a-pass: core-independent (38, 32)
    G1a = np.zeros((32, 19), dtype=np.float32)
    G1d = np.zeros((32, 19), dtype=np.float32)
    for u in range(32):
        for v in range(19):
            j = u + 6 - 2 * v
            if 0 <= j < F:
                G1a[u, v] = REC_LO[j]
                G1d[u, v] = REC_HI[j]
    m["IA1"] = np.ascontiguousarray(np.concatenate([G1a.T, G1d.T], 0))
    # L2 inv a-pass (per-core)
    glo1 = G_mat(69, 131, REC_LO)
    ghi1 = G_mat(69, 131, REC_HI)
    g2a_full = glo1[16 * c:16 * c + 19, :]                    # (19, 69)
    g2a13 = glo1[16 * c:16 * c + 19, 8 * c:8 * c + 13]
    g2d13 = ghi1[16 * c:16 * c + 19, 8 * c:8 * c + 13]
    m["IA2"] = pad_even(np.ascontiguousarray(
        np.concatenate([g2a13.T, g2d13.T], 0)))
    m["IA2LL"] = pad_even(np.ascontiguousarray(
        np.concatenate([g2a_full.T, g2d13.T], 0)))
    return {k: v.astype(np.float32) for k, v in m.items()}


MAT_SHAPES = {k: v.shape for k, v in host_matrices(0).items()}
# partition-chunk splits for SBUF-resident matrices (K dim)
MAT_SPLITS = {
    "IAB1": [(0, 128), (128, 3), (131, 128), (259, 3)],
    "IAB2": [(0, 69), (69, 69)],
    "IAB3": [(0, 38), (38, 38)],
    "IAB4": [(0, 22), (22, 22)],
    "IAB5": [(0, 14), (14, 14)],
}


def chunks_of(total, size=128):
    return [(i, min(size, total - i)) for i in range(0, total, size)]


class Builder:
    def __init__(self, nc, tc, ctx, thresh):
        self.nc = nc
        self.tc = tc
        self.thresh = float(thresh)
        self.p_dram = ctx.enter_context(
            tc.tile_pool(name="dram", bufs=1, space=bass.MemorySpace.DRAM))
        self.p_wts = ctx.enter_context(tc.tile_pool(name="wts", bufs=1))
        self.p_work = ctx.enter_context(tc.tile_pool(name="work", bufs=1))
        self.p_psum = ctx.enter_context(
            tc.tile_pool(name="psum", bufs=1, space=bass.MemorySpace.PSUM))
        self.mats = {}
        self.dram = {}
        self.uid = 0
        self._dmai = 0

    def _id(self):
        self.uid += 1
        return self.uid

    def dram_tile(self, name, shape, addr_space="Local"):
        t = self.p_dram.tile(list(shape), DT, name=name, tag=name,
                             addr_space=addr_space)
        self.dram[name] = t
        return t

    def sbuf(self, shape, tag, bufs=1, dt=DT):
        return self.p_work.tile(list(shape), dt, name=f"t{self._id()}",
                                tag=tag, bufs=bufs)

    def psum(self, shape, tag):
        return self.p_psum.tile(list(shape), DT, name=f"p{self._id()}",
                                tag=tag, bufs=1)

    # fp32r matmul: 1 cycle/row (vs 4 for fp32) when moving free size >= 256.
    # Operand tiles are declared float32r (bit-identical to fp32); the BIR
    # verifier requires producers to carry the fp32r dtype, so loads bitcast
    # the DRAM AP and psum->sbuf copies write fp32r-typed tiles.
    def mm(self, out, lhsT, rhs, **kw):
        self.nc.tensor.matmul(out, lhsT, rhs, **kw)

    # DMA triggers: each dma_start costs ~630ns of serialized queue time.
    # (Round-robin over SP+Activation HWDGE queues corrupted early deep-level
    # reads — cross-queue write->read ordering was not honored — so all
    # triggers stay on the SP queue.)
    def _dmaq(self):
        self._dmai += 1
        return (self.nc.sync, self.nc.scalar)[self._dmai & 1]

    def dma(self, dst, src):
        self._dmaq().dma_start(dst, src)

    def dmaR(self, dst, src):
        self._dmaq().dma_start(dst, src.bitcast(R32))

    def load_mat(self, name, dram_ap, splits=None):
        K, M = dram_ap.shape
        if splits is None:
            splits = MAT_SPLITS.get(name, chunks_of(K))
        tiles = []
        for (k0, kn) in splits:
            t = self.p_wts.tile([kn, M], R32, name=f"{name}_{k0}",
                                tag=f"{name}_{k0}", bufs=1)
            self.dmaR(t[:, :], dram_ap[k0:k0 + kn, :])
            tiles.append((t, k0, kn))
        self.mats[name] = tiles

    # ---- soft threshold: returns thresholded (re, im) tiles (full-shape)
    def soft_pair(self, s_re, s_im, shape, gb):
        nc = self.nc
        t = self.thresh
        mn = shape[0]
        tmp1 = self.sbuf(shape, "sm1")
        tmp2 = self.sbuf(shape, "sm2")
        a = tmp1[:, :gb, :]
        m = tmp2[:, :gb, :]
        nc.vector.tensor_mul(a, s_re, s_re)
        nc.vector.tensor_mul(m, s_im, s_im)
        nc.vector.tensor_add(a, a, m)
        nc.scalar.activation(m, a, mybir.ActivationFunctionType.Sqrt,
                             bias=self.bias_eps[:mn, :])
        nc.vector.tensor_scalar(a, m, -t, 0.0,
                                mybir.AluOpType.add, mybir.AluOpType.max)
        nc.vector.reciprocal(m, m)
        nc.vector.tensor_mul(a, a, m)
        th_re = self.sbuf(shape, "str", bufs=2)
        th_im = self.sbuf(shape, "sti", bufs=2)
        nc.vector.tensor_mul(th_re[:, :gb, :], s_re, a)
        nc.vector.tensor_mul(th_im[:, :gb, :], s_im, a)
        return th_re, th_im

    # ---- forward a-pass: out (M, n, n) = lhsT^T @ in (K, n, n)
    # Supertiled: one DMA load/store per `lds` columns, matmuls per `ntile`.
    def fwd_a(self, lname, in_keys, out_keys, M, n, ntile=512, lds=2048):
        nc = self.nc
        lhsT = self.mats[lname]
        for comp in COMPS:
            srcf = self.dram[in_keys[comp]].rearrange("a b c -> a (b c)")
            dstf = self.dram[out_keys[comp]].rearrange("a b c -> a (b c)")
            tot = n * n
            for t0 in range(0, tot, lds):
                ls = min(lds, tot - t0)
                rts = []
                for i, (lt, k0, kn) in enumerate(lhsT):
                    rt = self.sbuf([kn, lds], f"fa_in_{i}", bufs=2, dt=R32)
                    self.dmaR(rt[:, :ls], srcf[k0:k0 + kn, t0:t0 + ls])
                    rts.append(rt)
                s = self.sbuf([M, lds], "fa_o", bufs=2)
                for ui, u0 in enumerate(range(0, ls, ntile)):
                    tn = min(ntile, ls - u0)
                    tp = tn + (tn & 1)  # fp32r needs even psum width
                    p = self.psum([M, ntile], f"P{ui & 1}")
                    for i, (lt, k0, kn) in enumerate(lhsT):
                        self.mm(p[:, :tp], lt[:, :], rts[i][:, u0:u0 + tp],
                                start=(i == 0), stop=(i == len(lhsT) - 1))
                    nc.vector.tensor_copy(s[:, u0:u0 + tn], p[:, :tn])
                self.dma(dstf[:, t0:t0 + ls], s[:, :ls])

    # ---- merged L2 a-pass: contract ag1_out rows directly; outputs both
    # the per-core 26 band rows and the replicated 69 full-lo rows.
    def fwd_a2_merged(self, ag_out, ntile=512, lds=1024):
        nc = self.nc
        lhs = {c: self.mats[f"A2WT{c}"] for c in COMPS}
        src = ag_out.rearrange("a b c -> a (b c)")
        dsts = {c: self.dram[f"Af2C{c}"].rearrange("a b c -> a (b c)")
                for c in COMPS}
        tot = 131 * 131
        for t0 in range(0, tot, lds):
            ls = min(lds, tot - t0)
            rts = []
            for i, (lt, k0, kn) in enumerate(lhs["re"]):
                rt = self.sbuf([kn, lds], f"fa2_in_{i}", bufs=2, dt=R32)
                self.dmaR(rt[:, :ls], src[k0:k0 + kn, t0:t0 + ls])
                rts.append(rt)
            for pi, comp in enumerate(COMPS):
                lT = lhs[comp]
                s = self.sbuf([95, lds], f"fa2_o_{comp}", bufs=2)
                for ui, u0 in enumerate(range(0, ls, ntile)):
                    tn = min(ntile, ls - u0)
                    tp = tn + (tn & 1)
                    p = self.psum([96, ntile], f"P{2 * pi + (ui & 1)}")
                    for i, (lt, k0, kn) in enumerate(lT):
                        self.mm(p[:, :tp], lt[:, :], rts[i][:, u0:u0 + tp],
                                start=(i == 0), stop=(i == len(lT) - 1))
                    nc.vector.tensor_copy(s[:, u0:u0 + tn], p[0:95, :tn])
                self.dma(dsts[comp][:, t0:t0 + ls], s[:, :ls])

    # ---- forward bc-pass for one level
    def bc_fwd(self, lvl, rows, band_dest):
        nc = self.nc
        bs = BC_BS[lvl]
        Q = NS[lvl - 1]
        Qp = Q + (Q & 1)                # fp32r even-width padding
        L = NS[lvl]
        twoL = 2 * L
        WT = self.mats[f"WT{lvl}"]
        qch = chunks_of(Q)
        mch = chunks_of(twoL)
        half = rows // 2
        for af, g0 in [(a, g) for a in (0, 1) for g in range(0, half, bs)]:
            gb = min(bs, half - g0)
            ga = af * half + g0         # absolute a-row of this batch
            S3 = {}
            for comp in COMPS:
                src = self.dram[f"Af{lvl}{comp}"]
                ins = []
                for qi, (q0, qn) in enumerate(qch):
                    it = self.sbuf([qn, bs, Qp], f"bci_{qi}", bufs=2, dt=R32)
                    sap = src[ga:ga + gb, q0:q0 + qn, :].rearrange(
                        "b q n -> q b n")
                    self.dmaR(it[:, :gb, :Q], sap)
                    ins.append(it)
                # M1: transform q -> (twoL chunks, gb, Q)
                s1 = []
                for mi, (m0, mn) in enumerate(mch):
                    p = self.psum([mn, bs, Qp], f"P{mi}")
                    for ki in range(len(qch)):
                        self.mm(p[:, :gb, :],
                                WT[ki][0][:, m0:m0 + mn],
                                ins[ki][:, :gb, :],
                                start=(ki == 0),
                                stop=(ki == len(qch) - 1))
                    s = self.sbuf([mn, bs, Q], f"bs1_{mi}")
                    nc.vector.tensor_copy(s[:, :gb, :], p[:, :gb, :Q])
                    s1.append(s)
                # transpose -> (Q chunks, gb, twoL)
                pT = [self.psum([fn, bs, twoL], f"P{3 + fi}")
                      for fi, (f0, fn) in enumerate(qch)]
                for b in range(gb):
                    for mi, (m0, mn) in enumerate(mch):
                        for fi, (f0, fn) in enumerate(qch):
                            nc.tensor.transpose(
                                pT[fi][0:fn, b, m0:m0 + mn],
                                s1[mi][:, b, f0:f0 + fn],
                                self.ident[:mn, :mn])
                s2 = []
                for fi, (f0, fn) in enumerate(qch):
                    s = self.sbuf([fn, bs, twoL], f"bs2_{fi}", dt=R32)
                    nc.scalar.copy(s[:, :gb, :], pT[fi][:, :gb, :])
                    s2.append(s)
                # M2: transform r -> (twoL chunks, gb, twoL)
                S3[comp] = []
                for mi, (m0, mn) in enumerate(mch):
                    p = self.psum([mn, bs, twoL], f"P{5 + mi}")
                    for ki in range(len(qch)):
                        self.mm(p[:, :gb, :],
                                WT[ki][0][:, m0:m0 + mn],
                                s2[ki][:, :gb, :],
                                start=(ki == 0),
                                stop=(ki == len(qch) - 1))
                    s = self.sbuf([mn, bs, twoL], f"bs3_{comp}_{mi}")
                    nc.vector.tensor_copy(s[:, :gb, :], p[:, :gb, :])
                    S3[comp].append(s)
            TH = {"re": [], "im": []}
            for mi, (m0, mn) in enumerate(mch):
                tr, ti = self.soft_pair(S3["re"][mi][:, :gb, :],
                                        S3["im"][mi][:, :gb, :],
                                        [mn, bs, twoL], gb)
                TH["re"].append(tr)
                TH["im"].append(ti)
            for comp in COMPS:
                for mi, (m0, mn) in enumerate(mch):
                    for X in (0, 1):
                        lo = max(m0, X * L)
                        hi = min(m0 + mn, (X + 1) * L)
                        if lo >= hi:
                            continue
                        rr0, h = lo - m0, hi - lo
                        rx0 = lo - X * L
                        for Y in (0, 1):
                            for dest, use_th in band_dest(
                                    comp, af, g0, gb, X, Y, rx0, h):
                                st = TH[comp][mi] if use_th else S3[comp][mi]
                                self.dma(
                                    dest.rearrange("b r q -> r b q"),
                                    st[rr0:rr0 + h, 0:gb,
                                       Y * L:(Y + 1) * L])

    # ---- replicated lo-lo-lo quadrant of L2 (full 69 rows) -> VA2full
    def bc_ll_l2(self):
        nc = self.nc
        bs = 3
        Q, L = 131, 69
        Qp, Lp = Q + 1, L + 1
        WT = self.mats["WT2"]
        qch = chunks_of(Q)
        for comp in COMPS:
            src = self.dram[f"Af2F{comp}"]
            dst = self.dram[f"VA2full{comp}"]
            for g0 in range(0, L, bs):
                gb = min(bs, L - g0)
                ins = []
                for qi, (q0, qn) in enumerate(qch):
                    it = self.sbuf([qn, bs, Qp], f"bci_{qi}", bufs=2, dt=R32)
                    sap = src[g0:g0 + gb, q0:q0 + qn, :].rearrange(
                        "b q n -> q b n")
                    self.dmaR(it[:, :gb, :Q], sap)
                    ins.append(it)
                p = self.psum([Lp, bs, Qp], "P0")
                for ki in range(len(qch)):
                    self.mm(p[:, :gb, :], WT[ki][0][:, 0:Lp],
                            ins[ki][:, :gb, :], start=(ki == 0),
                            stop=(ki == len(qch) - 1))
                s1 = self.sbuf([L, bs, Q], "bs1_0")
                nc.vector.tensor_copy(s1[:, :gb, :], p[0:L, :gb, :Q])
                pT = [self.psum([fn, bs, L], f"P{3 + fi}")
                      for fi, (f0, fn) in enumerate(qch)]
                for b in range(gb):
                    for fi, (f0, fn) in enumerate(qch):
                        nc.tensor.transpose(pT[fi][0:fn, b, 0:L],
                                            s1[:, b, f0:f0 + fn],
                                            self.ident[:L, :L])
                s2 = []
                for fi, (f0, fn) in enumerate(qch):
                    s = self.sbuf([fn, bs, Lp], f"bs2_{fi}", dt=R32)
                    nc.scalar.copy(s[:, :gb, :L], pT[fi][:, :gb, :])
                    s2.append(s)
                p2 = self.psum([Lp, bs, Lp], "P5")
                for ki in range(len(qch)):
                    self.mm(p2[:, :gb, :], WT[ki][0][:, 0:Lp],
                            s2[ki][:, :gb, :], start=(ki == 0),
                            stop=(ki == len(qch) - 1))
                s3 = self.sbuf([L, bs, L], "bs3_re_0")
                nc.vector.tensor_copy(s3[:, :gb, :], p2[0:L, :gb, :L])
                self.dma(dst[g0:g0 + gb, :, :].rearrange("b r q -> r b q"),
                         s3[:, :gb, :])

    # ---- inverse a-pass (A and D band loads into one stacked sbuf tile)
    def inv_a(self, lvl, band_src, ntile=512, lds=2048):
        nc = self.nc
        L = NS[lvl]
        M = INV_OUT_ROWS[lvl]
        Mp = M + (M & 1)
        tot = L * L
        for comp in COMPS:
            for X in (0, 1):
                for Y in (0, 1):
                    A_ap, KA, D_ap, KD, lname = band_src(comp, X, Y)
                    lt = self.mats[lname][0][0]
                    dst = self.dram[f"O{lvl}{comp}{X}{Y}"].rearrange(
                        "a b c -> a (b c)")
                    for t0 in range(0, tot, lds):
                        ls = min(lds, tot - t0)
                        rt = self.sbuf([KA + KD, lds], "ia_in", bufs=2,
                                       dt=R32)
                        self.dmaR(rt[0:KA, :ls], A_ap[:, t0:t0 + ls])
                        self.dmaR(rt[KA:KA + KD, :ls], D_ap[:, t0:t0 + ls])
                        s = self.sbuf([M, lds], "ia_o", bufs=2)
                        for ui, u0 in enumerate(range(0, ls, ntile)):
                            tn = min(ntile, ls - u0)
                            tp = tn + (tn & 1)
                            p = self.psum([Mp, ntile], f"P{6 + (ui & 1)}")
                            self.mm(p[:, :tp], lt[:, :], rt[:, u0:u0 + tp],
                                    start=True, stop=True)
                            nc.vector.tensor_copy(s[:, u0:u0 + tn],
                                                  p[0:M, :tn])
                        self.dma(dst[:, t0:t0 + ls], s[:, :ls])

    # ---- inverse bc-pass: O tensors (rows, L, L) -> parent rows (rows, P, P)
    def inv_bc(self, lvl, out_dest):
        nc = self.nc
        rows = INV_OUT_ROWS[lvl]
        bs = IBC_BS[lvl]
        L = NS[lvl]
        P = NS[lvl - 1]
        Lp = L + (L & 1)
        Pp = P + (P & 1)
        IAB = self.mats[f"IAB{lvl}"]
        lch = chunks_of(L)
        pch = chunks_of(Pp)             # even chunks (fp32r stationary rule)

        def iab_slice(half, l0, ln, m0, mn):
            r0 = half * L + l0
            for (t, k0, kn) in IAB:
                if k0 <= r0 and r0 + ln <= k0 + kn:
                    return t[r0 - k0:r0 - k0 + ln, m0:m0 + mn]
            raise AssertionError(f"IAB{lvl} chunk misaligned {half} {l0} {ln}")

        for comp in COMPS:
            dst = out_dest(comp)
            for g0 in range(0, rows, bs):
                gb = min(bs, rows - g0)
                ot = {}
                for X in (0, 1):
                    for Y in (0, 1):
                        src = self.dram[f"O{lvl}{comp}{X}{Y}"]
                        for li, (l0, ln) in enumerate(lch):
                            t = self.sbuf([ln, bs, Lp], f"ibi_{X}{Y}_{li}",
                                          dt=R32)
                            sap = src[g0:g0 + gb, l0:l0 + ln, :].rearrange(
                                "b l n -> l b n")
                            self.dmaR(t[:, :gb, :L], sap)
                            ot[(X, Y, li)] = t
                sU = {}
                for Y in (0, 1):
                    sU[Y] = []
                    for mi, (m0, mn) in enumerate(pch):
                        p = self.psum([mn, bs, Lp], f"P{mi}")
                        nkt = 2 * len(lch)
                        ki = 0
                        for X in (0, 1):
                            for li, (l0, ln) in enumerate(lch):
                                self.mm(
                                    p[:, :gb, :],
                                    iab_slice(X, l0, ln, m0, mn),
                                    ot[(X, Y, li)][:, :gb, :],
                                    start=(ki == 0), stop=(ki == nkt - 1))
                                ki += 1
                        s = self.sbuf([mn, bs, L], f"ibsu_{Y}_{mi}")
                        nc.vector.tensor_copy(s[:, :gb, :], p[:, :gb, :L])
                        sU[Y].append(s)
                sT = {}
                gsub = max(1, min(bs, 512 // Pp))  # b-rows per transpose psum
                for Y in (0, 1):
                    sT[Y] = [self.sbuf([ln, bs, Pp], f"ibst_{Y}_{li}", dt=R32)
                             for li, (l0, ln) in enumerate(lch)]
                    for b0 in range(0, gb, gsub):
                        bn = min(gsub, gb - b0)
                        pT = [self.psum([ln, gsub, Pp], f"P{2 + li}")
                              for li, (l0, ln) in enumerate(lch)]
                        for b in range(b0, b0 + bn):
                            for mi, (m0, mn) in enumerate(pch):
                                for li, (l0, ln) in enumerate(lch):
                                    nc.tensor.transpose(
                                        pT[li][0:ln, b - b0, m0:m0 + mn],
                                        sU[Y][mi][:, b, l0:l0 + ln],
                                        self.ident[:mn, :mn])
                        for li, (l0, ln) in enumerate(lch):
                            nc.scalar.copy(sT[Y][li][:, b0:b0 + bn, :],
                                           pT[li][:, :bn, :])
                # final matmul, column-chunked (even widths, one psum bank)
                nfch = max(1, -(-(bs * Pp * 4) // 2048))
                fw = -(-Pp // nfch)
                fw += fw & 1
                fch = chunks_of(Pp, fw)
                for mi, (m0, mn) in enumerate(pch):
                    s = self.sbuf([mn, bs, Pp], f"ibs3_{mi}", bufs=2)
                    for fi, (f0, fn) in enumerate(fch):
                        p = self.psum([mn, bs, fn], f"P{(6, 4)[fi] + mi}")
                        nkt = 2 * len(lch)
                        ki = 0
                        for Y in (0, 1):
                            for li, (l0, ln) in enumerate(lch):
                                self.mm(
                                    p[:, :gb, :],
                                    iab_slice(Y, l0, ln, m0, mn),
                                    sT[Y][li][:, :gb, f0:f0 + fn],
                                    start=(ki == 0), stop=(ki == nkt - 1))
                                ki += 1
                        nc.vector.tensor_copy(s[:, :gb, f0:f0 + fn],
                                              p[:, :gb, :])
                    rmn = min(mn, P - m0)
                    self.dma(dst[g0:g0 + gb, m0:m0 + rmn, :].rearrange(
                        "b m q -> m b q"), s[0:rmn, :gb, :P])


def build_program(thresh, use_collective=(True, True), debug_dump=False):
    if isinstance(use_collective, bool):
        use_collective = (use_collective, use_collective)
    nc = bacc.Bacc("TRN2", target_bir_lowering=False, debug=False,
                   num_devices=NCORE)
    ext = {}
    for comp in COMPS:
        ext[f"xs_{comp}"] = nc.dram_tensor(f"xs_{comp}", [44, 256, 256], DT,
                                           kind="ExternalInput").ap()
    for name, shp in MAT_SHAPES.items():
        ext[name] = nc.dram_tensor(name, list(shp), DT,
                                   kind="ExternalInput").ap()
    outs = {}
    for comp in COMPS:
        outs[comp] = nc.dram_tensor(f"out_{comp}", [32, 256, 256], DT,
                                    kind="ExternalOutput").ap()

    with tile.TileContext(nc) as tc, ExitStack() as ctx:
        b = Builder(nc, tc, ctx, thresh)

        ident = b.p_wts.tile([128, 128], DT, name="ident", tag="ident")
        make_identity(nc, ident[:, :])
        b.ident = ident
        bias_eps = b.p_wts.tile([128, 1], DT, name="bias_eps", tag="bias_eps")
        nc.gpsimd.memset(bias_eps[:, :], 1e-38)
        b.bias_eps = bias_eps

        for name in MAT_SHAPES:
            b.load_mat(name, ext[name])
        for lvl in (3, 4, 5):
            b.load_mat(f"IABF{lvl}", ext[f"IAB{lvl}"],
                       splits=[(0, 2 * NS[lvl])])

        for comp in COMPS:
            b.dram[f"xs{comp}"] = ext[f"xs_{comp}"]
            b.dram_tile(f"Af1{comp}", (38, 256, 256))
            af2c = b.dram_tile(f"Af2C{comp}", (95, 131, 131))
            b.dram[f"Af2{comp}"] = af2c[0:26]
            b.dram[f"Af2F{comp}"] = af2c[26:95]
            b.dram_tile(f"Af3{comp}", (76, 69, 69))
            b.dram_tile(f"Af4{comp}", (44, 38, 38))
            b.dram_tile(f"Af5{comp}", (28, 22, 22))
            for af in (0, 1):
                for X in (0, 1):
                    for Y in (0, 1):
                        if af == 0 and X == 0 and Y == 0:
                            continue
                        for lvl, (rn, L) in {1: (19, 131), 2: (13, 69),
                                             3: (38, 38), 4: (22, 22),
                                             5: (14, 14)}.items():
                            b.dram_tile(f"B{lvl}{comp}{af}{X}{Y}", (rn, L, L))
            b.dram_tile(f"B5{comp}000", (14, 14, 14))
            b.dram_tile(f"VA3{comp}", (38, 38, 38))
            b.dram_tile(f"VA4{comp}", (22, 22, 22))
            b.dram_tile(f"VA2full{comp}", (69, 69, 69))
            b.dram_tile(f"VA1rec{comp}", (19, 131, 131))
            b.dram_tile(f"VA2rec{comp}", (69, 69, 69))
            b.dram_tile(f"VA3rec{comp}", (38, 38, 38))
            b.dram_tile(f"VA4rec{comp}", (22, 22, 22))
            for lvl, L in {1: 131, 2: 69, 3: 38, 4: 22, 5: 14}.items():
                for X in (0, 1):
                    for Y in (0, 1):
                        b.dram_tile(f"O{lvl}{comp}{X}{Y}",
                                    (INV_OUT_ROWS[lvl], L, L))
        ag1_in = b.dram_tile("ag1_in", (38, 131, 131))
        ag1_out = b.dram_tile("ag1_out", (NCORE * 38, 131, 131),
                              addr_space="Shared")

        # ============ forward ============
        b.fwd_a("A1T", {c: f"xs{c}" for c in COMPS},
                {c: f"Af1{c}" for c in COMPS}, 38, 256)

        def bd1(comp, af, b0, gb, X, Y, rx0, h):
            if af == 0 and X == 0 and Y == 0:
                ci = 0 if comp == "re" else 1
                return [(ag1_in[ci * 19 + b0:ci * 19 + b0 + gb,
                                rx0:rx0 + h, :], False)]
            return [(b.dram[f"B1{comp}{af}{X}{Y}"][b0:b0 + gb,
                                                   rx0:rx0 + h, :], True)]

        b.bc_fwd(1, 38, bd1)

        if use_collective[0]:
            nc.gpsimd.collective_compute(
                "AllGather", mybir.AluOpType.bypass,
                ins=[ag1_in.opt()], outs=[ag1_out.opt()],
                replica_groups=[list(range(NCORE))])
        else:
            nc.sync.dma_start(ag1_out[0:38], ag1_in[0:38])

        # merged L2 a-pass straight off ag1_out (no VA1full materialization)
        b.fwd_a2_merged(ag1_out)

        def bd2(comp, af, b0, gb, X, Y, rx0, h):
            if af == 0 and X == 0 and Y == 0:
                return []    # full aaa2 is recomputed replicated below
            return [(b.dram[f"B2{comp}{af}{X}{Y}"][b0:b0 + gb,
                                                   rx0:rx0 + h, :], True)]

        b.bc_fwd(2, 26, bd2)
        b.bc_ll_l2()

        def bd_rep(lvl, half_rows, va_name):
            def f(comp, af, b0, gb, X, Y, rx0, h):
                if af == 0 and X == 0 and Y == 0:
                    if lvl == 5:
                        return [(b.dram[f"B5{comp}000"][b0:b0 + gb,
                                                        rx0:rx0 + h, :],
                                 True)]
                    return [(b.dram[f"{va_name}{comp}"][b0:b0 + gb,
                                                        rx0:rx0 + h, :],
                             False)]
                return [(b.dram[f"B{lvl}{comp}{af}{X}{Y}"][b0:b0 + gb,
                                                           rx0:rx0 + h, :],
                         True)]
            return f

        b.fwd_a("WT3", {c: f"VA2full{c}" for c in COMPS},
                {c: f"Af3{c}" for c in COMPS}, 76, 69)
        b.bc_fwd(3, 76, bd_rep(3, 38, "VA3"))
        b.fwd_a("WT4", {c: f"VA3{c}" for c in COMPS},
                {c: f"Af4{c}" for c in COMPS}, 44, 38)
        b.bc_fwd(4, 44, bd_rep(4, 22, "VA4"))
        b.fwd_a("WT5", {c: f"VA4{c}" for c in COMPS},
                {c: f"Af5{c}" for c in COMPS}, 28, 22)
        b.bc_fwd(5, 28, bd_rep(5, 14, None))

        # ============ inverse ============
        def bsrc_rep(lvl, va_rec):
            L = NS[lvl]

            def f(comp, X, Y):
                if X == 0 and Y == 0:
                    A = (b.dram[f"B5{comp}000"] if lvl == 5
                         else b.dram[va_rec + comp])
                else:
                    A = b.dram[f"B{lvl}{comp}0{X}{Y}"]
                D = b.dram[f"B{lvl}{comp}1{X}{Y}"]
                return (A.rearrange("a b c -> a (b c)"), L,
                        D.rearrange("a b c -> a (b c)"), L, f"IABF{lvl}")
            return f

        b.inv_a(5, bsrc_rep(5, None))
        b.inv_bc(5, lambda comp: b.dram[f"VA4rec{comp}"])
        b.inv_a(4, bsrc_rep(4, "VA4rec"))
        b.inv_bc(4, lambda comp: b.dram[f"VA3rec{comp}"])
        b.inv_a(3, bsrc_rep(3, "VA3rec"))
        b.inv_bc(3, lambda comp: b.dram[f"VA2rec{comp}"])

        def bsrc2(comp, X, Y):
            D = b.dram[f"B2{comp}1{X}{Y}"].rearrange("a b c -> a (b c)")
            if X == 0 and Y == 0:
                A = b.dram[f"VA2rec{comp}"].rearrange("a b c -> a (b c)")
                return (A, 69, D, 13, "IA2LL")
            A = b.dram[f"B2{comp}0{X}{Y}"].rearrange("a b c -> a (b c)")
            return (A, 13, D, 13, "IA2")

        b.inv_a(2, bsrc2)
        b.inv_bc(2, lambda comp: b.dram[f"VA1rec{comp}"])

        def bsrc1(comp, X, Y):
            D = b.dram[f"B1{comp}1{X}{Y}"].rearrange("a b c -> a (b c)")
            if X == 0 and Y == 0:
                A = b.dram[f"VA1rec{comp}"].rearrange("a b c -> a (b c)")
            else:
                A = b.dram[f"B1{comp}0{X}{Y}"].rearrange("a b c -> a (b c)")
            return (A, 19, D, 19, "IA1")

        b.inv_a(1, bsrc1)
        b.inv_bc(1, lambda comp: outs[comp])

    nc.compile()
    return nc


_CACHE = {}


def make_in_maps(x_real, x_imag):
    x_real = np.ascontiguousarray(x_real, dtype=np.float32)
    x_imag = np.ascontiguousarray(x_imag, dtype=np.float32)
    in_maps = []
    for c in range(NCORE):
        m = host_matrices(c)
        slab_lo = 32 * c - 6
        im = {}
        for comp, x in (("re", x_real), ("im", x_imag)):
            s = np.zeros((44, 256, 256), dtype=np.float32)
            g0, g1 = max(0, slab_lo), min(256, slab_lo + 44)
            s[g0 - slab_lo:g1 - slab_lo] = x[g0:g1]
            im[f"xs_{comp}"] = s
        im.update(m)
        in_maps.append(im)
    return in_maps


def kernel(x_real, x_imag, alpha):
    thresh = 1e-3 * float(np.asarray(alpha))
    if thresh not in _CACHE:
        _CACHE[thresh] = build_program(thresh)
    nc = _CACHE[thresh]

    in_maps = make_in_maps(x_real, x_imag)
    res = run_bass_kernel_spmd(nc, in_maps, core_ids=list(range(NCORE)))
    out = np.empty((256, 256, 256), dtype=np.complex64)
    for c in range(NCORE):
        r = res.results[c]
        out[32 * c:32 * c + 32] = r["out_re"] + 1j * r["out_im"]
    return out



# revision 93
# speedup vs baseline: 1.6159x; 1.0071x over previous
"""Trainium2 Bass kernel for nn_L1Wav: 5-level 3D db4 wavelet soft-threshold
denoising of a 256^3 complex volume, SPMD over 8 NeuronCores.

Math notes (verified against the jax reference in a numpy sim):
  - The deterministic rng(1000) shift is 0 and the unit-modulus phase cancels
    through the prox (DWT is real-linear; |phase*w| = |w|), so the computation
    is exactly: 5-level 3D DWT -> complex soft-threshold -> inverse DWT.
  - Every 1D DWT/IDWT pass is a matmul against a banded filter matrix.
  - Sharding: volume split along axis 0 (32 planes/core). All a-axis passes
    use per-core weight-matrix slices, so the core-dependence lives entirely
    in host-provided matrices and one SPMD program serves all cores.
    Levels 1-2 are distributed; levels 3-5 are replicated on every core.
    The only communication is two small AllGathers of approx bands.

Level sizes: 256 -> 131 -> 69 -> 38 -> 22 -> 14.
Per-core windows: L1 band rows [16c,16c+19); L2 band rows [8c,8c+13);
output rows [32c,32c+32); input slab rows [32c-6,32c+38) zero-padded.

Layout: a volume at any level is stored (p, q, r). The forward a-pass
contracts p; the per-row bc-pass transforms q then r, emitting tiles
(r', q'), so child band tensors are stored (a_row, r', q').
"""
import sys
from contextlib import ExitStack

import numpy as np

sys.path.insert(0, "/opt/trn_rl_repo")

import concourse.bass as bass
import concourse.mybir as mybir
import concourse.tile as tile
from concourse import bacc
from concourse.bass_utils import run_bass_kernel_spmd
from concourse.masks import make_identity

DT = mybir.dt.float32
R32 = mybir.dt.float32r
F = 8
DEC_LO = np.array([-0.010597401784997278, 0.032883011666982945, 0.030841381835986965,
                   -0.18703481171888114, -0.02798376941698385, 0.6308807679295904,
                   0.7148465705525415, 0.23037781330885523])
REC_LO = DEC_LO[::-1].copy()
REC_HI = np.array([((-1) ** n) * DEC_LO[n] for n in range(F)])
DEC_HI = REC_HI[::-1].copy()

NS = [256, 131, 69, 38, 22, 14]     # sizes level 0..5
NCORE = 8
COMPS = ("re", "im")
BC_BS = {1: 2, 2: 3, 3: 6, 4: 11, 5: 14}       # fwd bc row batch
IBC_BS = {1: 3, 2: 5, 3: 7, 4: 13, 5: 14}      # inv bc row batch
INV_OUT_ROWS = {1: 32, 2: 19, 3: 69, 4: 38, 5: 22}


def W_mat(N, flt):
    L = (N + F - 1) // 2
    W = np.zeros((L, N), dtype=np.float32)
    for l in range(L):
        for j in range(F):
            n = 2 * l + 1 - j
            if 0 <= n < N:
                W[l, n] = flt[j]
    return W


def G_mat(L, crop, flt):
    G = np.zeros((crop, L), dtype=np.float32)
    for t in range(crop):
        for m in range(L):
            j = t + 6 - 2 * m
            if 0 <= j < F:
                G[t, m] = flt[j]
    return G


def pad_even(a):
    """Pad the last (M) dim to an even count (fp32r stationary rule)."""
    if a.shape[-1] % 2:
        a = np.pad(a, [(0, 0)] * (a.ndim - 1) + [(0, 1)])
    return a


def host_matrices(core):
    """All weight matrices for one core (lhsT layout: (K, M))."""
    c = core
    m = {}
    for l in range(5):
        W2 = np.concatenate([W_mat(NS[l], DEC_LO), W_mat(NS[l], DEC_HI)], 0)
        m[f"WT{l + 1}"] = np.ascontiguousarray(W2.T)
        glo = G_mat(NS[l + 1], NS[l], REC_LO)
        ghi = G_mat(NS[l + 1], NS[l], REC_HI)
        m[f"IAB{l + 1}"] = pad_even(np.ascontiguousarray(
            np.concatenate([glo.T, ghi.T], 0)))
    # L1 fwd a-pass (per-core): A1 (38, 44) -> lhsT (44, 38)
    A1 = np.zeros((38, 44), dtype=np.float32)
    slab_lo = 32 * c - 6
    for half, flt in ((0, DEC_LO), (1, DEC_HI)):
        for i in range(19):
            l = 16 * c + i
            for k in range(44):
                n = slab_lo + k
                j = 2 * l + 1 - n
                if 0 <= j < F and 0 <= n < 256:
                    A1[half * 19 + i, k] = flt[j]
    m["A1T"] = np.ascontiguousarray(A1.T)
    # L2 fwd a-pass, merged with the replicated full-lo pass, contracting
    # directly over ag1_out rows (38k+19ci+i = VA1full row 16k+i, owner
    # k = min(row//16, 7)).  M = 26 per-core band rows + 69 full-lo rows.
    A2 = np.concatenate([W_mat(131, DEC_LO)[8 * c:8 * c + 13],
                         W_mat(131, DEC_HI)[8 * c:8 * c + 13]], 0)
    M95 = np.concatenate([A2, W_mat(131, DEC_LO)], 0)       # (95, 131)
    for ci in range(2):
        A2WT = np.zeros((304, 96), dtype=np.float32)
        for r in range(131):
            k = min(r // 16, 7)
            A2WT[38 * k + 19 * ci + (r - 16 * k), :95] = M95[:, r]
        m["A2WTre" if ci == 0 else "A2WTim"] = A2WT
    # L1 inv a-pass: core-independent (38, 32)
    G1a = np.zeros((32, 19), dtype=np.float32)
    G1d = np.zeros((32, 19), dtype=np.float32)
    for u in range(32):
        for v in range(19):
            j = u + 6 - 2 * v
            if 0 <= j < F:
                G1a[u, v] = REC_LO[j]
                G1d[u, v] = REC_HI[j]
    m["IA1"] = np.ascontiguousarray(np.concatenate([G1a.T, G1d.T], 0))
    # L2 inv a-pass (per-core)
    glo1 = G_mat(69, 131, REC_LO)
    ghi1 = G_mat(69, 131, REC_HI)
    g2a_full = glo1[16 * c:16 * c + 19, :]                    # (19, 69)
    g2a13 = glo1[16 * c:16 * c + 19, 8 * c:8 * c + 13]
    g2d13 = ghi1[16 * c:16 * c + 19, 8 * c:8 * c + 13]
    m["IA2"] = pad_even(np.ascontiguousarray(
        np.concatenate([g2a13.T, g2d13.T], 0)))
    m["IA2LL"] = pad_even(np.ascontiguousarray(
        np.concatenate([g2a_full.T, g2d13.T], 0)))
    return {k: v.astype(np.float32) for k, v in m.items()}


MAT_SHAPES = {k: v.shape for k, v in host_matrices(0).items()}
# partition-chunk splits for SBUF-resident matrices (K dim)
MAT_SPLITS = {
    "IAB1": [(0, 128), (128, 3), (131, 128), (259, 3)],
    "IAB2": [(0, 69), (69, 69)],
    "IAB3": [(0, 38), (38, 38)],
    "IAB4": [(0, 22), (22, 22)],
    "IAB5": [(0, 14), (14, 14)],
}


def chunks_of(total, size=128):
    return [(i, min(size, total - i)) for i in range(0, total, size)]


class Builder:
    def __init__(self, nc, tc, ctx, thresh):
        self.nc = nc
        self.tc = tc
        self.thresh = float(thresh)
        self.p_dram = ctx.enter_context(
            tc.tile_pool(name="dram", bufs=1, space=bass.MemorySpace.DRAM))
        self.p_wts = ctx.enter_context(tc.tile_pool(name="wts", bufs=1))
        self.p_work = ctx.enter_context(tc.tile_pool(name="work", bufs=1))
        self.p_psum = ctx.enter_context(
            tc.tile_pool(name="psum", bufs=1, space=bass.MemorySpace.PSUM))
        self.mats = {}
        self.dram = {}
        self.uid = 0
        self._dmai = 0

    def _id(self):
        self.uid += 1
        return self.uid

    def dram_tile(self, name, shape, addr_space="Local"):
        t = self.p_dram.tile(list(shape), DT, name=name, tag=name,
                             addr_space=addr_space)
        self.dram[name] = t
        return t

    def sbuf(self, shape, tag, bufs=1, dt=DT):
        return self.p_work.tile(list(shape), dt, name=f"t{self._id()}",
                                tag=tag, bufs=bufs)

    def psum(self, shape, tag):
        return self.p_psum.tile(list(shape), DT, name=f"p{self._id()}",
                                tag=tag, bufs=1)

    # fp32r matmul: 1 cycle/row (vs 4 for fp32) when moving free size >= 256.
    # Operand tiles are declared float32r (bit-identical to fp32); the BIR
    # verifier requires producers to carry the fp32r dtype, so loads bitcast
    # the DRAM AP and psum->sbuf copies write fp32r-typed tiles.
    def mm(self, out, lhsT, rhs, **kw):
        self.nc.tensor.matmul(out, lhsT, rhs, **kw)

    # DMA triggers: each dma_start costs ~630ns of serialized queue time.
    # (Round-robin over SP+Activation HWDGE queues corrupted early deep-level
    # reads — cross-queue write->read ordering was not honored — so all
    # triggers stay on the SP queue.)
    def _dmaq(self):
        self._dmai += 1
        return (self.nc.sync, self.nc.scalar)[self._dmai & 1]

    def dma(self, dst, src):
        self._dmaq().dma_start(dst, src)

    def dmaR(self, dst, src):
        self._dmaq().dma_start(dst, src.bitcast(R32))

    def load_mat(self, name, dram_ap, splits=None):
        K, M = dram_ap.shape
        if splits is None:
            splits = MAT_SPLITS.get(name, chunks_of(K))
        tiles = []
        for (k0, kn) in splits:
            t = self.p_wts.tile([kn, M], R32, name=f"{name}_{k0}",
                                tag=f"{name}_{k0}", bufs=1)
            self.dmaR(t[:, :], dram_ap[k0:k0 + kn, :])
            tiles.append((t, k0, kn))
        self.mats[name] = tiles

    # ---- soft threshold: returns thresholded (re, im) tiles (full-shape)
    def soft_pair(self, s_re, s_im, shape, gb):
        nc = self.nc
        t = self.thresh
        mn = shape[0]
        tmp1 = self.sbuf(shape, "sm1")
        tmp2 = self.sbuf(shape, "sm2")
        a = tmp1[:, :gb, :]
        m = tmp2[:, :gb, :]
        nc.vector.tensor_mul(a, s_re, s_re)
        nc.vector.tensor_mul(m, s_im, s_im)
        nc.vector.tensor_add(a, a, m)
        nc.scalar.activation(m, a, mybir.ActivationFunctionType.Sqrt,
                             bias=self.bias_eps[:mn, :])
        nc.vector.tensor_scalar(a, m, -t, 0.0,
                                mybir.AluOpType.add, mybir.AluOpType.max)
        nc.vector.reciprocal(m, m)
        nc.vector.tensor_mul(a, a, m)
        th_re = self.sbuf(shape, "str", bufs=2)
        th_im = self.sbuf(shape, "sti", bufs=2)
        nc.vector.tensor_mul(th_re[:, :gb, :], s_re, a)
        nc.vector.tensor_mul(th_im[:, :gb, :], s_im, a)
        return th_re, th_im

    # ---- forward a-pass: out (M, n, n) = lhsT^T @ in (K, n, n)
    # Supertiled: one DMA load/store per `lds` columns, matmuls per `ntile`.
    def fwd_a(self, lname, in_keys, out_keys, M, n, ntile=512, lds=2048):
        nc = self.nc
        lhsT = self.mats[lname]
        for comp in COMPS:
            srcf = self.dram[in_keys[comp]].rearrange("a b c -> a (b c)")
            dstf = self.dram[out_keys[comp]].rearrange("a b c -> a (b c)")
            tot = n * n
            for t0 in range(0, tot, lds):
                ls = min(lds, tot - t0)
                rts = []
                for i, (lt, k0, kn) in enumerate(lhsT):
                    rt = self.sbuf([kn, lds], f"fa_in_{i}", bufs=2, dt=R32)
                    self.dmaR(rt[:, :ls], srcf[k0:k0 + kn, t0:t0 + ls])
                    rts.append(rt)
                s = self.sbuf([M, lds], "fa_o", bufs=2)
                for ui, u0 in enumerate(range(0, ls, ntile)):
                    tn = min(ntile, ls - u0)
                    tp = tn + (tn & 1)  # fp32r needs even psum width
                    p = self.psum([M, ntile], f"P{ui & 1}")
                    for i, (lt, k0, kn) in enumerate(lhsT):
                        self.mm(p[:, :tp], lt[:, :], rts[i][:, u0:u0 + tp],
                                start=(i == 0), stop=(i == len(lhsT) - 1))
                    nc.vector.tensor_copy(s[:, u0:u0 + tn], p[:, :tn])
                self.dma(dstf[:, t0:t0 + ls], s[:, :ls])

    # ---- merged L2 a-pass: contract ag1_out rows directly; outputs both
    # the per-core 26 band rows and the replicated 69 full-lo rows.
    def fwd_a2_merged(self, ag_out, ntile=512, lds=1024):
        nc = self.nc
        lhs = {c: self.mats[f"A2WT{c}"] for c in COMPS}
        src = ag_out.rearrange("a b c -> a (b c)")
        dsts = {c: self.dram[f"Af2C{c}"].rearrange("a b c -> a (b c)")
                for c in COMPS}
        tot = 131 * 131
        for t0 in range(0, tot, lds):
            ls = min(lds, tot - t0)
            rts = []
            for i, (lt, k0, kn) in enumerate(lhs["re"]):
                rt = self.sbuf([kn, lds], f"fa2_in_{i}", bufs=2, dt=R32)
                self.dmaR(rt[:, :ls], src[k0:k0 + kn, t0:t0 + ls])
                rts.append(rt)
            for pi, comp in enumerate(COMPS):
                lT = lhs[comp]
                s = self.sbuf([95, lds], f"fa2_o_{comp}", bufs=2)
                for ui, u0 in enumerate(range(0, ls, ntile)):
                    tn = min(ntile, ls - u0)
                    tp = tn + (tn & 1)
                    p = self.psum([96, ntile], f"P{2 * pi + (ui & 1)}")
                    for i, (lt, k0, kn) in enumerate(lT):
                        self.mm(p[:, :tp], lt[:, :], rts[i][:, u0:u0 + tp],
                                start=(i == 0), stop=(i == len(lT) - 1))
                    nc.vector.tensor_copy(s[:, u0:u0 + tn], p[0:95, :tn])
                self.dma(dsts[comp][:, t0:t0 + ls], s[:, :ls])

    # ---- forward bc-pass for one level
    def bc_fwd(self, lvl, rows, band_dest):
        nc = self.nc
        bs = BC_BS[lvl]
        Q = NS[lvl - 1]
        Qp = Q + (Q & 1)                # fp32r even-width padding
        L = NS[lvl]
        twoL = 2 * L
        WT = self.mats[f"WT{lvl}"]
        qch = chunks_of(Q)
        mch = chunks_of(twoL)
        half = rows // 2
        for af, g0 in [(a, g) for a in (0, 1) for g in range(0, half, bs)]:
            gb = min(bs, half - g0)
            ga = af * half + g0         # absolute a-row of this batch
            S3 = {}
            for comp in COMPS:
                src = self.dram[f"Af{lvl}{comp}"]
                ins = []
                for qi, (q0, qn) in enumerate(qch):
                    it = self.sbuf([qn, bs, Qp], f"bci_{qi}", bufs=2, dt=R32)
                    sap = src[ga:ga + gb, q0:q0 + qn, :].rearrange(
                        "b q n -> q b n")
                    self.dmaR(it[:, :gb, :Q], sap)
                    ins.append(it)
                # M1: transform q -> (twoL chunks, gb, Q)
                s1 = []
                for mi, (m0, mn) in enumerate(mch):
                    p = self.psum([mn, bs, Qp], f"P{mi}")
                    for ki in range(len(qch)):
                        self.mm(p[:, :gb, :],
                                WT[ki][0][:, m0:m0 + mn],
                                ins[ki][:, :gb, :],
                                start=(ki == 0),
                                stop=(ki == len(qch) - 1))
                    s = self.sbuf([mn, bs, Q], f"bs1_{mi}")
                    nc.vector.tensor_copy(s[:, :gb, :], p[:, :gb, :Q])
                    s1.append(s)
                # transpose -> (Q chunks, gb, twoL), b-subgrouped so each
                # psum tile fits one bank
                tsub = max(1, min(bs, 512 // twoL))
                s2 = [self.sbuf([fn, bs, twoL], f"bs2_{fi}", dt=R32)
                      for fi, (f0, fn) in enumerate(qch)]
                for b0 in range(0, gb, tsub):
                    bn = min(tsub, gb - b0)
                    pT = [self.psum([fn, tsub, twoL], f"P{3 + fi}")
                          for fi, (f0, fn) in enumerate(qch)]
                    for b in range(b0, b0 + bn):
                        for mi, (m0, mn) in enumerate(mch):
                            for fi, (f0, fn) in enumerate(qch):
                                nc.tensor.transpose(
                                    pT[fi][0:fn, b - b0, m0:m0 + mn],
                                    s1[mi][:, b, f0:f0 + fn],
                                    self.ident[:mn, :mn])
                    for fi, (f0, fn) in enumerate(qch):
                        nc.scalar.copy(s2[fi][:, b0:b0 + bn, :],
                                       pT[fi][:, :bn, :])
                # M2: transform r -> (twoL chunks, gb, twoL).  When the full
                # row batch exceeds one psum bank, split into two even-width
                # column windows (fp32r needs even psum widths).
                S3[comp] = []
                nw = 1 if bs * twoL * 4 <= 2048 else 2
                for mi, (m0, mn) in enumerate(mch):
                    s = self.sbuf([mn, bs, twoL], f"bs3_{comp}_{mi}")
                    if nw == 1:
                        p = self.psum([mn, bs, twoL], f"P{5 + mi}")
                        for ki in range(len(qch)):
                            self.mm(p[:, :gb, :],
                                    WT[ki][0][:, m0:m0 + mn],
                                    s2[ki][:, :gb, :],
                                    start=(ki == 0),
                                    stop=(ki == len(qch) - 1))
                        nc.vector.tensor_copy(s[:, :gb, :], p[:, :gb, :])
                    else:
                        Lw = L + (L & 1)
                        for c0, d0, d1 in ((0, 0, L), (twoL - Lw, L, twoL)):
                            p = self.psum([mn, bs, Lw], f"P{5 + mi}")
                            for ki in range(len(qch)):
                                self.mm(p[:, :gb, :],
                                        WT[ki][0][:, m0:m0 + mn],
                                        s2[ki][:, :gb, c0:c0 + Lw],
                                        start=(ki == 0),
                                        stop=(ki == len(qch) - 1))
                            nc.vector.tensor_copy(
                                s[:, :gb, d0:d1],
                                p[:, :gb, d0 - c0:d1 - c0])
                    S3[comp].append(s)
            TH = {"re": [], "im": []}
            for mi, (m0, mn) in enumerate(mch):
                tr, ti = self.soft_pair(S3["re"][mi][:, :gb, :],
                                        S3["im"][mi][:, :gb, :],
                                        [mn, bs, twoL], gb)
                TH["re"].append(tr)
                TH["im"].append(ti)
            for comp in COMPS:
                for mi, (m0, mn) in enumerate(mch):
                    for X in (0, 1):
                        lo = max(m0, X * L)
                        hi = min(m0 + mn, (X + 1) * L)
                        if lo >= hi:
                            continue
                        rr0, h = lo - m0, hi - lo
                        rx0 = lo - X * L
                        for Y in (0, 1):
                            for dest, use_th in band_dest(
                                    comp, af, g0, gb, X, Y, rx0, h):
                                st = TH[comp][mi] if use_th else S3[comp][mi]
                                self.dma(
                                    dest.rearrange("b r q -> r b q"),
                                    st[rr0:rr0 + h, 0:gb,
                                       Y * L:(Y + 1) * L])

    # ---- replicated lo-lo-lo quadrant of L2 (full 69 rows) -> VA2full
    def bc_ll_l2(self):
        nc = self.nc
        bs = 3
        Q, L = 131, 69
        Qp, Lp = Q + 1, L + 1
        WT = self.mats["WT2"]
        qch = chunks_of(Q)
        for comp in COMPS:
            src = self.dram[f"Af2F{comp}"]
            dst = self.dram[f"VA2full{comp}"]
            for g0 in range(0, L, bs):
                gb = min(bs, L - g0)
                ins = []
                for qi, (q0, qn) in enumerate(qch):
                    it = self.sbuf([qn, bs, Qp], f"bci_{qi}", bufs=2, dt=R32)
                    sap = src[g0:g0 + gb, q0:q0 + qn, :].rearrange(
                        "b q n -> q b n")
                    self.dmaR(it[:, :gb, :Q], sap)
                    ins.append(it)
                p = self.psum([Lp, bs, Qp], "P0")
                for ki in range(len(qch)):
                    self.mm(p[:, :gb, :], WT[ki][0][:, 0:Lp],
                            ins[ki][:, :gb, :], start=(ki == 0),
                            stop=(ki == len(qch) - 1))
                s1 = self.sbuf([L, bs, Q], "bs1_0")
                nc.vector.tensor_copy(s1[:, :gb, :], p[0:L, :gb, :Q])
                pT = [self.psum([fn, bs, L], f"P{3 + fi}")
                      for fi, (f0, fn) in enumerate(qch)]
                for b in range(gb):
                    for fi, (f0, fn) in enumerate(qch):
                        nc.tensor.transpose(pT[fi][0:fn, b, 0:L],
                                            s1[:, b, f0:f0 + fn],
                                            self.ident[:L, :L])
                s2 = []
                for fi, (f0, fn) in enumerate(qch):
                    s = self.sbuf([fn, bs, Lp], f"bs2_{fi}", dt=R32)
                    nc.scalar.copy(s[:, :gb, :L], pT[fi][:, :gb, :])
                    s2.append(s)
                p2 = self.psum([Lp, bs, Lp], "P5")
                for ki in range(len(qch)):
                    self.mm(p2[:, :gb, :], WT[ki][0][:, 0:Lp],
                            s2[ki][:, :gb, :], start=(ki == 0),
                            stop=(ki == len(qch) - 1))
                s3 = self.sbuf([L, bs, L], "bs3_re_0")
                nc.vector.tensor_copy(s3[:, :gb, :], p2[0:L, :gb, :L])
                self.dma(dst[g0:g0 + gb, :, :].rearrange("b r q -> r b q"),
                         s3[:, :gb, :])

    # ---- inverse a-pass (A and D band loads into one stacked sbuf tile)
    def inv_a(self, lvl, band_src, ntile=512, lds=2048):
        nc = self.nc
        L = NS[lvl]
        M = INV_OUT_ROWS[lvl]
        Mp = M + (M & 1)
        tot = L * L
        for comp in COMPS:
            for X in (0, 1):
                for Y in (0, 1):
                    A_ap, KA, D_ap, KD, lname = band_src(comp, X, Y)
                    lt = self.mats[lname][0][0]
                    dst = self.dram[f"O{lvl}{comp}{X}{Y}"].rearrange(
                        "a b c -> a (b c)")
                    for t0 in range(0, tot, lds):
                        ls = min(lds, tot - t0)
                        rt = self.sbuf([KA + KD, lds], "ia_in", bufs=2,
                                       dt=R32)
                        self.dmaR(rt[0:KA, :ls], A_ap[:, t0:t0 + ls])
                        self.dmaR(rt[KA:KA + KD, :ls], D_ap[:, t0:t0 + ls])
                        s = self.sbuf([M, lds], "ia_o", bufs=2)
                        for ui, u0 in enumerate(range(0, ls, ntile)):
                            tn = min(ntile, ls - u0)
                            tp = tn + (tn & 1)
                            p = self.psum([Mp, ntile], f"P{6 + (ui & 1)}")
                            self.mm(p[:, :tp], lt[:, :], rt[:, u0:u0 + tp],
                                    start=True, stop=True)
                            nc.vector.tensor_copy(s[:, u0:u0 + tn],
                                                  p[0:M, :tn])
                        self.dma(dst[:, t0:t0 + ls], s[:, :ls])

    # ---- inverse bc-pass: O tensors (rows, L, L) -> parent rows (rows, P, P)
    def inv_bc(self, lvl, out_dest):
        nc = self.nc
        rows = INV_OUT_ROWS[lvl]
        bs = IBC_BS[lvl]
        L = NS[lvl]
        P = NS[lvl - 1]
        Lp = L + (L & 1)
        Pp = P + (P & 1)
        IAB = self.mats[f"IAB{lvl}"]
        lch = chunks_of(L)
        pch = chunks_of(Pp)             # even chunks (fp32r stationary rule)

        def iab_slice(half, l0, ln, m0, mn):
            r0 = half * L + l0
            for (t, k0, kn) in IAB:
                if k0 <= r0 and r0 + ln <= k0 + kn:
                    return t[r0 - k0:r0 - k0 + ln, m0:m0 + mn]
            raise AssertionError(f"IAB{lvl} chunk misaligned {half} {l0} {ln}")

        for comp in COMPS:
            dst = out_dest(comp)
            for g0 in range(0, rows, bs):
                gb = min(bs, rows - g0)
                ot = {}
                for X in (0, 1):
                    for Y in (0, 1):
                        src = self.dram[f"O{lvl}{comp}{X}{Y}"]
                        for li, (l0, ln) in enumerate(lch):
                            t = self.sbuf([ln, bs, Lp], f"ibi_{X}{Y}_{li}",
                                          dt=R32)
                            sap = src[g0:g0 + gb, l0:l0 + ln, :].rearrange(
                                "b l n -> l b n")
                            self.dmaR(t[:, :gb, :L], sap)
                            ot[(X, Y, li)] = t
                sU = {}
                for Y in (0, 1):
                    sU[Y] = []
                    for mi, (m0, mn) in enumerate(pch):
                        p = self.psum([mn, bs, Lp], f"P{mi}")
                        nkt = 2 * len(lch)
                        ki = 0
                        for X in (0, 1):
                            for li, (l0, ln) in enumerate(lch):
                                self.mm(
                                    p[:, :gb, :],
                                    iab_slice(X, l0, ln, m0, mn),
                                    ot[(X, Y, li)][:, :gb, :],
                                    start=(ki == 0), stop=(ki == nkt - 1))
                                ki += 1
                        s = self.sbuf([mn, bs, L], f"ibsu_{Y}_{mi}")
                        nc.vector.tensor_copy(s[:, :gb, :], p[:, :gb, :L])
                        sU[Y].append(s)
                sT = {}
                gsub = max(1, min(bs, 512 // Pp))  # b-rows per transpose psum
                for Y in (0, 1):
                    sT[Y] = [self.sbuf([ln, bs, Pp], f"ibst_{Y}_{li}", dt=R32)
                             for li, (l0, ln) in enumerate(lch)]
                    for b0 in range(0, gb, gsub):
                        bn = min(gsub, gb - b0)
                        pT = [self.psum([ln, gsub, Pp], f"P{2 + li}")
                              for li, (l0, ln) in enumerate(lch)]
                        for b in range(b0, b0 + bn):
                            for mi, (m0, mn) in enumerate(pch):
                                for li, (l0, ln) in enumerate(lch):
                                    nc.tensor.transpose(
                                        pT[li][0:ln, b - b0, m0:m0 + mn],
                                        sU[Y][mi][:, b, l0:l0 + ln],
                                        self.ident[:mn, :mn])
                        for li, (l0, ln) in enumerate(lch):
                            nc.scalar.copy(sT[Y][li][:, b0:b0 + bn, :],
                                           pT[li][:, :bn, :])
                # final matmul, column-chunked (even widths, one psum bank)
                nfch = max(1, -(-(bs * Pp * 4) // 2048))
                fw = -(-Pp // nfch)
                fw += fw & 1
                fch = chunks_of(Pp, fw)
                for mi, (m0, mn) in enumerate(pch):
                    s = self.sbuf([mn, bs, Pp], f"ibs3_{mi}", bufs=2)
                    for fi, (f0, fn) in enumerate(fch):
                        p = self.psum([mn, bs, fn], f"P{(6, 4)[fi] + mi}")
                        nkt = 2 * len(lch)
                        ki = 0
                        for Y in (0, 1):
                            for li, (l0, ln) in enumerate(lch):
                                self.mm(
                                    p[:, :gb, :],
                                    iab_slice(Y, l0, ln, m0, mn),
                                    sT[Y][li][:, :gb, f0:f0 + fn],
                                    start=(ki == 0), stop=(ki == nkt - 1))
                                ki += 1
                        nc.vector.tensor_copy(s[:, :gb, f0:f0 + fn],
                                              p[:, :gb, :])
                    rmn = min(mn, P - m0)
                    self.dma(dst[g0:g0 + gb, m0:m0 + rmn, :].rearrange(
                        "b m q -> m b q"), s[0:rmn, :gb, :P])


def build_program(thresh, use_collective=(True, True), debug_dump=False):
    if isinstance(use_collective, bool):
        use_collective = (use_collective, use_collective)
    nc = bacc.Bacc("TRN2", target_bir_lowering=False, debug=False,
                   num_devices=NCORE)
    ext = {}
    for comp in COMPS:
        ext[f"xs_{comp}"] = nc.dram_tensor(f"xs_{comp}", [44, 256, 256], DT,
                                           kind="ExternalInput").ap()
    for name, shp in MAT_SHAPES.items():
        ext[name] = nc.dram_tensor(name, list(shp), DT,
                                   kind="ExternalInput").ap()
    outs = {}
    for comp in COMPS:
        outs[comp] = nc.dram_tensor(f"out_{comp}", [32, 256, 256], DT,
                                    kind="ExternalOutput").ap()

    with tile.TileContext(nc) as tc, ExitStack() as ctx:
        b = Builder(nc, tc, ctx, thresh)

        ident = b.p_wts.tile([128, 128], DT, name="ident", tag="ident")
        make_identity(nc, ident[:, :])
        b.ident = ident
        bias_eps = b.p_wts.tile([128, 1], DT, name="bias_eps", tag="bias_eps")
        nc.gpsimd.memset(bias_eps[:, :], 1e-38)
        b.bias_eps = bias_eps

        for name in MAT_SHAPES:
            b.load_mat(name, ext[name])
        for lvl in (3, 4, 5):
            b.load_mat(f"IABF{lvl}", ext[f"IAB{lvl}"],
                       splits=[(0, 2 * NS[lvl])])

        for comp in COMPS:
            b.dram[f"xs{comp}"] = ext[f"xs_{comp}"]
            b.dram_tile(f"Af1{comp}", (38, 256, 256))
            af2c = b.dram_tile(f"Af2C{comp}", (95, 131, 131))
            b.dram[f"Af2{comp}"] = af2c[0:26]
            b.dram[f"Af2F{comp}"] = af2c[26:95]
            b.dram_tile(f"Af3{comp}", (76, 69, 69))
            b.dram_tile(f"Af4{comp}", (44, 38, 38))
            b.dram_tile(f"Af5{comp}", (28, 22, 22))
            for af in (0, 1):
                for X in (0, 1):
                    for Y in (0, 1):
                        if af == 0 and X == 0 and Y == 0:
                            continue
                        for lvl, (rn, L) in {1: (19, 131), 2: (13, 69),
                                             3: (38, 38), 4: (22, 22),
                                             5: (14, 14)}.items():
                            b.dram_tile(f"B{lvl}{comp}{af}{X}{Y}", (rn, L, L))
            b.dram_tile(f"B5{comp}000", (14, 14, 14))
            b.dram_tile(f"VA3{comp}", (38, 38, 38))
            b.dram_tile(f"VA4{comp}", (22, 22, 22))
            b.dram_tile(f"VA2full{comp}", (69, 69, 69))
            b.dram_tile(f"VA1rec{comp}", (19, 131, 131))
            b.dram_tile(f"VA2rec{comp}", (69, 69, 69))
            b.dram_tile(f"VA3rec{comp}", (38, 38, 38))
            b.dram_tile(f"VA4rec{comp}", (22, 22, 22))
            for lvl, L in {1: 131, 2: 69, 3: 38, 4: 22, 5: 14}.items():
                for X in (0, 1):
                    for Y in (0, 1):
                        b.dram_tile(f"O{lvl}{comp}{X}{Y}",
                                    (INV_OUT_ROWS[lvl], L, L))
        ag1_in = b.dram_tile("ag1_in", (38, 131, 131))
        ag1_out = b.dram_tile("ag1_out", (NCORE * 38, 131, 131),
                              addr_space="Shared")

        # ============ forward ============
        b.fwd_a("A1T", {c: f"xs{c}" for c in COMPS},
                {c: f"Af1{c}" for c in COMPS}, 38, 256)

        def bd1(comp, af, b0, gb, X, Y, rx0, h):
            if af == 0 and X == 0 and Y == 0:
                ci = 0 if comp == "re" else 1
                return [(ag1_in[ci * 19 + b0:ci * 19 + b0 + gb,
                                rx0:rx0 + h, :], False)]
            return [(b.dram[f"B1{comp}{af}{X}{Y}"][b0:b0 + gb,
                                                   rx0:rx0 + h, :], True)]

        b.bc_fwd(1, 38, bd1)

        if use_collective[0]:
            nc.gpsimd.collective_compute(
                "AllGather", mybir.AluOpType.bypass,
                ins=[ag1_in.opt()], outs=[ag1_out.opt()],
                replica_groups=[list(range(NCORE))])
        else:
            nc.sync.dma_start(ag1_out[0:38], ag1_in[0:38])

        # merged L2 a-pass straight off ag1_out (no VA1full materialization)
        b.fwd_a2_merged(ag1_out)

        def bd2(comp, af, b0, gb, X, Y, rx0, h):
            if af == 0 and X == 0 and Y == 0:
                return []    # full aaa2 is recomputed replicated below
            return [(b.dram[f"B2{comp}{af}{X}{Y}"][b0:b0 + gb,
                                                   rx0:rx0 + h, :], True)]

        b.bc_fwd(2, 26, bd2)
        b.bc_ll_l2()

        def bd_rep(lvl, half_rows, va_name):
            def f(comp, af, b0, gb, X, Y, rx0, h):
                if af == 0 and X == 0 and Y == 0:
                    if lvl == 5:
                        return [(b.dram[f"B5{comp}000"][b0:b0 + gb,
                                                        rx0:rx0 + h, :],
                                 True)]
                    return [(b.dram[f"{va_name}{comp}"][b0:b0 + gb,
                                                        rx0:rx0 + h, :],
                             False)]
                return [(b.dram[f"B{lvl}{comp}{af}{X}{Y}"][b0:b0 + gb,
                                                           rx0:rx0 + h, :],
                         True)]
            return f

        b.fwd_a("WT3", {c: f"VA2full{c}" for c in COMPS},
                {c: f"Af3{c}" for c in COMPS}, 76, 69)
        b.bc_fwd(3, 76, bd_rep(3, 38, "VA3"))
        b.fwd_a("WT4", {c: f"VA3{c}" for c in COMPS},
                {c: f"Af4{c}" for c in COMPS}, 44, 38)
        b.bc_fwd(4, 44, bd_rep(4, 22, "VA4"))
        b.fwd_a("WT5", {c: f"VA4{c}" for c in COMPS},
                {c: f"Af5{c}" for c in COMPS}, 28, 22)
        b.bc_fwd(5, 28, bd_rep(5, 14, None))

        # ============ inverse ============
        def bsrc_rep(lvl, va_rec):
            L = NS[lvl]

            def f(comp, X, Y):
                if X == 0 and Y == 0:
                    A = (b.dram[f"B5{comp}000"] if lvl == 5
                         else b.dram[va_rec + comp])
                else:
                    A = b.dram[f"B{lvl}{comp}0{X}{Y}"]
                D = b.dram[f"B{lvl}{comp}1{X}{Y}"]
                return (A.rearrange("a b c -> a (b c)"), L,
                        D.rearrange("a b c -> a (b c)"), L, f"IABF{lvl}")
            return f

        b.inv_a(5, bsrc_rep(5, None))
        b.inv_bc(5, lambda comp: b.dram[f"VA4rec{comp}"])
        b.inv_a(4, bsrc_rep(4, "VA4rec"))
        b.inv_bc(4, lambda comp: b.dram[f"VA3rec{comp}"])
        b.inv_a(3, bsrc_rep(3, "VA3rec"))
        b.inv_bc(3, lambda comp: b.dram[f"VA2rec{comp}"])

        def bsrc2(comp, X, Y):
            D = b.dram[f"B2{comp}1{X}{Y}"].rearrange("a b c -> a (b c)")
            if X == 0 and Y == 0:
                A = b.dram[f"VA2rec{comp}"].rearrange("a b c -> a (b c)")
                return (A, 69, D, 13, "IA2LL")
            A = b.dram[f"B2{comp}0{X}{Y}"].rearrange("a b c -> a (b c)")
            return (A, 13, D, 13, "IA2")

        b.inv_a(2, bsrc2)
        b.inv_bc(2, lambda comp: b.dram[f"VA1rec{comp}"])

        def bsrc1(comp, X, Y):
            D = b.dram[f"B1{comp}1{X}{Y}"].rearrange("a b c -> a (b c)")
            if X == 0 and Y == 0:
                A = b.dram[f"VA1rec{comp}"].rearrange("a b c -> a (b c)")
            else:
                A = b.dram[f"B1{comp}0{X}{Y}"].rearrange("a b c -> a (b c)")
            return (A, 19, D, 19, "IA1")

        b.inv_a(1, bsrc1)
        b.inv_bc(1, lambda comp: outs[comp])

    nc.compile()
    return nc


_CACHE = {}


def make_in_maps(x_real, x_imag):
    x_real = np.ascontiguousarray(x_real, dtype=np.float32)
    x_imag = np.ascontiguousarray(x_imag, dtype=np.float32)
    in_maps = []
    for c in range(NCORE):
        m = host_matrices(c)
        slab_lo = 32 * c - 6
        im = {}
        for comp, x in (("re", x_real), ("im", x_imag)):
            s = np.zeros((44, 256, 256), dtype=np.float32)
            g0, g1 = max(0, slab_lo), min(256, slab_lo + 44)
            s[g0 - slab_lo:g1 - slab_lo] = x[g0:g1]
            im[f"xs_{comp}"] = s
        im.update(m)
        in_maps.append(im)
    return in_maps


def kernel(x_real, x_imag, alpha):
    thresh = 1e-3 * float(np.asarray(alpha))
    if thresh not in _CACHE:
        _CACHE[thresh] = build_program(thresh)
    nc = _CACHE[thresh]

    in_maps = make_in_maps(x_real, x_imag)
    res = run_bass_kernel_spmd(nc, in_maps, core_ids=list(range(NCORE)))
    out = np.empty((256, 256, 256), dtype=np.complex64)
    for c in range(NCORE):
        r = res.results[c]
        out[32 * c:32 * c + 32] = r["out_re"] + 1j * r["out_im"]
    return out



# revision 95
# speedup vs baseline: 1.6420x; 1.0162x over previous
"""Trainium2 Bass kernel for nn_L1Wav: 5-level 3D db4 wavelet soft-threshold
denoising of a 256^3 complex volume, SPMD over 8 NeuronCores.

Math notes (verified against the jax reference in a numpy sim):
  - The deterministic rng(1000) shift is 0 and the unit-modulus phase cancels
    through the prox (DWT is real-linear; |phase*w| = |w|), so the computation
    is exactly: 5-level 3D DWT -> complex soft-threshold -> inverse DWT.
  - Every 1D DWT/IDWT pass is a matmul against a banded filter matrix.
  - Sharding: volume split along axis 0 (32 planes/core). All a-axis passes
    use per-core weight-matrix slices, so the core-dependence lives entirely
    in host-provided matrices and one SPMD program serves all cores.
    Levels 1-2 are distributed; levels 3-5 are replicated on every core.
    The only communication is two small AllGathers of approx bands.

Level sizes: 256 -> 131 -> 69 -> 38 -> 22 -> 14.
Per-core windows: L1 band rows [16c,16c+19); L2 band rows [8c,8c+13);
output rows [32c,32c+32); input slab rows [32c-6,32c+38) zero-padded.

Layout: a volume at any level is stored (p, q, r). The forward a-pass
contracts p; the per-row bc-pass transforms q then r, emitting tiles
(r', q'), so child band tensors are stored (a_row, r', q').
"""
import sys
from contextlib import ExitStack

import numpy as np

sys.path.insert(0, "/opt/trn_rl_repo")

import concourse.bass as bass
import concourse.mybir as mybir
import concourse.tile as tile
from concourse import bacc
from concourse.bass_utils import run_bass_kernel_spmd
from concourse.masks import make_identity

DT = mybir.dt.float32
R32 = mybir.dt.float32r
F = 8
DEC_LO = np.array([-0.010597401784997278, 0.032883011666982945, 0.030841381835986965,
                   -0.18703481171888114, -0.02798376941698385, 0.6308807679295904,
                   0.7148465705525415, 0.23037781330885523])
REC_LO = DEC_LO[::-1].copy()
REC_HI = np.array([((-1) ** n) * DEC_LO[n] for n in range(F)])
DEC_HI = REC_HI[::-1].copy()

NS = [256, 131, 69, 38, 22, 14]     # sizes level 0..5
NCORE = 8
COMPS = ("re", "im")
BC_BS = {1: 2, 2: 3, 3: 6, 4: 11, 5: 14}       # fwd bc row batch
IBC_BS = {1: 3, 2: 5, 3: 7, 4: 13, 5: 14}      # inv bc row batch
INV_OUT_ROWS = {1: 32, 2: 19, 3: 69, 4: 38, 5: 22}


def W_mat(N, flt):
    L = (N + F - 1) // 2
    W = np.zeros((L, N), dtype=np.float32)
    for l in range(L):
        for j in range(F):
            n = 2 * l + 1 - j
            if 0 <= n < N:
                W[l, n] = flt[j]
    return W


def G_mat(L, crop, flt):
    G = np.zeros((crop, L), dtype=np.float32)
    for t in range(crop):
        for m in range(L):
            j = t + 6 - 2 * m
            if 0 <= j < F:
                G[t, m] = flt[j]
    return G


def pad_even(a):
    """Pad the last (M) dim to an even count (fp32r stationary rule)."""
    if a.shape[-1] % 2:
        a = np.pad(a, [(0, 0)] * (a.ndim - 1) + [(0, 1)])
    return a


def host_matrices(core):
    """All weight matrices for one core (lhsT layout: (K, M))."""
    c = core
    m = {}
    for l in range(5):
        W2 = np.concatenate([W_mat(NS[l], DEC_LO), W_mat(NS[l], DEC_HI)], 0)
        m[f"WT{l + 1}"] = np.ascontiguousarray(W2.T)
        glo = G_mat(NS[l + 1], NS[l], REC_LO)
        ghi = G_mat(NS[l + 1], NS[l], REC_HI)
        m[f"IAB{l + 1}"] = pad_even(np.ascontiguousarray(
            np.concatenate([glo.T, ghi.T], 0)))
    # L1 fwd a-pass (per-core): A1 (38, 44) -> lhsT (44, 38)
    A1 = np.zeros((38, 44), dtype=np.float32)
    slab_lo = 32 * c - 6
    for half, flt in ((0, DEC_LO), (1, DEC_HI)):
        for i in range(19):
            l = 16 * c + i
            for k in range(44):
                n = slab_lo + k
                j = 2 * l + 1 - n
                if 0 <= j < F and 0 <= n < 256:
                    A1[half * 19 + i, k] = flt[j]
    m["A1T"] = np.ascontiguousarray(A1.T)
    # L2 fwd a-pass, merged with the replicated full-lo pass, contracting
    # directly over ag1_out rows (38k+19ci+i = VA1full row 16k+i, owner
    # k = min(row//16, 7)).  M = 26 per-core band rows + 69 full-lo rows.
    A2 = np.concatenate([W_mat(131, DEC_LO)[8 * c:8 * c + 13],
                         W_mat(131, DEC_HI)[8 * c:8 * c + 13]], 0)
    M95 = np.concatenate([A2, W_mat(131, DEC_LO)], 0)       # (95, 131)
    for ci in range(2):
        A2WT = np.zeros((304, 96), dtype=np.float32)
        for r in range(131):
            k = min(r // 16, 7)
            A2WT[38 * k + 19 * ci + (r - 16 * k), :95] = M95[:, r]
        m["A2WTre" if ci == 0 else "A2WTim"] = A2WT
    # L1 inv a-pass: core-independent (38, 32)
    G1a = np.zeros((32, 19), dtype=np.float32)
    G1d = np.zeros((32, 19), dtype=np.float32)
    for u in range(32):
        for v in range(19):
            j = u + 6 - 2 * v
            if 0 <= j < F:
                G1a[u, v] = REC_LO[j]
                G1d[u, v] = REC_HI[j]
    m["IA1"] = np.ascontiguousarray(np.concatenate([G1a.T, G1d.T], 0))
    # L2 inv a-pass (per-core)
    glo1 = G_mat(69, 131, REC_LO)
    ghi1 = G_mat(69, 131, REC_HI)
    g2a_full = glo1[16 * c:16 * c + 19, :]                    # (19, 69)
    g2a13 = glo1[16 * c:16 * c + 19, 8 * c:8 * c + 13]
    g2d13 = ghi1[16 * c:16 * c + 19, 8 * c:8 * c + 13]
    m["IA2"] = pad_even(np.ascontiguousarray(
        np.concatenate([g2a13.T, g2d13.T], 0)))
    m["IA2LL"] = pad_even(np.ascontiguousarray(
        np.concatenate([g2a_full.T, g2d13.T], 0)))
    return {k: v.astype(np.float32) for k, v in m.items()}


MAT_SHAPES = {k: v.shape for k, v in host_matrices(0).items()}
# partition-chunk splits for SBUF-resident matrices (K dim)
MAT_SPLITS = {
    "IAB1": [(0, 128), (128, 3), (131, 128), (259, 3)],
    "IAB2": [(0, 69), (69, 69)],
    "IAB3": [(0, 38), (38, 38)],
    "IAB4": [(0, 22), (22, 22)],
    "IAB5": [(0, 14), (14, 14)],
}


def chunks_of(total, size=128):
    return [(i, min(size, total - i)) for i in range(0, total, size)]


class Builder:
    def __init__(self, nc, tc, ctx, thresh):
        self.nc = nc
        self.tc = tc
        self.thresh = float(thresh)
        self.p_dram = ctx.enter_context(
            tc.tile_pool(name="dram", bufs=1, space=bass.MemorySpace.DRAM))
        self.p_wts = ctx.enter_context(tc.tile_pool(name="wts", bufs=1))
        self.p_work = ctx.enter_context(tc.tile_pool(name="work", bufs=1))
        self.p_psum = ctx.enter_context(
            tc.tile_pool(name="psum", bufs=1, space=bass.MemorySpace.PSUM))
        self.mats = {}
        self.dram = {}
        self.uid = 0
        self._dmai = 0

    def _id(self):
        self.uid += 1
        return self.uid

    def dram_tile(self, name, shape, addr_space="Local"):
        t = self.p_dram.tile(list(shape), DT, name=name, tag=name,
                             addr_space=addr_space)
        self.dram[name] = t
        return t

    def sbuf(self, shape, tag, bufs=1, dt=DT):
        return self.p_work.tile(list(shape), dt, name=f"t{self._id()}",
                                tag=tag, bufs=bufs)

    def psum(self, shape, tag):
        return self.p_psum.tile(list(shape), DT, name=f"p{self._id()}",
                                tag=tag, bufs=1)

    # fp32r matmul: 1 cycle/row (vs 4 for fp32) when moving free size >= 256.
    # Operand tiles are declared float32r (bit-identical to fp32); the BIR
    # verifier requires producers to carry the fp32r dtype, so loads bitcast
    # the DRAM AP and psum->sbuf copies write fp32r-typed tiles.
    def mm(self, out, lhsT, rhs, **kw):
        self.nc.tensor.matmul(out, lhsT, rhs, **kw)

    # DMA triggers: each dma_start costs ~630ns of serialized queue time.
    # (Round-robin over SP+Activation HWDGE queues corrupted early deep-level
    # reads — cross-queue write->read ordering was not honored — so all
    # triggers stay on the SP queue.)
    def _dmaq(self):
        self._dmai += 1
        return (self.nc.sync, self.nc.scalar)[self._dmai & 1]

    def dma(self, dst, src):
        self._dmaq().dma_start(dst, src)

    def dmaR(self, dst, src):
        self._dmaq().dma_start(dst, src.bitcast(R32))

    def load_mat(self, name, dram_ap, splits=None):
        K, M = dram_ap.shape
        if splits is None:
            splits = MAT_SPLITS.get(name, chunks_of(K))
        tiles = []
        for (k0, kn) in splits:
            t = self.p_wts.tile([kn, M], R32, name=f"{name}_{k0}",
                                tag=f"{name}_{k0}", bufs=1)
            self.dmaR(t[:, :], dram_ap[k0:k0 + kn, :])
            tiles.append((t, k0, kn))
        self.mats[name] = tiles

    # ---- soft threshold: returns thresholded (re, im) tiles (full-shape)
    def soft_pair(self, s_re, s_im, shape, gb):
        nc = self.nc
        t = self.thresh
        mn = shape[0]
        tmp1 = self.sbuf(shape, "sm1")
        tmp2 = self.sbuf(shape, "sm2")
        a = tmp1[:, :gb, :]
        m = tmp2[:, :gb, :]
        nc.vector.tensor_mul(a, s_re, s_re)
        nc.vector.tensor_mul(m, s_im, s_im)
        nc.vector.tensor_add(a, a, m)
        nc.scalar.activation(m, a, mybir.ActivationFunctionType.Sqrt,
                             bias=self.bias_eps[:mn, :])
        nc.vector.tensor_scalar(a, m, -t, 0.0,
                                mybir.AluOpType.add, mybir.AluOpType.max)
        nc.vector.reciprocal(m, m)
        nc.vector.tensor_mul(a, a, m)
        th_re = self.sbuf(shape, "str", bufs=2)
        th_im = self.sbuf(shape, "sti", bufs=2)
        nc.vector.tensor_mul(th_re[:, :gb, :], s_re, a)
        nc.vector.tensor_mul(th_im[:, :gb, :], s_im, a)
        return th_re, th_im

    # ---- forward a-pass: out (M, n, n) = lhsT^T @ in (K, n, n)
    # Supertiled: one DMA load/store per `lds` columns, matmuls per `ntile`.
    def fwd_a(self, lname, in_keys, out_keys, M, n, ntile=512, lds=2048):
        nc = self.nc
        lhsT = self.mats[lname]
        for comp in COMPS:
            srcf = self.dram[in_keys[comp]].rearrange("a b c -> a (b c)")
            dstf = self.dram[out_keys[comp]].rearrange("a b c -> a (b c)")
            tot = n * n
            for t0 in range(0, tot, lds):
                ls = min(lds, tot - t0)
                rts = []
                for i, (lt, k0, kn) in enumerate(lhsT):
                    rt = self.sbuf([kn, lds], f"fa_in_{i}", bufs=2, dt=R32)
                    self.dmaR(rt[:, :ls], srcf[k0:k0 + kn, t0:t0 + ls])
                    rts.append(rt)
                s = self.sbuf([M, lds], "fa_o", bufs=2)
                for ui, u0 in enumerate(range(0, ls, ntile)):
                    tn = min(ntile, ls - u0)
                    tp = tn + (tn & 1)  # fp32r needs even psum width
                    p = self.psum([M, ntile], f"P{ui & 1}")
                    for i, (lt, k0, kn) in enumerate(lhsT):
                        self.mm(p[:, :tp], lt[:, :], rts[i][:, u0:u0 + tp],
                                start=(i == 0), stop=(i == len(lhsT) - 1))
                    nc.vector.tensor_copy(s[:, u0:u0 + tn], p[:, :tn])
                self.dma(dstf[:, t0:t0 + ls], s[:, :ls])

    # ---- merged L2 a-pass: contract ag1_out rows directly; outputs both
    # the per-core 26 band rows and the replicated 69 full-lo rows.
    def fwd_a2_merged(self, ag_out, ntile=512, lds=1024):
        nc = self.nc
        lhs = {c: self.mats[f"A2WT{c}"] for c in COMPS}
        src = ag_out.rearrange("a b c -> a (b c)")
        dsts = {c: self.dram[f"Af2C{c}"].rearrange("a b c -> a (b c)")
                for c in COMPS}
        tot = 131 * 131
        for t0 in range(0, tot, lds):
            ls = min(lds, tot - t0)
            rts = []
            for i, (lt, k0, kn) in enumerate(lhs["re"]):
                rt = self.sbuf([kn, lds], f"fa2_in_{i}", bufs=2, dt=R32)
                self.dmaR(rt[:, :ls], src[k0:k0 + kn, t0:t0 + ls])
                rts.append(rt)
            for pi, comp in enumerate(COMPS):
                lT = lhs[comp]
                s = self.sbuf([95, lds], f"fa2_o_{comp}", bufs=2)
                for ui, u0 in enumerate(range(0, ls, ntile)):
                    tn = min(ntile, ls - u0)
                    tp = tn + (tn & 1)
                    p = self.psum([96, ntile], f"P{2 * pi + (ui & 1)}")
                    for i, (lt, k0, kn) in enumerate(lT):
                        self.mm(p[:, :tp], lt[:, :], rts[i][:, u0:u0 + tp],
                                start=(i == 0), stop=(i == len(lT) - 1))
                    nc.vector.tensor_copy(s[:, u0:u0 + tn], p[0:95, :tn])
                self.dma(dsts[comp][:, t0:t0 + ls], s[:, :ls])

    # ---- forward bc-pass for one level
    def bc_fwd(self, lvl, rows, band_dest):
        nc = self.nc
        bs = BC_BS[lvl]
        Q = NS[lvl - 1]
        Qp = Q + (Q & 1)                # fp32r even-width padding
        L = NS[lvl]
        twoL = 2 * L
        WT = self.mats[f"WT{lvl}"]
        qch = chunks_of(Q)
        mch = chunks_of(twoL)
        half = rows // 2
        for af, g0 in [(a, g) for a in (0, 1) for g in range(0, half, bs)]:
            gb = min(bs, half - g0)
            ga = af * half + g0         # absolute a-row of this batch
            S3 = {}
            for comp in COMPS:
                src = self.dram[f"Af{lvl}{comp}"]
                ins = []
                for qi, (q0, qn) in enumerate(qch):
                    it = self.sbuf([qn, bs, Qp], f"bci_{qi}", bufs=2, dt=R32)
                    sap = src[ga:ga + gb, q0:q0 + qn, :].rearrange(
                        "b q n -> q b n")
                    self.dmaR(it[:, :gb, :Q], sap)
                    ins.append(it)
                # M1: transform q -> (twoL chunks, gb, Q)
                s1 = []
                for mi, (m0, mn) in enumerate(mch):
                    p = self.psum([mn, bs, Qp], f"P{mi}")
                    for ki in range(len(qch)):
                        self.mm(p[:, :gb, :],
                                WT[ki][0][:, m0:m0 + mn],
                                ins[ki][:, :gb, :],
                                start=(ki == 0),
                                stop=(ki == len(qch) - 1))
                    s = self.sbuf([mn, bs, Q], f"bs1_{mi}", bufs=2)
                    nc.vector.tensor_copy(s[:, :gb, :], p[:, :gb, :Q])
                    s1.append(s)
                # transpose -> (Q chunks, gb, twoL), b-subgrouped so each
                # psum tile fits one bank
                tsub = max(1, min(bs, 512 // twoL))
                s2 = [self.sbuf([fn, bs, twoL], f"bs2_{fi}", bufs=2, dt=R32)
                      for fi, (f0, fn) in enumerate(qch)]
                for b0 in range(0, gb, tsub):
                    bn = min(tsub, gb - b0)
                    pT = [self.psum([fn, tsub, twoL], f"P{3 + fi}")
                          for fi, (f0, fn) in enumerate(qch)]
                    for b in range(b0, b0 + bn):
                        for mi, (m0, mn) in enumerate(mch):
                            for fi, (f0, fn) in enumerate(qch):
                                nc.tensor.transpose(
                                    pT[fi][0:fn, b - b0, m0:m0 + mn],
                                    s1[mi][:, b, f0:f0 + fn],
                                    self.ident[:mn, :mn])
                    for fi, (f0, fn) in enumerate(qch):
                        nc.scalar.copy(s2[fi][:, b0:b0 + bn, :],
                                       pT[fi][:, :bn, :])
                # M2: transform r -> (twoL chunks, gb, twoL).  When the full
                # row batch exceeds one psum bank, split into two even-width
                # column windows (fp32r needs even psum widths).
                S3[comp] = []
                nw = 1 if bs * twoL * 4 <= 2048 else 2
                for mi, (m0, mn) in enumerate(mch):
                    s = self.sbuf([mn, bs, twoL], f"bs3_{comp}_{mi}")
                    if nw == 1:
                        p = self.psum([mn, bs, twoL], f"P{5 + mi}")
                        for ki in range(len(qch)):
                            self.mm(p[:, :gb, :],
                                    WT[ki][0][:, m0:m0 + mn],
                                    s2[ki][:, :gb, :],
                                    start=(ki == 0),
                                    stop=(ki == len(qch) - 1))
                        nc.vector.tensor_copy(s[:, :gb, :], p[:, :gb, :])
                    else:
                        Lw = L + (L & 1)
                        for c0, d0, d1 in ((0, 0, L), (twoL - Lw, L, twoL)):
                            p = self.psum([mn, bs, Lw], f"P{5 + mi}")
                            for ki in range(len(qch)):
                                self.mm(p[:, :gb, :],
                                        WT[ki][0][:, m0:m0 + mn],
                                        s2[ki][:, :gb, c0:c0 + Lw],
                                        start=(ki == 0),
                                        stop=(ki == len(qch) - 1))
                            nc.vector.tensor_copy(
                                s[:, :gb, d0:d1],
                                p[:, :gb, d0 - c0:d1 - c0])
                    S3[comp].append(s)
            TH = {"re": [], "im": []}
            for mi, (m0, mn) in enumerate(mch):
                tr, ti = self.soft_pair(S3["re"][mi][:, :gb, :],
                                        S3["im"][mi][:, :gb, :],
                                        [mn, bs, twoL], gb)
                TH["re"].append(tr)
                TH["im"].append(ti)
            for comp in COMPS:
                for mi, (m0, mn) in enumerate(mch):
                    for X in (0, 1):
                        lo = max(m0, X * L)
                        hi = min(m0 + mn, (X + 1) * L)
                        if lo >= hi:
                            continue
                        rr0, h = lo - m0, hi - lo
                        rx0 = lo - X * L
                        for Y in (0, 1):
                            for dest, use_th in band_dest(
                                    comp, af, g0, gb, X, Y, rx0, h):
                                st = TH[comp][mi] if use_th else S3[comp][mi]
                                self.dma(
                                    dest.rearrange("b r q -> r b q"),
                                    st[rr0:rr0 + h, 0:gb,
                                       Y * L:(Y + 1) * L])

    # ---- replicated lo-lo-lo quadrant of L2 (full 69 rows) -> VA2full
    def bc_ll_l2(self):
        nc = self.nc
        bs = 3
        Q, L = 131, 69
        Qp, Lp = Q + 1, L + 1
        WT = self.mats["WT2"]
        qch = chunks_of(Q)
        for comp in COMPS:
            src = self.dram[f"Af2F{comp}"]
            dst = self.dram[f"VA2full{comp}"]
            for g0 in range(0, L, bs):
                gb = min(bs, L - g0)
                ins = []
                for qi, (q0, qn) in enumerate(qch):
                    it = self.sbuf([qn, bs, Qp], f"bci_{qi}", bufs=2, dt=R32)
                    sap = src[g0:g0 + gb, q0:q0 + qn, :].rearrange(
                        "b q n -> q b n")
                    self.dmaR(it[:, :gb, :Q], sap)
                    ins.append(it)
                p = self.psum([Lp, bs, Qp], "P0")
                for ki in range(len(qch)):
                    self.mm(p[:, :gb, :], WT[ki][0][:, 0:Lp],
                            ins[ki][:, :gb, :], start=(ki == 0),
                            stop=(ki == len(qch) - 1))
                s1 = self.sbuf([L, bs, Q], "bs1_0", bufs=2)
                nc.vector.tensor_copy(s1[:, :gb, :], p[0:L, :gb, :Q])
                pT = [self.psum([fn, bs, L], f"P{3 + fi}")
                      for fi, (f0, fn) in enumerate(qch)]
                for b in range(gb):
                    for fi, (f0, fn) in enumerate(qch):
                        nc.tensor.transpose(pT[fi][0:fn, b, 0:L],
                                            s1[:, b, f0:f0 + fn],
                                            self.ident[:L, :L])
                s2 = []
                for fi, (f0, fn) in enumerate(qch):
                    s = self.sbuf([fn, bs, Lp], f"bs2_{fi}", bufs=2, dt=R32)
                    nc.scalar.copy(s[:, :gb, :L], pT[fi][:, :gb, :])
                    s2.append(s)
                p2 = self.psum([Lp, bs, Lp], "P5")
                for ki in range(len(qch)):
                    self.mm(p2[:, :gb, :], WT[ki][0][:, 0:Lp],
                            s2[ki][:, :gb, :], start=(ki == 0),
                            stop=(ki == len(qch) - 1))
                s3 = self.sbuf([L, bs, L], "bs3_re_0")
                nc.vector.tensor_copy(s3[:, :gb, :], p2[0:L, :gb, :L])
                self.dma(dst[g0:g0 + gb, :, :].rearrange("b r q -> r b q"),
                         s3[:, :gb, :])

    # ---- inverse a-pass (A and D band loads into one stacked sbuf tile)
    def inv_a(self, lvl, band_src, ntile=512, lds=1536):
        nc = self.nc
        L = NS[lvl]
        M = INV_OUT_ROWS[lvl]
        Mp = M + (M & 1)
        tot = L * L
        for comp in COMPS:
            for X in (0, 1):
                for Y in (0, 1):
                    A_ap, KA, D_ap, KD, lname = band_src(comp, X, Y)
                    lt = self.mats[lname][0][0]
                    dst = self.dram[f"O{lvl}{comp}{X}{Y}"].rearrange(
                        "a b c -> a (b c)")
                    for t0 in range(0, tot, lds):
                        ls = min(lds, tot - t0)
                        rt = self.sbuf([KA + KD, lds], "ia_in", bufs=2,
                                       dt=R32)
                        self.dmaR(rt[0:KA, :ls], A_ap[:, t0:t0 + ls])
                        self.dmaR(rt[KA:KA + KD, :ls], D_ap[:, t0:t0 + ls])
                        s = self.sbuf([M, lds], "ia_o", bufs=2)
                        for ui, u0 in enumerate(range(0, ls, ntile)):
                            tn = min(ntile, ls - u0)
                            tp = tn + (tn & 1)
                            p = self.psum([Mp, ntile], f"P{6 + (ui & 1)}")
                            self.mm(p[:, :tp], lt[:, :], rt[:, u0:u0 + tp],
                                    start=True, stop=True)
                            nc.vector.tensor_copy(s[:, u0:u0 + tn],
                                                  p[0:M, :tn])
                        self.dma(dst[:, t0:t0 + ls], s[:, :ls])

    # ---- inverse bc-pass: O tensors (rows, L, L) -> parent rows (rows, P, P)
    def inv_bc(self, lvl, out_dest):
        nc = self.nc
        rows = INV_OUT_ROWS[lvl]
        bs = IBC_BS[lvl]
        L = NS[lvl]
        P = NS[lvl - 1]
        Lp = L + (L & 1)
        Pp = P + (P & 1)
        IAB = self.mats[f"IAB{lvl}"]
        lch = chunks_of(L)
        pch = chunks_of(Pp)             # even chunks (fp32r stationary rule)

        def iab_slice(half, l0, ln, m0, mn):
            r0 = half * L + l0
            for (t, k0, kn) in IAB:
                if k0 <= r0 and r0 + ln <= k0 + kn:
                    return t[r0 - k0:r0 - k0 + ln, m0:m0 + mn]
            raise AssertionError(f"IAB{lvl} chunk misaligned {half} {l0} {ln}")

        for comp in COMPS:
            dst = out_dest(comp)
            for g0 in range(0, rows, bs):
                gb = min(bs, rows - g0)
                ot = {}
                for X in (0, 1):
                    for Y in (0, 1):
                        src = self.dram[f"O{lvl}{comp}{X}{Y}"]
                        for li, (l0, ln) in enumerate(lch):
                            t = self.sbuf([ln, bs, Lp], f"ibi_{X}{Y}_{li}",
                                          dt=R32)
                            sap = src[g0:g0 + gb, l0:l0 + ln, :].rearrange(
                                "b l n -> l b n")
                            self.dmaR(t[:, :gb, :L], sap)
                            ot[(X, Y, li)] = t
                sU = {}
                for Y in (0, 1):
                    sU[Y] = []
                    for mi, (m0, mn) in enumerate(pch):
                        p = self.psum([mn, bs, Lp], f"P{mi}")
                        nkt = 2 * len(lch)
                        ki = 0
                        for X in (0, 1):
                            for li, (l0, ln) in enumerate(lch):
                                self.mm(
                                    p[:, :gb, :],
                                    iab_slice(X, l0, ln, m0, mn),
                                    ot[(X, Y, li)][:, :gb, :],
                                    start=(ki == 0), stop=(ki == nkt - 1))
                                ki += 1
                        s = self.sbuf([mn, bs, L], f"ibsu_{Y}_{mi}")
                        nc.vector.tensor_copy(s[:, :gb, :], p[:, :gb, :L])
                        sU[Y].append(s)
                sT = {}
                gsub = max(1, min(bs, 512 // Pp))  # b-rows per transpose psum
                for Y in (0, 1):
                    sT[Y] = [self.sbuf([ln, bs, Pp], f"ibst_{Y}_{li}", dt=R32)
                             for li, (l0, ln) in enumerate(lch)]
                    for b0 in range(0, gb, gsub):
                        bn = min(gsub, gb - b0)
                        pT = [self.psum([ln, gsub, Pp], f"P{2 + li}")
                              for li, (l0, ln) in enumerate(lch)]
                        for b in range(b0, b0 + bn):
                            for mi, (m0, mn) in enumerate(pch):
                                for li, (l0, ln) in enumerate(lch):
                                    nc.tensor.transpose(
                                        pT[li][0:ln, b - b0, m0:m0 + mn],
                                        sU[Y][mi][:, b, l0:l0 + ln],
                                        self.ident[:mn, :mn])
                        for li, (l0, ln) in enumerate(lch):
                            nc.scalar.copy(sT[Y][li][:, b0:b0 + bn, :],
                                           pT[li][:, :bn, :])
                # final matmul, column-chunked (even widths, one psum bank)
                nfch = max(1, -(-(bs * Pp * 4) // 2048))
                fw = -(-Pp // nfch)
                fw += fw & 1
                fch = chunks_of(Pp, fw)
                for mi, (m0, mn) in enumerate(pch):
                    s = self.sbuf([mn, bs, Pp], f"ibs3_{mi}", bufs=2)
                    for fi, (f0, fn) in enumerate(fch):
                        p = self.psum([mn, bs, fn], f"P{(6, 4)[fi] + mi}")
                        nkt = 2 * len(lch)
                        ki = 0
                        for Y in (0, 1):
                            for li, (l0, ln) in enumerate(lch):
                                self.mm(
                                    p[:, :gb, :],
                                    iab_slice(Y, l0, ln, m0, mn),
                                    sT[Y][li][:, :gb, f0:f0 + fn],
                                    start=(ki == 0), stop=(ki == nkt - 1))
                                ki += 1
                        nc.vector.tensor_copy(s[:, :gb, f0:f0 + fn],
                                              p[:, :gb, :])
                    rmn = min(mn, P - m0)
                    self.dma(dst[g0:g0 + gb, m0:m0 + rmn, :].rearrange(
                        "b m q -> m b q"), s[0:rmn, :gb, :P])


def build_program(thresh, use_collective=(True, True), debug_dump=False):
    if isinstance(use_collective, bool):
        use_collective = (use_collective, use_collective)
    nc = bacc.Bacc("TRN2", target_bir_lowering=False, debug=False,
                   num_devices=NCORE)
    ext = {}
    for comp in COMPS:
        ext[f"xs_{comp}"] = nc.dram_tensor(f"xs_{comp}", [44, 256, 256], DT,
                                           kind="ExternalInput").ap()
    for name, shp in MAT_SHAPES.items():
        ext[name] = nc.dram_tensor(name, list(shp), DT,
                                   kind="ExternalInput").ap()
    outs = {}
    for comp in COMPS:
        outs[comp] = nc.dram_tensor(f"out_{comp}", [32, 256, 256], DT,
                                    kind="ExternalOutput").ap()

    with tile.TileContext(nc) as tc, ExitStack() as ctx:
        b = Builder(nc, tc, ctx, thresh)

        ident = b.p_wts.tile([128, 128], DT, name="ident", tag="ident")
        make_identity(nc, ident[:, :])
        b.ident = ident
        bias_eps = b.p_wts.tile([128, 1], DT, name="bias_eps", tag="bias_eps")
        nc.gpsimd.memset(bias_eps[:, :], 1e-38)
        b.bias_eps = bias_eps

        for name in MAT_SHAPES:
            b.load_mat(name, ext[name])
        for lvl in (3, 4, 5):
            b.load_mat(f"IABF{lvl}", ext[f"IAB{lvl}"],
                       splits=[(0, 2 * NS[lvl])])

        for comp in COMPS:
            b.dram[f"xs{comp}"] = ext[f"xs_{comp}"]
            b.dram_tile(f"Af1{comp}", (38, 256, 256))
            af2c = b.dram_tile(f"Af2C{comp}", (95, 131, 131))
            b.dram[f"Af2{comp}"] = af2c[0:26]
            b.dram[f"Af2F{comp}"] = af2c[26:95]
            b.dram_tile(f"Af3{comp}", (76, 69, 69))
            b.dram_tile(f"Af4{comp}", (44, 38, 38))
            b.dram_tile(f"Af5{comp}", (28, 22, 22))
            for af in (0, 1):
                for X in (0, 1):
                    for Y in (0, 1):
                        if af == 0 and X == 0 and Y == 0:
                            continue
                        for lvl, (rn, L) in {1: (19, 131), 2: (13, 69),
                                             3: (38, 38), 4: (22, 22),
                                             5: (14, 14)}.items():
                            b.dram_tile(f"B{lvl}{comp}{af}{X}{Y}", (rn, L, L))
            b.dram_tile(f"B5{comp}000", (14, 14, 14))
            b.dram_tile(f"VA3{comp}", (38, 38, 38))
            b.dram_tile(f"VA4{comp}", (22, 22, 22))
            b.dram_tile(f"VA2full{comp}", (69, 69, 69))
            b.dram_tile(f"VA1rec{comp}", (19, 131, 131))
            b.dram_tile(f"VA2rec{comp}", (69, 69, 69))
            b.dram_tile(f"VA3rec{comp}", (38, 38, 38))
            b.dram_tile(f"VA4rec{comp}", (22, 22, 22))
            for lvl, L in {1: 131, 2: 69, 3: 38, 4: 22, 5: 14}.items():
                for X in (0, 1):
                    for Y in (0, 1):
                        b.dram_tile(f"O{lvl}{comp}{X}{Y}",
                                    (INV_OUT_ROWS[lvl], L, L))
        ag1_in = b.dram_tile("ag1_in", (38, 131, 131))
        ag1_out = b.dram_tile("ag1_out", (NCORE * 38, 131, 131),
                              addr_space="Shared")

        # ============ forward ============
        b.fwd_a("A1T", {c: f"xs{c}" for c in COMPS},
                {c: f"Af1{c}" for c in COMPS}, 38, 256)

        def bd1(comp, af, b0, gb, X, Y, rx0, h):
            if af == 0 and X == 0 and Y == 0:
                ci = 0 if comp == "re" else 1
                return [(ag1_in[ci * 19 + b0:ci * 19 + b0 + gb,
                                rx0:rx0 + h, :], False)]
            return [(b.dram[f"B1{comp}{af}{X}{Y}"][b0:b0 + gb,
                                                   rx0:rx0 + h, :], True)]

        b.bc_fwd(1, 38, bd1)

        if use_collective[0]:
            nc.gpsimd.collective_compute(
                "AllGather", mybir.AluOpType.bypass,
                ins=[ag1_in.opt()], outs=[ag1_out.opt()],
                replica_groups=[list(range(NCORE))])
        else:
            nc.sync.dma_start(ag1_out[0:38], ag1_in[0:38])

        # merged L2 a-pass straight off ag1_out (no VA1full materialization)
        b.fwd_a2_merged(ag1_out)

        def bd2(comp, af, b0, gb, X, Y, rx0, h):
            if af == 0 and X == 0 and Y == 0:
                return []    # full aaa2 is recomputed replicated below
            return [(b.dram[f"B2{comp}{af}{X}{Y}"][b0:b0 + gb,
                                                   rx0:rx0 + h, :], True)]

        b.bc_fwd(2, 26, bd2)
        b.bc_ll_l2()

        def bd_rep(lvl, half_rows, va_name):
            def f(comp, af, b0, gb, X, Y, rx0, h):
                if af == 0 and X == 0 and Y == 0:
                    if lvl == 5:
                        return [(b.dram[f"B5{comp}000"][b0:b0 + gb,
                                                        rx0:rx0 + h, :],
                                 True)]
                    return [(b.dram[f"{va_name}{comp}"][b0:b0 + gb,
                                                        rx0:rx0 + h, :],
                             False)]
                return [(b.dram[f"B{lvl}{comp}{af}{X}{Y}"][b0:b0 + gb,
                                                           rx0:rx0 + h, :],
                         True)]
            return f

        b.fwd_a("WT3", {c: f"VA2full{c}" for c in COMPS},
                {c: f"Af3{c}" for c in COMPS}, 76, 69)
        b.bc_fwd(3, 76, bd_rep(3, 38, "VA3"))
        b.fwd_a("WT4", {c: f"VA3{c}" for c in COMPS},
                {c: f"Af4{c}" for c in COMPS}, 44, 38)
        b.bc_fwd(4, 44, bd_rep(4, 22, "VA4"))
        b.fwd_a("WT5", {c: f"VA4{c}" for c in COMPS},
                {c: f"Af5{c}" for c in COMPS}, 28, 22)
        b.bc_fwd(5, 28, bd_rep(5, 14, None))

        # ============ inverse ============
        def bsrc_rep(lvl, va_rec):
            L = NS[lvl]

            def f(comp, X, Y):
                if X == 0 and Y == 0:
                    A = (b.dram[f"B5{comp}000"] if lvl == 5
                         else b.dram[va_rec + comp])
                else:
                    A = b.dram[f"B{lvl}{comp}0{X}{Y}"]
                D = b.dram[f"B{lvl}{comp}1{X}{Y}"]
                return (A.rearrange("a b c -> a (b c)"), L,
                        D.rearrange("a b c -> a (b c)"), L, f"IABF{lvl}")
            return f

        b.inv_a(5, bsrc_rep(5, None))
        b.inv_bc(5, lambda comp: b.dram[f"VA4rec{comp}"])
        b.inv_a(4, bsrc_rep(4, "VA4rec"))
        b.inv_bc(4, lambda comp: b.dram[f"VA3rec{comp}"])
        b.inv_a(3, bsrc_rep(3, "VA3rec"))
        b.inv_bc(3, lambda comp: b.dram[f"VA2rec{comp}"])

        def bsrc2(comp, X, Y):
            D = b.dram[f"B2{comp}1{X}{Y}"].rearrange("a b c -> a (b c)")
            if X == 0 and Y == 0:
                A = b.dram[f"VA2rec{comp}"].rearrange("a b c -> a (b c)")
                return (A, 69, D, 13, "IA2LL")
            A = b.dram[f"B2{comp}0{X}{Y}"].rearrange("a b c -> a (b c)")
            return (A, 13, D, 13, "IA2")

        b.inv_a(2, bsrc2)
        b.inv_bc(2, lambda comp: b.dram[f"VA1rec{comp}"])

        def bsrc1(comp, X, Y):
            D = b.dram[f"B1{comp}1{X}{Y}"].rearrange("a b c -> a (b c)")
            if X == 0 and Y == 0:
                A = b.dram[f"VA1rec{comp}"].rearrange("a b c -> a (b c)")
            else:
                A = b.dram[f"B1{comp}0{X}{Y}"].rearrange("a b c -> a (b c)")
            return (A, 19, D, 19, "IA1")

        b.inv_a(1, bsrc1)
        b.inv_bc(1, lambda comp: outs[comp])

    nc.compile()
    return nc


_CACHE = {}


def make_in_maps(x_real, x_imag):
    x_real = np.ascontiguousarray(x_real, dtype=np.float32)
    x_imag = np.ascontiguousarray(x_imag, dtype=np.float32)
    in_maps = []
    for c in range(NCORE):
        m = host_matrices(c)
        slab_lo = 32 * c - 6
        im = {}
        for comp, x in (("re", x_real), ("im", x_imag)):
            s = np.zeros((44, 256, 256), dtype=np.float32)
            g0, g1 = max(0, slab_lo), min(256, slab_lo + 44)
            s[g0 - slab_lo:g1 - slab_lo] = x[g0:g1]
            im[f"xs_{comp}"] = s
        im.update(m)
        in_maps.append(im)
    return in_maps


def kernel(x_real, x_imag, alpha):
    thresh = 1e-3 * float(np.asarray(alpha))
    if thresh not in _CACHE:
        _CACHE[thresh] = build_program(thresh)
    nc = _CACHE[thresh]

    in_maps = make_in_maps(x_real, x_imag)
    res = run_bass_kernel_spmd(nc, in_maps, core_ids=list(range(NCORE)))
    out = np.empty((256, 256, 256), dtype=np.complex64)
    for c in range(NCORE):
        r = res.results[c]
        out[32 * c:32 * c + 32] = r["out_re"] + 1j * r["out_im"]
    return out

